# revision 1
# baseline (speedup 1.0000x reference)
"""Trainium2 Bass kernel for nn_BaselineModel_74509092651544 (CLRS-style MPNN).

Strategy
--------
Data-parallel over graphs: 32 graphs -> 8 cores x 4 graphs.  The dense
[B,N,N,H] message tensor of the reference is never materialized: only the
~61k unique (graph,src,dst) edge slots survive the masked max, so the
message MLP runs on a padded CSR slot layout (~8.5x less compute).

Per core, everything lives in SBUF feature-major [H=128, cols]:
  * node/edge embeddings via one-hot matmuls (host builds integer one-hots),
  * m1[src]+m2[dst]+edge_fts@We accumulated in PSUM via 3 chained matmuls
    (gather matrices G_src/G_dst are host-built 0/1 matrices, resident in
    SBUF, used as the moving operand),
  * the 2-layer message MLP as f32r matmuls at N=512 (full PE rate),
  * masked max over senders as DVE segmented reduces straight out of PSUM
    (receivers are relabeled per graph by in-degree so equal-K groups pack
    into 512-slot tiles; padding duplicates a real slot so no masking is
    needed),
  * LayerNorm in node-major layout via PE transposes,
  * graph pooling + prediction MLP on-device; output is [OUT, 4] per core.

All float math happens on device.  Host work is integer indexing /
relayout only.  Matmuls use float32r (full-rate fp32, ~1.7e-4 rel err).
"""

import sys
import numpy as np

sys.path.insert(0, "/opt/trn_rl_repo")

B, N, H, L, E, OUT = 32, 128, 128, 3, 65536, 128
M = 8                 # NeuronCores
BL = B // M           # graphs per core
NEG = -1e9
EPS = 1e-5
AV, BV = 128, 16

_CACHE = {}


# --------------------------------------------------------------------------
# Host preprocessing: pure integer / relayout work.
# --------------------------------------------------------------------------

def _prep(inputs):
    x = np.asarray(inputs["x"]).astype(np.int64)            # [B*N, 9]
    ea = np.asarray(inputs["edge_attr"]).astype(np.int64)   # [E, 3]
    ei = np.asarray(inputs["edge_index"]).astype(np.int64)  # [2, E]

    g = ei[0] // N
    s = ei[0] % N
    d = ei[1] % N
    key = (g * N + s) * N + d
    uniq, inv = np.unique(key, return_inverse=True)
    US = uniq.size
    ug = uniq // (N * N)
    us = (uniq // N) % N
    ud = uniq % N

    # bond one-hot counts per unique slot  [US, 48]
    oh48 = np.zeros((US, 48), np.float32)
    for c in range(3):
        np.add.at(oh48, (inv, ea[:, c] + 16 * c), 1.0)

    # unique in-degree per (graph, receiver)
    deg = np.zeros((B, N), np.int64)
    np.add.at(deg, (ug, ud), 1)

    # receiver relabeling: position p holds the p-th highest-degree receiver
    rho = np.argsort(-deg, axis=1, kind="stable")        # [B, N] pos -> orig
    rho_inv = np.argsort(rho, axis=1)                    # orig -> pos
    degS = -np.sort(-deg, axis=1)                        # [B, N] desc
    Kp = np.maximum(degS.max(axis=0), 1)                 # [N]

    # group schedule (shared by all graphs/cores): (p0, R, K)
    groups = []
    p = 0
    while p < N:
        K = int(Kp[p])
        if 16 * K <= 512:
            R = 16
        elif 8 * K <= 512:
            R = 8
        else:
            R = 4
        R = min(R, N - p)
        groups.append((p, R, K))
        p += R

    # bin-pack groups into 512-wide slot tiles (first-fit decreasing)
    sizes = [R * K for (_, R, K) in groups]
    order_g = np.argsort(-np.asarray(sizes), kind="stable")
    tiles_used = []
    place = [None] * len(groups)
    for gi in order_g:
        sz = sizes[gi]
        for t in range(len(tiles_used)):
            if tiles_used[t] + sz <= 512:
                place[gi] = (t, tiles_used[t])
                tiles_used[t] += sz
                break
        else:
            place[gi] = (len(tiles_used), 0)
            tiles_used.append(sz)
    n_tiles = len(tiles_used)
    S_graph = 512 * n_tiles
    S_core = BL * S_graph

    # per-position lookup tables
    col_base_of_pos = np.zeros(N, np.int64)   # first column of the receiver
    K_of_pos = np.zeros(N, np.int64)
    for gi, (p0, R, K) in enumerate(groups):
        t, off = place[gi]
        for r in range(R):
            col_base_of_pos[p0 + r] = t * 512 + off + r * K
            K_of_pos[p0 + r] = K

    # slots ordered by (g, d, s): contiguous per receiver
    order = np.lexsort((us, ud, ug))
    og, od, osl = ug[order], ud[order], order
    osrc = us[order]
    recv_id = og * N + od
    first = np.concatenate([[0], np.flatnonzero(np.diff(recv_id)) + 1])
    k_rank = np.arange(len(og)) - first[np.searchsorted(recv_id[first], recv_id)]

    pos = rho_inv[og, od]
    core_r = og // BL
    col_r = (og % BL) * S_graph + col_base_of_pos[pos] + k_rank

    # padding: receivers with deg < K duplicate their first slot
    fg, fd = og[first], od[first]
    fpos = rho_inv[fg, fd]
    fdeg = deg[fg, fd]
    fK = K_of_pos[fpos]
    padc = (fK - fdeg).astype(np.int64)
    assert (padc >= 0).all()
    rep = np.repeat(np.arange(len(first)), padc)
    # k index within each padded receiver: deg .. K-1
    kpad = np.arange(len(rep)) - np.repeat(
        np.concatenate([[0], np.cumsum(padc)[:-1]]), padc
    ) + np.repeat(fdeg, padc)
    pg = fg[rep]
    core_p = pg // BL
    col_p = (pg % BL) * S_graph + col_base_of_pos[fpos[rep]] + kpad
    slot_p = osl[first][rep]
    src_p = osrc[first][rep]
    pos_p = fpos[rep]

    a_core = np.concatenate([core_r, core_p])
    a_col = np.concatenate([col_r, col_p])
    a_slot = np.concatenate([osl, slot_p])
    a_srcnew = np.concatenate(
        [rho_inv[og, osrc], rho_inv[pg, src_p]]
    )
    a_dstpos = np.concatenate([pos, pos_p])

    import ml_dtypes
    flat = a_core * S_core + a_col
    Gsrc = np.zeros((M * S_core, 128), np.float32)
    Gdst = np.zeros((M * S_core, 128), np.float32)
    Gsrc[flat, a_srcnew] = 1.0
    Gdst[flat, a_dstpos] = 1.0
    SOH = np.zeros((M * S_core, 48), np.float32)
    SOH[flat] = oh48[a_slot]
    FP8 = ml_dtypes.float8_e4m3fn
    assert float(SOH.max()) <= 16.0, "edge-attr counts exceed exact fp8 range"
    Gsrc = np.ascontiguousarray(
        Gsrc.reshape(M, S_core, 128).transpose(0, 2, 1)).astype(FP8)
    Gdst = np.ascontiguousarray(
        Gdst.reshape(M, S_core, 128).transpose(0, 2, 1)).astype(FP8)
    SOH = np.ascontiguousarray(
        SOH.reshape(M, S_core, 48).transpose(0, 2, 1)).astype(FP8)

    # atom one-hot per core: [9, 128, BL*N] in relabeled node order
    gg = np.repeat(np.arange(B), N)
    pp = np.tile(np.arange(N), B)
    orig = gg * N + rho[gg, pp]                    # [B*N] column -> orig node
    XOH = np.zeros((M, 9, AV, BL * N), np.float32)
    mcol = np.tile(np.arange(BL * N), M)
    mcore = np.repeat(np.arange(M), BL * N)
    for c in range(9):
        XOH[mcore, c, x[orig, c], mcol] = 1.0

    # empty receivers (deg==0) -> need NEG mask path
    empt = (deg == 0)
    has_empty = bool(empt.any())
    maskrow = np.ones((M, BL * N), np.float32)
    negrow = np.zeros((M, BL * N), np.float32)
    if has_empty:
        eg, en = np.nonzero(empt)
        epos = rho_inv[eg, en]
        maskrow[eg // BL, (eg % BL) * N + epos] = 0.0
        negrow[eg // BL, (eg % BL) * N + epos] = NEG

    tile_used = [max(256, ((u + 7) // 8) * 8) for u in tiles_used]
    struct = dict(
        S_graph=S_graph, S_core=S_core, n_tiles=n_tiles,
        groups=[(p0, R, K, place[gi][0], place[gi][1])
                for gi, (p0, R, K) in enumerate(groups)],
        tile_used=tuple(tile_used),
        has_empty=has_empty,
    )
    XOH = XOH.astype(ml_dtypes.float8_e4m3fn)
    percore = dict(Gsrc=Gsrc, Gdst=Gdst, SOH=SOH, XOH=XOH,
                   maskrow=maskrow, negrow=negrow)
    return struct, percore


def _weight_arrays(inputs):
    f32 = np.float32
    A = {}

    # one [128, *] blob holding every matmul operand tile: single DMA.
    Wm1 = np.asarray(inputs["Wm1"], f32)
    Wm2 = np.asarray(inputs["Wm2"], f32)
    atom = np.asarray(inputs["atom_emb"], f32)
    cols = []
    wmap = {}

    def add(name, arr):
        wmap[name] = (sum(c.shape[1] for c in cols), arr.shape[1])
        cols.append(arr)

    bond_T = np.zeros((128, 48), f32)
    bond_T[:, :] = np.asarray(inputs["bond_emb"], f32).reshape(48, H).T
    add("bondT", bond_T)
    for l in range(L):
        add(f"We_{l}", np.asarray(inputs["We"], f32)[l])
        add(f"Wo2_{l}", np.asarray(inputs["Wo2"], f32)[l])
    # split point 2: layer-0 weights (needed as soon as graph 0's G lands)
    add("idn", np.eye(128, dtype=f32))
    for l in range(L):
        add(f"m12_{l}_0", np.concatenate([Wm1[l, 0:128], Wm2[l, 0:128]], 1))
        add(f"m12_{l}_1", np.concatenate([Wm1[l, 128:256], Wm2[l, 128:256]], 1))
        add(f"Wp1_{l}", np.asarray(inputs["Wp1"], f32)[l])
        add(f"Wp2_{l}", np.asarray(inputs["Wp2"], f32)[l])
        add(f"Wo1_{l}_0", np.asarray(inputs["Wo1"], f32)[l, 0:128])
        add(f"Wo1_{l}_1", np.asarray(inputs["Wo1"], f32)[l, 128:256])
        if l == 0:
            add("Wh1", np.asarray(inputs["Wh1"], f32))
            add("Wh2", np.asarray(inputs["Wh2"], f32))
    A["wblob"] = np.ascontiguousarray(np.concatenate(cols, 1))
    A["_wmap"] = wmap
    import ml_dtypes
    at = atom.transpose(1, 0, 2).reshape(AV, 9 * H)        # [128, 1152]
    hi = at.astype(ml_dtypes.bfloat16)
    res = (at - hi.astype(f32)).astype(ml_dtypes.bfloat16)
    A["atomb"] = np.ascontiguousarray(np.concatenate([hi, res], 1))

    # bias columns [128, 26]: per l: 4 pre-terms, 2 o-terms, bp1, bp2; + bh1 bh2
    bc = np.zeros((H, 33), f32)
    bc[:, 26] = EPS
    bc[:, 27:30] = np.asarray(inputs["ln_s"], f32).T
    bc[:, 30:33] = np.asarray(inputs["ln_b"], f32).T
    for l in range(L):
        bc[:, 4 * l + 0] = np.asarray(inputs["bm1"], f32)[l]
        bc[:, 4 * l + 1] = np.asarray(inputs["bm2"], f32)[l]
        bc[:, 4 * l + 2] = np.asarray(inputs["be"], f32)[l]
        bc[:, 4 * l + 3] = np.asarray(inputs["bg"], f32)[l]
        bc[:, 12 + 2 * l + 0] = np.asarray(inputs["bo1"], f32)[l]
        bc[:, 12 + 2 * l + 1] = np.asarray(inputs["bo2"], f32)[l]
        bc[:, 18 + l] = np.asarray(inputs["bp1"], f32)[l]
        bc[:, 23 + l] = np.asarray(inputs["bp2"], f32)[l]
    bc[:, 21] = np.asarray(inputs["bh1"], f32)
    bc[:, 22] = np.asarray(inputs["bh2"], f32)[:H]
    A["bias_cols"] = bc
    A["bh2_full"] = np.ascontiguousarray(
        np.asarray(inputs["bh2"], f32).reshape(OUT, 1))
    bp2f = np.zeros((H, 4), f32)
    bp2f[:, :L] = np.asarray(inputs["bp2"], f32).T
    A["bp2f"] = bp2f  # [128, 4] (padded so N=2 slices stay in range)
    return A


# --------------------------------------------------------------------------
# Bass program.
# --------------------------------------------------------------------------

def _build_program(struct, wmap, wcols):
    import concourse.bacc as bacc
    import concourse.mybir as mybir
    import concourse.tile as tile

    F32 = mybir.dt.float32
    F32R = mybir.dt.float32r
    AF = mybir.ActivationFunctionType
    ALU = mybir.AluOpType
    AX = mybir.AxisListType

    S_graph = struct["S_graph"]
    S_core = struct["S_core"]
    n_tiles = struct["n_tiles"]
    groups = struct["groups"]
    has_empty = struct["has_empty"]

    nc = bacc.Bacc("TRN2", target_bir_lowering=False, debug=False)

    # ---- DRAM tensors
    BF16 = mybir.dt.bfloat16
    FP8 = mybir.dt.float8e4
    d_gs = nc.dram_tensor("gsrc", [128, S_core], FP8, kind="ExternalInput")
    d_gd = nc.dram_tensor("gdst", [128, S_core], FP8, kind="ExternalInput")
    d_soh = nc.dram_tensor("soh", [48, S_core], FP8, kind="ExternalInput")
    d_xoh = nc.dram_tensor("xoh", [9, AV, BL * N], FP8, kind="ExternalInput")
    d_atomb = nc.dram_tensor("atomb", [AV, 2 * 9 * H], BF16, kind="ExternalInput")
    d_wblob = nc.dram_tensor("wblob", [128, wcols], F32R, kind="ExternalInput")
    d_bc = nc.dram_tensor("bias_cols", [H, 33], F32, kind="ExternalInput")
    d_bh2 = nc.dram_tensor("bh2_full", [OUT, 1], F32, kind="ExternalInput")
    d_bp2f = nc.dram_tensor("bp2f", [H, 4], F32R, kind="ExternalInput")
    d_mask = nc.dram_tensor("maskrow", [1, BL * N], F32, kind="ExternalInput")
    d_neg = nc.dram_tensor("negrow", [1, BL * N], F32, kind="ExternalInput")
    d_out = nc.dram_tensor("out", [OUT, BL], F32, kind="ExternalOutput")

    with tile.TileContext(nc) as tc:
        _emit(tc, nc, locals(), struct, wmap, mybir, F32, F32R, AF, ALU, AX)
    nc.compile()
    return nc


def _emit(tc, nc, d, struct, wmap, mybir, F32, F32R, AF, ALU, AX):
    import contextlib
    ctx = contextlib.ExitStack()
    S_graph = struct["S_graph"]
    S_core = struct["S_core"]
    n_tiles = struct["n_tiles"]
    groups = struct["groups"]
    tile_used = struct["tile_used"]
    has_empty = struct["has_empty"]

    pG = ctx.enter_context(tc.tile_pool(name="pG", bufs=1))
    pW = ctx.enter_context(tc.tile_pool(name="pW", bufs=1))
    pAct = ctx.enter_context(tc.tile_pool(name="pAct", bufs=4))
    pNM = ctx.enter_context(tc.tile_pool(name="pNM", bufs=1))
    pMB = ctx.enter_context(tc.tile_pool(name="pMB", bufs=2))
    pLN = ctx.enter_context(tc.tile_pool(name="pLN", bufs=2))
    pIn = ctx.enter_context(tc.tile_pool(name="pIn", bufs=2))
    ps_pre = ctx.enter_context(tc.tile_pool(name="ps_pre", bufs=2, space="PSUM"))
    ps_p1 = ctx.enter_context(tc.tile_pool(name="ps_p1", bufs=2, space="PSUM"))
    ps_p2 = ctx.enter_context(tc.tile_pool(name="ps_p2", bufs=2, space="PSUM"))
    ps_misc = ctx.enter_context(tc.tile_pool(name="ps_misc", bufs=2, space="PSUM"))

    def mps(name, dt=F32):
        return ps_misc.tile([128, 512], dt, name=name, tag="mps")

    # ---- resident loads (small/early-needed tensors first)
    BF16 = mybir.dt.bfloat16
    FP8 = mybir.dt.float8e4
    gs_sb = pG.tile([128, S_core], FP8, name="gs_sb")
    gd_sb = pG.tile([128, S_core], FP8, name="gd_sb")
    soh_res = pG.tile([48, S_core], FP8, name="soh_res")

    wcols = sum(w for (_, w) in wmap.values())
    wblob_sb = pW.tile([128, wcols], F32R, name="wblob_sb")
    ws1 = wmap["idn"][0]
    ws2 = wmap["m12_1_0"][0]
    nc.sync.dma_start(wblob_sb[:, 0:ws1], d["d_wblob"].ap()[:, 0:ws1])
    atomb_sb = pW.tile([AV, 2 * 9 * H], BF16, name="atomb_sb")
    nc.sync.dma_start(atomb_sb[:], d["d_atomb"].ap())

    def W(name):
        off, w = wmap[name]
        return wblob_sb[:, off:off + w]

    idn_sb = W("idn")
    idn32_sb = W("idn").bitcast(F32)
    wh1_sb = W("Wh1")
    wh2_sb = W("Wh2")
    bc_sb = pW.tile([H, 33], F32, name="bc_sb")
    nc.sync.dma_start(bc_sb[:], d["d_bc"].ap())
    bh2_sb = pW.tile([OUT, 1], F32, name="bh2_sb")
    nc.sync.dma_start(bh2_sb[:], d["d_bh2"].ap())
    bp2f_sb = pW.tile([H, 4], F32R, name="bp2f_sb")
    nc.sync.dma_start(bp2f_sb[:], d["d_bp2f"].ap())

    if has_empty:
        mrow_sb = pW.tile([1, BL * N], F32, name="mrow_sb")
        nc.sync.dma_start(mrow_sb[:], d["d_mask"].ap())
        nrow_sb = pW.tile([1, BL * N], F32, name="nrow_sb")
        nc.sync.dma_start(nrow_sb[:], d["d_neg"].ap())
        mask_bc = pW.tile([128, BL * N], F32, name="mask_bc")
        nc.gpsimd.partition_broadcast(mask_bc[:], mrow_sb[:])
        neg_bc = pW.tile([128, BL * N], F32, name="neg_bc")
        nc.gpsimd.partition_broadcast(neg_bc[:], nrow_sb[:])

    # bias prework (all layers at once)
    bias_pre = pW.tile([128, L], F32, name="bias_pre")
    nc.vector.tensor_reduce(
        bias_pre[:], bc_sb[:, 0:4 * L].rearrange("p (l f) -> p l f", l=L),
        axis=AX.X, op=ALU.add)
    bo12 = pW.tile([128, L], F32, name="bo12")
    nc.vector.tensor_reduce(
        bo12[:], bc_sb[:, 12:12 + 2 * L].rearrange("p (l f) -> p l f", l=L),
        axis=AX.X, op=ALU.add)

    # per-layer prework (layer-independent of activations): BW_l, bias_h_l,
    # ln broadcast tiles -- all hoisted so they overlap the big G DMAs.
    bw_l, bias_h_l = {}, {}
    for l in range(L):
        bw_ps = mps("bw_ps")
        nc.tensor.matmul(bw_ps[0:48, 0:H], W("bondT"), W(f"We_{l}"),
                         start=True, stop=True)
        bw_sb = pMB.tile([48, H], BF16, name="bw_sb", tag=f"bw_sb{l}", bufs=1)
        nc.scalar.activation(bw_sb[:], bw_ps[0:48, 0:H], AF.Copy)
        bw_l[l] = bw_sb

        bh_ps = mps("bh_ps")
        nc.tensor.matmul(bh_ps[:, 0:2], W(f"Wo2_{l}"),
                         bp2f_sb[:, l:l + 2], start=True, stop=True)
        bh_tmp = pMB.tile([128, 1], F32, name="bh_tmp", tag="bh_tmp")
        nc.scalar.activation(bh_tmp[:], bh_ps[:, 0:1], AF.Copy)
        bias_h = pMB.tile([128, 1], F32, name="bias_h", tag=f"bias_h{l}",
                          bufs=1)
        nc.vector.tensor_tensor(bias_h[:], bh_tmp[:], bo12[:, l:l + 1],
                                op=ALU.add)
        bias_h_l[l] = bias_h


    # ---- node features (feature-major) + zero hidden
    nf_ps = mps("nf_ps")
    for c in range(9):
        xoh_sb = pIn.tile([AV, BL * N], FP8, name="xoh_sb", tag="xoh")
        nc.sync.dma_start(xoh_sb[:], d["d_xoh"].ap()[c])
        nc.tensor.matmul(nf_ps[:], atomb_sb[:, c * H:(c + 1) * H], xoh_sb[:],
                         start=(c == 0), stop=False)
        nc.tensor.matmul(nf_ps[:], atomb_sb[:, (9 + c) * H:(10 + c) * H],
                         xoh_sb[:], start=False, stop=(c == 8))
    nf = pNM.tile([128, BL * N], F32R, name="nf")
    nc.scalar.activation(nf[:], nf_ps[:], AF.Copy)
    hid0 = pNM.tile([128, BL * N], F32R, name="hid0", tag="hid0")
    nc.scalar.mul(hid0[:], nf[:].bitcast(F32), 0.0)
    nc.sync.dma_start(wblob_sb[:, ws1:ws2], d["d_wblob"].ap()[:, ws1:ws2])

    for gg in range(BL):
        sl = slice(gg * S_graph, (gg + 1) * S_graph)
        nc.sync.dma_start(gs_sb[:, sl], d["d_gs"].ap()[:, sl])
        nc.sync.dma_start(gd_sb[:, sl], d["d_gd"].ap()[:, sl])
        nc.sync.dma_start(soh_res[:, sl], d["d_soh"].ap()[:, sl])
        if gg == 0:
            nc.sync.dma_start(wblob_sb[:, ws2:], d["d_wblob"].ap()[:, ws2:])

    hid_prev = hid0
    for l in range(L):
        bw_sb = bw_l[l]
        bias_h = bias_h_l[l]

        msgs_max = pLN.tile([128, BL * N], F32, name="msgs_max",
                            tag="msgs_max", bufs=1)
        m1_nm, m2_nm = {}, {}
        for gg in range(BL):
            # m1/m2 for this graph right before its slot tiles, so the PE
            # (in-order) doesn't stall on a later graph's hidden state
            gsl = slice(gg * N, (gg + 1) * N)
            ps_m = mps("ps_m")
            nc.tensor.matmul(ps_m[:, 0:2 * H], nf[:, gsl],
                             W(f"m12_{l}_0"), start=True, stop=False)
            nc.tensor.matmul(ps_m[:, 0:2 * H], hid_prev[:, gsl],
                             W(f"m12_{l}_1"), start=False, stop=True)
            mt = pMB.tile([128, 2 * H], BF16, name=f"m12_nm{gg}",
                          tag=f"m12_nm{gg}")
            nc.scalar.activation(mt[:], ps_m[:, 0:2 * H], AF.Copy)
            m1_nm[gg] = mt[:, 0:H]
            m2_nm[gg] = mt[:, H:2 * H]
            for t in range(n_tiles):
                c0 = gg * S_graph + t * 512
                w = tile_used[t]
                pre = ps_pre.tile([128, 512], F32, name="pre")
                nc.tensor.matmul(pre[:, 0:w], m1_nm[gg], gs_sb[:, c0:c0 + w],
                                 start=True, stop=False)
                nc.tensor.matmul(pre[:, 0:w], m2_nm[gg], gd_sb[:, c0:c0 + w],
                                 start=False, stop=False)
                nc.tensor.matmul(pre[:, 0:w], bw_sb[:],
                                 soh_res[:, c0:c0 + w], start=False, stop=True)
                msgs1 = pAct.tile([128, 512], F32R, name="msgs1", tag="msgs1")
                nc.scalar.activation(msgs1[:, 0:w], pre[:, 0:w], AF.Relu,
                                     bias=bias_pre[:, l:l + 1])
                p1 = ps_p1.tile([128, 512], F32, name="p1")
                nc.tensor.matmul(p1[:, 0:w], W(f"Wp1_{l}"), msgs1[:, 0:w],
                                 start=True, stop=True)
                msgs2 = pAct.tile([128, 512], F32R, name="msgs2", tag="msgs2")
                if (gg * n_tiles + t) * 9 % 20 < 10:
                    # DVE relu: (p1 + bias) max 0  -- offloads the ACT engine
                    nc.vector.tensor_scalar(
                        msgs2[:, 0:w], p1[:, 0:w], bc_sb[:, 18 + l:19 + l],
                        0.0, op0=ALU.add, op1=ALU.max)
                else:
                    nc.scalar.activation(msgs2[:, 0:w], p1[:, 0:w], AF.Relu,
                                         bias=bc_sb[:, 18 + l:19 + l])
                p2 = ps_p2.tile([128, 512], F32, name="p2")
                nc.tensor.matmul(p2[:, 0:w], W(f"Wp2_{l}"), msgs2[:, 0:w],
                                 start=True, stop=True)
                for (p0, R, K, gt, off) in groups:
                    if gt != t:
                        continue
                    nc.vector.tensor_reduce(
                        msgs_max[:, gg * N + p0: gg * N + p0 + R],
                        p2[:, off:off + R * K].rearrange(
                            "p (r k) -> p r k", r=R),
                        axis=AX.X, op=ALU.max)

        # per-graph: + bp2, h = relu(z @ Wo1 + msgs @ Wo2 + bias_h), LN stats
        msgs_used = pLN.tile([128, BL * N], F32R, name="msgs_used",
                             tag="msgs_used", bufs=1)
        h_fm = pLN.tile([128, BL * N], F32R, name="h_fm", tag="h_fm", bufs=1)
        sumh = pLN.tile([128, BL], F32, name="sumh", tag="sumh")
        sumsq = pLN.tile([128, BL], F32, name="sumsq", tag="sumsq")
        h_nm = []
        for gg in range(BL):
            gsl = slice(gg * N, (gg + 1) * N)
            if has_empty:
                mm1 = pLN.tile([128, N], F32, name="mm1", tag="mm1")
                nc.scalar.activation(mm1[:], msgs_max[:, gsl], AF.Identity,
                                     bias=bc_sb[:, 23 + l:24 + l])
                nc.vector.tensor_tensor(mm1[:], mm1[:], mask_bc[:, gsl],
                                        op=ALU.mult)
                nc.vector.tensor_tensor(mm1[:], mm1[:], neg_bc[:, gsl],
                                        op=ALU.add)
                nc.scalar.activation(msgs_used[:, gsl], mm1[:], AF.Copy)
            else:
                nc.vector.tensor_scalar(msgs_used[:, gsl], msgs_max[:, gsl],
                                        bc_sb[:, 23 + l:24 + l], None,
                                        op0=ALU.add)
            h_ps = mps("h_ps")
            nc.tensor.matmul(h_ps[:, 0:N], W(f"Wo1_{l}_0"), nf[:, gsl],
                             start=True, stop=False)
            nc.tensor.matmul(h_ps[:, 0:N], W(f"Wo1_{l}_1"), hid_prev[:, gsl],
                             start=False, stop=False)
            nc.tensor.matmul(h_ps[:, 0:N], W(f"Wo2_{l}"), msgs_used[:, gsl],
                             start=False, stop=True)
            nc.scalar.activation(h_fm[:, gsl], h_ps[:, 0:N], AF.Relu,
                                 bias=bias_h[:])
            tp = mps("tp_ps", F32R)
            nc.tensor.transpose(tp[:, 0:128], h_fm[:, gsl], idn_sb)
            hn = pLN.tile([128, 128], F32, name=f"h_nm{gg}", tag=f"h_nm{gg}")
            nc.scalar.activation(hn[:], tp[:, 0:128].bitcast(F32), AF.Copy,
                                 accum_out=sumh[:, gg:gg + 1])
            hsq = pLN.tile([128, 128], F32, name="hsq", tag="hsq")
            if gg % 2 == 0:
                nc.vector.tensor_tensor(hsq[:], hn[:], hn[:], op=ALU.mult)
                nc.vector.tensor_reduce(sumsq[:, gg:gg + 1], hsq[:],
                                        axis=AX.X, op=ALU.add)
            else:
                nc.scalar.activation(hsq[:], hn[:], AF.Square,
                                     accum_out=sumsq[:, gg:gg + 1])
            h_nm.append(hn)
        negmean = pLN.tile([128, BL], F32, name="negmean", tag="negmean")
        var = pLN.tile([128, BL], F32, name="var", tag="var")
        msq = pLN.tile([128, BL], F32, name="msq", tag="msq")
        std = pLN.tile([128, BL], F32, name="std", tag="std")
        rstd = pLN.tile([128, BL], F32, name="rstd", tag="rstd")
        for gg in range(BL):
            gsl1 = slice(gg, gg + 1)
            nc.vector.tensor_scalar(negmean[:, gsl1], sumh[:, gsl1],
                                    -1.0 / H, None, op0=ALU.mult)
            nc.vector.tensor_scalar(var[:, gsl1], sumsq[:, gsl1],
                                    1.0 / H, None, op0=ALU.mult)
            nc.vector.tensor_tensor(msq[:, gsl1], negmean[:, gsl1],
                                    negmean[:, gsl1], op=ALU.mult)
            nc.vector.tensor_tensor(var[:, gsl1], var[:, gsl1],
                                    msq[:, gsl1], op=ALU.subtract)
            nc.scalar.activation(std[:, gsl1], var[:, gsl1], AF.Sqrt,
                                 bias=bc_sb[:, 26:27])
            nc.vector.reciprocal(rstd[:, gsl1], std[:, gsl1])

        hid_new = pNM.tile([128, BL * N], F32R, name=f"hid{l + 1}",
                           tag=f"hid{(l + 1) % 2}")
        for gg in range(BL):
            gsl = slice(gg * N, (gg + 1) * N)
            hnorm = pLN.tile([128, 128], F32, name="hnorm", tag="hnorm")
            nc.vector.tensor_scalar(hnorm[:], h_nm[gg][:],
                                    negmean[:, gg:gg + 1], rstd[:, gg:gg + 1],
                                    op0=ALU.add, op1=ALU.mult)
            tp2 = mps("tp2_ps")
            nc.tensor.transpose(tp2[:, 0:128], hnorm[:], idn32_sb)
            # per-feature ln scale/bias is per-partition in feature-major
            nc.vector.tensor_scalar(hid_new[:, gsl], tp2[:, 0:128],
                                    bc_sb[:, 27 + l:28 + l],
                                    bc_sb[:, 30 + l:31 + l],
                                    op0=ALU.mult, op1=ALU.add)
        hid_prev = hid_new

    # ---- pooling + prediction MLP
    ge_sum = pLN.tile([128, BL], F32, name="ge_sum", tag="ge_sum")
    nc.vector.tensor_reduce(
        ge_sum[:], hid_prev[:].bitcast(F32).rearrange("p (g n) -> p g n", g=BL),
        axis=AX.X, op=ALU.add)
    ge = pLN.tile([128, BL], F32R, name="ge", tag="ge")
    nc.scalar.activation(ge[:], ge_sum[:], AF.Copy, scale=1.0 / N)
    o1 = mps("o1_ps")
    nc.tensor.matmul(o1[:, 0:BL], wh1_sb, ge[:], start=True, stop=True)
    t1 = pLN.tile([128, BL], F32R, name="t1", tag="t1")
    nc.scalar.activation(t1[:], o1[:, 0:BL], AF.Relu,
                         bias=bc_sb[:, 21:22])
    o2 = mps("o2_ps")
    nc.tensor.matmul(o2[:, 0:BL], wh2_sb, t1[:], start=True, stop=True)
    out_sb = pLN.tile([OUT, BL], F32, name="out_sb", tag="out_sb")
    nc.scalar.activation(out_sb[:], o2[:, 0:BL], AF.Identity,
                         bias=bh2_sb[:])
    nc.sync.dma_start(d["d_out"].ap(), out_sb[:])
    ctx.close()


# --------------------------------------------------------------------------
# Entry point.
# --------------------------------------------------------------------------

def build(inputs):
    struct, percore = _prep(inputs)
    A = _weight_arrays(inputs)
    wmap = A.pop("_wmap")
    key = (struct["S_graph"], struct["n_tiles"],
           tuple(struct["groups"]), struct["has_empty"])
    if key not in _CACHE:
        _CACHE[key] = _build_program(struct, wmap, A["wblob"].shape[1])
    nc = _CACHE[key]

    in_maps = []
    for c in range(M):
        im = dict(
            gsrc=percore["Gsrc"][c], gdst=percore["Gdst"][c],
            soh=percore["SOH"][c], xoh=percore["XOH"][c],
            maskrow=percore["maskrow"][c:c + 1],
            negrow=percore["negrow"][c:c + 1],
        )
        for k, v in A.items():
            im[k] = v
        in_maps.append(im)
    return nc, in_maps, struct


def kernel(**inputs):
    from concourse import bass_utils
    nc, in_maps, struct = build(inputs)
    res = bass_utils.run_bass_kernel_spmd(nc, in_maps, core_ids=list(range(M)))
    out = np.zeros((B, OUT), np.float32)
    for c in range(M):
        out[c * BL:(c + 1) * BL] = res.results[c]["out"].T
    return out



# revision 17
# speedup vs baseline: 1.0284x; 1.0284x over previous
"""Trainium2 Bass kernel for nn_BaselineModel_74509092651544 (CLRS-style MPNN).

Strategy
--------
Data-parallel over graphs: 32 graphs -> 8 cores x 4 graphs.  The dense
[B,N,N,H] message tensor is never materialized: only the ~61k unique
(graph,src,dst) edge slots survive the masked max, so the message MLP runs
on a padded CSR slot layout.

v2 (this file) vs the earlier baseline:
  * fp8 DoubleRow matmuls for the gather stage: m1/m2 (quantized fp8e4m3)
    ride as a 2-k-tile stationary pair against an interleaved Gsrc/Gdst
    one-hot moving tensor; the bond term uses a 24+24 row split of the
    one-hot counts.  1 cycle/slot instead of 3 on the PE.
  * bf16 weights/activations everywhere precision allows (validated
    ~7e-3 rel err vs the 2e-2 gate).
  * h-matmuls and LayerNorm batched across the 4 graphs per layer
    (512-wide) instead of per-graph 128-wide (f32r <256 runs at 1/4 rate).
  * DP-optimized receiver grouping (minimizes padded slots + per-group
    DVE reduce overhead).
  * Software-pipelined slot loop (skew 2) with PSUM rings 3/3/2.
  * relu1 on ACT, relu2 split ACT/DVE, segmented max on DVE.
"""

import sys
import numpy as np

sys.path.insert(0, "/opt/trn_rl_repo")

B, N, H, L, E, OUT = 32, 128, 128, 3, 65536, 128
M = 8                 # NeuronCores
BL = B // M           # graphs per core
NEG = -1e9
EPS = 1e-5
AV, BV = 128, 16
ACT_RELU2_SHARE = 0.65   # fraction of relu2 ops on ACT (rest on DVE)

_CACHE = {}


# --------------------------------------------------------------------------
# Host preprocessing: pure integer / relayout work.
# --------------------------------------------------------------------------

def _ffd_pack(groups):
    sizes = [R * K for (_, R, K) in groups]
    order = np.argsort(-np.asarray(sizes), kind="stable")
    bins, place = [], [None] * len(groups)
    for gi in order:
        sz = sizes[gi]
        for t in range(len(bins)):
            if bins[t] + sz <= 512:
                place[gi] = (t, bins[t])
                bins[t] += sz
                break
        else:
            place[gi] = (len(bins), 0)
            bins.append(sz)
    return bins, place


def _dp_groups(Kp, c_slot, c_group, max_r=128):
    n = len(Kp)
    INF = float("inf")
    dp = [INF] * (n + 1)
    dp[n] = 0.0
    choice = [1] * (n + 1)
    for p in range(n - 1, -1, -1):
        K = int(Kp[p])
        mr = min(n - p, 512 // K, max_r)
        best, best_r = INF, 1
        for R in range(1, mr + 1):
            c = R * K * c_slot + c_group + dp[p + R]
            if c < best:
                best, best_r = c, R
        dp[p], choice[p] = best, best_r
    groups = []
    p = 0
    while p < n:
        R = choice[p]
        groups.append((p, R, int(Kp[p])))
        p += R
    return groups


def _fixed_groups(Kp, r0):
    groups, p, n = [], 0, len(Kp)
    while p < n:
        K = int(Kp[p])
        R = min(r0, n - p)
        while R * K > 512:
            R //= 2
        groups.append((p, R, K))
        p += R
    return groups


def _choose_groups(Kp):
    """Pick the candidate minimizing a per-graph-layer time proxy (ns)."""
    cands = [_fixed_groups(Kp, 16),
             _dp_groups(Kp, 4.0, 100.0),
             _dp_groups(Kp, 4.0, 170.0, 16),
             _dp_groups(Kp, 2.5, 170.0)]
    best, best_c = None, float("inf")
    for gs in cands:
        bins, _ = _ffd_pack(gs)
        S = sum(R * K for (_, R, K) in gs)
        c = S * 4.04 + len(gs) * 170.0 + len(bins) * 395.0
        if c < best_c:
            best, best_c = gs, c
    return best


def _prep(inputs):
    x = np.asarray(inputs["x"]).astype(np.int64)            # [B*N, 9]
    ea = np.asarray(inputs["edge_attr"]).astype(np.int64)   # [E, 3]
    ei = np.asarray(inputs["edge_index"]).astype(np.int64)  # [2, E]

    g = ei[0] // N
    s = ei[0] % N
    d = ei[1] % N
    key = (g * N + s) * N + d
    uniq, inv = np.unique(key, return_inverse=True)
    US = uniq.size
    ug = uniq // (N * N)
    us = (uniq // N) % N
    ud = uniq % N

    # bond one-hot counts per unique slot  [US, 48]
    oh48 = np.zeros((US, 48), np.float32)
    for c in range(3):
        np.add.at(oh48, (inv, ea[:, c] + 16 * c), 1.0)

    # unique in-degree per (graph, receiver)
    deg = np.zeros((B, N), np.int64)
    np.add.at(deg, (ug, ud), 1)

    # receiver relabeling: position p holds the p-th highest-degree receiver
    rho = np.argsort(-deg, axis=1, kind="stable")        # [B, N] pos -> orig
    rho_inv = np.argsort(rho, axis=1)                    # orig -> pos
    degS = -np.sort(-deg, axis=1)                        # [B, N] desc
    Kp = np.maximum(degS.max(axis=0), 1)                 # [N] non-increasing

    groups = _choose_groups(Kp)                          # (p0, R, K)
    tiles_used, place = _ffd_pack(groups)
    n_tiles = len(tiles_used)
    tile_w = [int(w) for w in tiles_used]
    tile_base = np.concatenate([[0], np.cumsum(tile_w)[:-1]]).astype(np.int64)
    S_graph = int(((sum(tile_w) + 15) // 16) * 16)
    S_core = BL * S_graph

    # per-position lookup tables
    col_base_of_pos = np.zeros(N, np.int64)   # first column of the receiver
    K_of_pos = np.zeros(N, np.int64)
    for gi, (p0, R, K) in enumerate(groups):
        t, off = place[gi]
        for r in range(R):
            col_base_of_pos[p0 + r] = tile_base[t] + off + r * K
            K_of_pos[p0 + r] = K

    # slots ordered by (g, d, s): contiguous per receiver
    order = np.lexsort((us, ud, ug))
    og, od, osl = ug[order], ud[order], order
    osrc = us[order]
    recv_id = og * N + od
    first = np.concatenate([[0], np.flatnonzero(np.diff(recv_id)) + 1])
    k_rank = np.arange(len(og)) - first[np.searchsorted(recv_id[first], recv_id)]

    pos = rho_inv[og, od]
    core_r = og // BL
    col_r = (og % BL) * S_graph + col_base_of_pos[pos] + k_rank

    # padding: receivers with deg < K duplicate their first slot
    fg, fd = og[first], od[first]
    fpos = rho_inv[fg, fd]
    fdeg = deg[fg, fd]
    fK = K_of_pos[fpos]
    padc = (fK - fdeg).astype(np.int64)
    assert (padc >= 0).all()
    rep = np.repeat(np.arange(len(first)), padc)
    kpad = np.arange(len(rep)) - np.repeat(
        np.concatenate([[0], np.cumsum(padc)[:-1]]), padc
    ) + np.repeat(fdeg, padc)
    pg = fg[rep]
    core_p = pg // BL
    col_p = (pg % BL) * S_graph + col_base_of_pos[fpos[rep]] + kpad
    slot_p = osl[first][rep]
    src_p = osrc[first][rep]

    a_core = np.concatenate([core_r, core_p])
    a_col = np.concatenate([col_r, col_p])
    a_slot = np.concatenate([osl, slot_p])
    a_srcnew = np.concatenate([rho_inv[og, osrc], rho_inv[pg, src_p]])
    a_dstpos = np.concatenate([pos, fpos[rep]])

    import ml_dtypes
    FP8 = ml_dtypes.float8_e4m3fn
    flat = a_core * S_core + a_col
    # column-interleaved src/dst one-hot k-tiles: [M, 128, S_core*2]
    # (column s occupies bytes [2s, 2s+1]: k=0 -> Gsrc, k=1 -> Gdst)
    Gpair = np.zeros((M * S_core, 2, 128), np.float32)
    Gpair[flat, 0, a_srcnew] = 1.0
    Gpair[flat, 1, a_dstpos] = 1.0
    Gpair = np.ascontiguousarray(
        Gpair.reshape(M, S_core, 2, 128).transpose(0, 3, 1, 2)
        .reshape(M, 128, 2 * S_core)).astype(FP8)
    # bond count k-tiles (rows 0:24 / 24:48), column-interleaved:
    # [M, 24, S_core*2]
    SOH = np.zeros((M * S_core, 48), np.float32)
    SOH[flat] = oh48[a_slot]
    assert float(SOH.max()) <= 16.0
    SOH = np.ascontiguousarray(
        SOH.reshape(M, S_core, 2, 24).transpose(0, 3, 1, 2)
        .reshape(M, 24, 2 * S_core)).astype(FP8)

    # atom one-hot per core: [M, 9, AV, BL*N] in relabeled node order
    gg = np.repeat(np.arange(B), N)
    pp = np.tile(np.arange(N), B)
    orig = gg * N + rho[gg, pp]                    # [B*N] column -> orig node
    XOH = np.zeros((M, 9, AV, BL * N), np.float32)
    mcol = np.tile(np.arange(BL * N), M)
    mcore = np.repeat(np.arange(M), BL * N)
    for c in range(9):
        XOH[mcore, c, x[orig, c], mcol] = 1.0
    XOH = XOH.astype(FP8)

    # empty receivers (deg==0) -> need NEG mask path
    empt = (deg == 0)
    has_empty = bool(empt.any())
    maskrow = np.ones((M, BL * N), np.float32)
    negrow = np.zeros((M, BL * N), np.float32)
    if has_empty:
        eg, en = np.nonzero(empt)
        epos = rho_inv[eg, en]
        maskrow[eg // BL, (eg % BL) * N + epos] = 0.0
        negrow[eg // BL, (eg % BL) * N + epos] = NEG

    struct = dict(
        S_graph=S_graph, S_core=S_core, n_tiles=n_tiles,
        groups=[(p0, R, K, place[gi][0], place[gi][1])
                for gi, (p0, R, K) in enumerate(groups)],
        tile_w=tuple(tile_w), tile_base=tuple(int(b) for b in tile_base),
        has_empty=has_empty,
    )
    percore = dict(Gpair=Gpair, SOH=SOH, XOH=XOH,
                   maskrow=maskrow, negrow=negrow)
    return struct, percore


def _weight_arrays(inputs):
    import ml_dtypes
    BF16 = ml_dtypes.bfloat16
    f32 = np.float32

    def blob(cols):
        wmap = {}
        off = 0
        for name, arr in cols:
            wmap[name] = (off, arr.shape[1])
            off += arr.shape[1]
        data = np.concatenate([a for _, a in cols], 1)
        return np.ascontiguousarray(data), wmap

    Wm1 = np.asarray(inputs["Wm1"], f32)
    Wm2 = np.asarray(inputs["Wm2"], f32)
    bond_T = np.asarray(inputs["bond_emb"], f32).reshape(48, H).T  # [128, 48]
    cols_b = [("bondT", bond_T)]
    for l in range(L):
        cols_b.append((f"We_{l}", np.asarray(inputs["We"], f32)[l]))
    cols_b.append(("idn", np.eye(128, dtype=f32)))
    for l in range(L):
        cols_b.append((f"m12_{l}_0",
                       np.concatenate([Wm1[l, 0:128], Wm2[l, 0:128]], 1)))
        cols_b.append((f"m12_{l}_1",
                       np.concatenate([Wm1[l, 128:256], Wm2[l, 128:256]], 1)))
        cols_b.append((f"Wp1_{l}", np.asarray(inputs["Wp1"], f32)[l]))
        cols_b.append((f"Wp2_{l}", np.asarray(inputs["Wp2"], f32)[l]))
        cols_b.append((f"Wo1_{l}_0", np.asarray(inputs["Wo1"], f32)[l, 0:128]))
        cols_b.append((f"Wo1_{l}_1", np.asarray(inputs["Wo1"], f32)[l, 128:256]))
    wb, wbmap = blob(cols_b)

    cols_r = []
    for l in range(L):
        cols_r.append((f"Wo2_{l}", np.asarray(inputs["Wo2"], f32)[l]))
    cols_r.append(("Wh1", np.asarray(inputs["Wh1"], f32)))
    cols_r.append(("Wh2", np.asarray(inputs["Wh2"], f32)))
    cols_r.append(("idn", np.eye(128, dtype=f32)))
    wr, wrmap = blob(cols_r)

    A = {}
    A["wb"] = wb.astype(BF16)
    A["wr"] = wr
    A["_wbmap"] = wbmap
    A["_wrmap"] = wrmap

    at = np.asarray(inputs["atom_emb"], f32).transpose(1, 0, 2).reshape(AV, 9 * H)
    A["atomb"] = np.ascontiguousarray(at).astype(BF16)

    # bias columns [128, 33]: 4 pre-terms x L, 2 o-terms x L, bh1, bh2, eps,
    # ln_s x L, ln_b x L, bp1 x L, bp2 x L (same layout as baseline)
    bc = np.zeros((H, 33), f32)
    bc[:, 26] = EPS
    bc[:, 27:30] = np.asarray(inputs["ln_s"], f32).T
    bc[:, 30:33] = np.asarray(inputs["ln_b"], f32).T
    for l in range(L):
        bc[:, 4 * l + 0] = np.asarray(inputs["bm1"], f32)[l]
        bc[:, 4 * l + 1] = np.asarray(inputs["bm2"], f32)[l]
        bc[:, 4 * l + 2] = np.asarray(inputs["be"], f32)[l]
        bc[:, 4 * l + 3] = np.asarray(inputs["bg"], f32)[l]
        bc[:, 12 + 2 * l + 0] = np.asarray(inputs["bo1"], f32)[l]
        bc[:, 12 + 2 * l + 1] = np.asarray(inputs["bo2"], f32)[l]
        bc[:, 18 + l] = np.asarray(inputs["bp1"], f32)[l]
        bc[:, 23 + l] = np.asarray(inputs["bp2"], f32)[l]
    bc[:, 21] = np.asarray(inputs["bh1"], f32)
    bc[:, 22] = np.asarray(inputs["bh2"], f32)[:H]
    A["bias_cols"] = bc
    A["bh2_full"] = np.ascontiguousarray(
        np.asarray(inputs["bh2"], f32).reshape(OUT, 1))
    bp2f = np.zeros((H, 4), f32)
    bp2f[:, :L] = np.asarray(inputs["bp2"], f32).T
    A["bp2f"] = bp2f
    return A


# --------------------------------------------------------------------------
# Bass program.
# --------------------------------------------------------------------------

def _build_program(struct, wbmap, wrmap, wbc, wrc):
    import concourse.bacc as bacc
    import concourse.mybir as mybir
    import concourse.tile as tile

    F32 = mybir.dt.float32
    nc = bacc.Bacc("TRN2", target_bir_lowering=False, debug=False)

    BF16 = mybir.dt.bfloat16
    FP8 = mybir.dt.float8e4
    F32R = mybir.dt.float32r
    S_core = struct["S_core"]
    d = {}
    d["d_gpair"] = nc.dram_tensor("gpair", [128, 2 * S_core], FP8,
                                  kind="ExternalInput")
    d["d_soh"] = nc.dram_tensor("soh", [24, 2 * S_core], FP8,
                                kind="ExternalInput")
    d["d_xoh"] = nc.dram_tensor("xoh", [9, AV, BL * N], FP8,
                                kind="ExternalInput")
    d["d_atomb"] = nc.dram_tensor("atomb", [AV, 9 * H], BF16,
                                  kind="ExternalInput")
    d["d_wb"] = nc.dram_tensor("wb", [128, wbc], BF16, kind="ExternalInput")
    d["d_wr"] = nc.dram_tensor("wr", [128, wrc], F32R, kind="ExternalInput")
    d["d_bc"] = nc.dram_tensor("bias_cols", [H, 33], F32, kind="ExternalInput")
    d["d_bh2"] = nc.dram_tensor("bh2_full", [OUT, 1], F32, kind="ExternalInput")
    d["d_bp2f"] = nc.dram_tensor("bp2f", [H, 4], F32R, kind="ExternalInput")
    d["d_mask"] = nc.dram_tensor("maskrow", [1, BL * N], F32,
                                 kind="ExternalInput")
    d["d_neg"] = nc.dram_tensor("negrow", [1, BL * N], F32,
                                kind="ExternalInput")
    d["d_out"] = nc.dram_tensor("out", [OUT, BL], F32, kind="ExternalOutput")

    with tile.TileContext(nc) as tc:
        _emit(tc, nc, d, struct, wbmap, wrmap, mybir)
    nc.compile()
    return nc


def _emit(tc, nc, d, struct, wbmap, wrmap, mybir):
    import contextlib
    ctx = contextlib.ExitStack()
    F32 = mybir.dt.float32
    F32R = mybir.dt.float32r
    BF16 = mybir.dt.bfloat16
    FP8 = mybir.dt.float8e4
    AF = mybir.ActivationFunctionType
    ALU = mybir.AluOpType
    AX = mybir.AxisListType
    DR = mybir.MatmulPerfMode.DoubleRow

    S_graph = struct["S_graph"]
    S_core = struct["S_core"]
    n_tiles = struct["n_tiles"]
    groups = struct["groups"]
    tile_w = struct["tile_w"]
    tile_base = struct["tile_base"]
    has_empty = struct["has_empty"]

    pG = ctx.enter_context(tc.tile_pool(name="pG", bufs=1))
    pW = ctx.enter_context(tc.tile_pool(name="pW", bufs=1))
    pAct = ctx.enter_context(tc.tile_pool(name="pAct", bufs=3))
    pNM = ctx.enter_context(tc.tile_pool(name="pNM", bufs=1))
    pMB = ctx.enter_context(tc.tile_pool(name="pMB", bufs=2))
    pLN = ctx.enter_context(tc.tile_pool(name="pLN", bufs=1))
    pIn = ctx.enter_context(tc.tile_pool(name="pIn", bufs=2))
    ps_a = ctx.enter_context(tc.tile_pool(name="ps_a", bufs=3, space="PSUM"))
    ps_b = ctx.enter_context(tc.tile_pool(name="ps_b", bufs=3, space="PSUM"))
    ps_c = ctx.enter_context(tc.tile_pool(name="ps_c", bufs=2, space="PSUM"))

    def psA(dt=F32):
        return ps_a.tile([128, 512], dt, name="psA", tag="psA")

    def psB(dt=F32):
        return ps_b.tile([128, 512], dt, name="psB", tag="psB")

    def psC(dt=F32):
        return ps_c.tile([128, 512], dt, name="psC", tag="psC")

    # ---- resident loads
    gpair_sb = pG.tile([128, 2 * S_core], FP8, name="gpair_sb")
    soh_sb = pG.tile([24, 2 * S_core], FP8, name="soh_sb")

    def k2(tile_sb, c0, w):
        """[p, 2*(c0..c0+w)] interleaved slice -> [p, k=2, s=w] AP."""
        return tile_sb[:, 2 * c0:2 * (c0 + w)].rearrange(
            "p (s k) -> p k s", k=2)

    wb_sb = pW.tile([128, sum(w for _, w in wbmap.values())], BF16,
                    name="wb_sb")
    nc.sync.dma_start(wb_sb[:], d["d_wb"].ap())
    wr_sb = pW.tile([128, sum(w for _, w in wrmap.values())], F32R,
                    name="wr_sb")
    nc.sync.dma_start(wr_sb[:], d["d_wr"].ap())
    bc_sb = pW.tile([H, 33], F32, name="bc_sb")
    nc.sync.dma_start(bc_sb[:], d["d_bc"].ap())
    bh2_sb = pW.tile([OUT, 1], F32, name="bh2_sb")
    nc.sync.dma_start(bh2_sb[:], d["d_bh2"].ap())
    bp2f_sb = pW.tile([H, 4], F32R, name="bp2f_sb")
    nc.sync.dma_start(bp2f_sb[:], d["d_bp2f"].ap())
    atomb_sb = pW.tile([AV, 9 * H], BF16, name="atomb_sb")
    nc.sync.dma_start(atomb_sb[:], d["d_atomb"].ap())

    def WB(name):
        off, w = wbmap[name]
        return wb_sb[:, off:off + w]

    def WR(name):
        off, w = wrmap[name]
        return wr_sb[:, off:off + w]

    if has_empty:
        mrow_sb = pW.tile([1, BL * N], F32, name="mrow_sb")
        nc.sync.dma_start(mrow_sb[:], d["d_mask"].ap())
        nrow_sb = pW.tile([1, BL * N], F32, name="nrow_sb")
        nc.sync.dma_start(nrow_sb[:], d["d_neg"].ap())
        mask_bc = pW.tile([128, BL * N], F32, name="mask_bc")
        nc.gpsimd.partition_broadcast(mask_bc[:], mrow_sb[:])
        neg_bc = pW.tile([128, BL * N], F32, name="neg_bc")
        nc.gpsimd.partition_broadcast(neg_bc[:], nrow_sb[:])

    # bias prework
    bias_pre = pW.tile([128, L], F32, name="bias_pre")
    nc.vector.tensor_reduce(
        bias_pre[:], bc_sb[:, 0:4 * L].rearrange("p (l f) -> p l f", l=L),
        axis=AX.X, op=ALU.add)
    bo12 = pW.tile([128, L], F32, name="bo12")
    nc.vector.tensor_reduce(
        bo12[:], bc_sb[:, 12:12 + 2 * L].rearrange("p (l f) -> p l f", l=L),
        axis=AX.X, op=ALU.add)

    # ---- node features (feature-major), bf16
    nf_ps = psA()
    for c in range(9):
        xoh_sb = pIn.tile([AV, BL * N], FP8, name="xoh_sb", tag="xoh")
        nc.sync.dma_start(xoh_sb[:], d["d_xoh"].ap()[c])
        nc.tensor.matmul(nf_ps[:], atomb_sb[:, c * H:(c + 1) * H], xoh_sb[:],
                         start=(c == 0), stop=(c == 8))
    nf = pNM.tile([128, BL * N], BF16, name="nf")
    nc.scalar.activation(nf[:], nf_ps[:], AF.Copy)

    # ---- per-layer prework: bw24 (fp8 folded bond@We), bias_h
    bw24_l, bias_h_l = {}, {}
    for l in range(L):
        bw_ps = psB()
        nc.tensor.matmul(bw_ps[0:24, 0:128], WB("bondT")[:, 0:24],
                         WB(f"We_{l}"), start=True, stop=True)
        nc.tensor.matmul(bw_ps[0:24, 128:256], WB("bondT")[:, 24:48],
                         WB(f"We_{l}"), start=True, stop=True,
                         skip_group_check=True)
        bw24 = pMB.tile([24, 256], FP8, name="bw24", tag=f"bw24_{l}",
                        bufs=1)
        nc.scalar.activation(bw24[:], bw_ps[0:24, 0:256], AF.Copy)
        bw24_l[l] = bw24
        if not has_empty:
            bh_ps = psC()
            nc.tensor.matmul(bh_ps[:, 0:2], WR(f"Wo2_{l}"),
                             bp2f_sb[:, l:l + 2], start=True, stop=True)
            bias_h = pMB.tile([128, 1], F32, name="bias_h", tag=f"bias_h{l}",
                              bufs=1)
            nc.vector.tensor_tensor(bias_h[:], bh_ps[:, 0:1], bo12[:, l:l + 1],
                                    op=ALU.add)
            bias_h_l[l] = bias_h

    # ---- per-graph gather-matrix DMAs (interleaved: one slice per graph)
    for gg in range(BL):
        sl = slice(2 * gg * S_graph, 2 * (gg + 1) * S_graph)
        nc.sync.dma_start(gpair_sb[:, sl], d["d_gpair"].ap()[:, sl])
        nc.sync.dma_start(soh_sb[:, sl], d["d_soh"].ap()[:, sl])

    # relu2 engine split (Bresenham over all units of all layers)
    relu2_acc = 0.0

    hid_prev = None
    for l in range(L):
        # -- m12 for all graphs, then one fp8 copy pair
        mt = pMB.tile([128, BL * 256], FP8, name="mt", tag="mt", bufs=2)
        for half in range(2):
            ps = psA()
            for gg in (2 * half, 2 * half + 1):
                off = (gg % 2) * 256
                gsl = slice(gg * N, (gg + 1) * N)
                nc.tensor.matmul(ps[:, off:off + 256], nf[:, gsl],
                                 WB(f"m12_{l}_0"), start=True, stop=(l == 0))
                if l > 0:
                    nc.tensor.matmul(ps[:, off:off + 256], hid_prev[:, gsl],
                                     WB(f"m12_{l}_1"), start=False, stop=True)
            nc.scalar.activation(mt[:, half * 512:(half + 1) * 512],
                                 ps[:, 0:512], AF.Copy)

        # -- slot units, software-pipelined (skew 2)
        msgs_max = pLN.tile([128, BL * N], F32R, name="msgs_max",
                            tag="msgs_max", bufs=1)
        units = [(gg, t) for gg in range(BL) for t in range(n_tiles)]
        nU = len(units)
        st1, st2, stP = {}, {}, {}
        for step in range(nU + 4):
            if step < nU:
                gg, t = units[step]
                w = tile_w[t]
                c0 = gg * S_graph + tile_base[t]
                pre = psA()
                mt_g = mt[:, gg * 256:(gg + 1) * 256].rearrange(
                    "p (k f) -> p k f", k=2)
                nc.tensor.matmul(
                    pre[:, 0:w], mt_g, k2(gpair_sb, c0, w),
                    start=True, stop=False, perf_mode=DR)
                bw_k = bw24_l[l][:].rearrange("p (k f) -> p k f", k=2)
                nc.tensor.matmul(
                    pre[:, 0:w], bw_k, k2(soh_sb, c0, w),
                    start=False, stop=True, perf_mode=DR)
                msgs1 = pAct.tile([128, 512], BF16, name="msgs1", tag="msgs1",
                                  bufs=3)
                nc.scalar.activation(msgs1[:, 0:w], pre[:, 0:w], AF.Relu,
                                     bias=bias_pre[:, l:l + 1])
                st1[step] = msgs1
            if 0 <= step - 2 < nU:
                u = step - 2
                gg, t = units[u]
                w = tile_w[t]
                p1 = psB()
                nc.tensor.matmul(p1[:, 0:w], WB(f"Wp1_{l}"),
                                 st1.pop(u)[:, 0:w], start=True, stop=True)
                msgs2 = pAct.tile([128, 512], BF16, name="msgs2", tag="msgs2",
                                  bufs=3)
                relu2_acc += ACT_RELU2_SHARE
                if relu2_acc >= 1.0:
                    relu2_acc -= 1.0
                    nc.scalar.activation(msgs2[:, 0:w], p1[:, 0:w], AF.Relu,
                                         bias=bc_sb[:, 18 + l:19 + l])
                else:
                    nc.vector.tensor_scalar(msgs2[:, 0:w], p1[:, 0:w],
                                            bc_sb[:, 18 + l:19 + l], 0.0,
                                            op0=ALU.add, op1=ALU.max)
                st2[u] = msgs2
            if 0 <= step - 4 < nU:
                u = step - 4
                gg, t = units[u]
                w = tile_w[t]
                p2 = psC()
                nc.tensor.matmul(p2[:, 0:w], WB(f"Wp2_{l}"),
                                 st2.pop(u)[:, 0:w], start=True, stop=True)
                for (p0, R, K, gt, off) in groups:
                    if gt != t:
                        continue
                    nc.vector.tensor_reduce(
                        msgs_max[:, gg * N + p0: gg * N + p0 + R],
                        p2[:, off:off + R * K].rearrange(
                            "p (r k) -> p r k", r=R),
                        axis=AX.X, op=ALU.max)

        # -- h for all graphs (512-wide)
        if has_empty:
            mm = pLN.tile([128, BL * N], F32R, name="mm", tag="mm", bufs=1)
            nc.vector.scalar_tensor_tensor(
                mm[:].bitcast(F32), msgs_max[:].bitcast(F32),
                bc_sb[:, 23 + l:24 + l], mask_bc[:],
                op0=ALU.add, op1=ALU.mult)
            nc.vector.tensor_tensor(mm[:], mm[:].bitcast(F32), neg_bc[:],
                                    op=ALU.add)
            h_moving = mm
            bias_h = bo12[:, l:l + 1]
        else:
            h_moving = msgs_max
            bias_h = bias_h_l[l][:]
        h_ps = psA()
        nc.tensor.matmul(h_ps[:, 0:512], WB(f"Wo1_{l}_0"), nf[:],
                         start=True, stop=False)
        if l > 0:
            nc.tensor.matmul(h_ps[:, 0:512], WB(f"Wo1_{l}_1"), hid_prev[:],
                             start=False, stop=False)
        nc.tensor.matmul(h_ps[:, 0:512], WR(f"Wo2_{l}"), h_moving[:],
                         start=False, stop=True)
        h_fm = pLN.tile([128, BL * N], F32R, name="h_fm", tag="h_fm", bufs=1)
        nc.scalar.activation(h_fm[:], h_ps[:, 0:512], AF.Relu, bias=bias_h)

        # -- LayerNorm: transpose to node-major, batched stats
        tp_ps = psB(F32R)
        for gg in range(BL):
            gsl = slice(gg * N, (gg + 1) * N)
            nc.tensor.transpose(tp_ps[:, gg * 128:(gg + 1) * 128],
                                h_fm[:, gsl], WR("idn"))
        hn = pLN.tile([128, BL * N], F32, name="hn", tag="hn", bufs=1)
        nc.scalar.activation(hn[:], tp_ps[:, 0:512].bitcast(F32), AF.Copy)
        hsq = pLN.tile([128, BL * N], F32, name="hsq", tag="hsq", bufs=1)
        nc.scalar.activation(hsq[:], tp_ps[:, 0:512].bitcast(F32), AF.Square)
        sumh = pLN.tile([128, BL], F32, name="sumh", tag="sumh")
        nc.vector.tensor_reduce(
            sumh[:], hn[:].rearrange("p (g n) -> p g n", g=BL),
            axis=AX.X, op=ALU.add)
        sumsq = pLN.tile([128, BL], F32, name="sumsq", tag="sumsq")
        nc.vector.tensor_reduce(
            sumsq[:], hsq[:].rearrange("p (g n) -> p g n", g=BL),
            axis=AX.X, op=ALU.add)
        negmean = pLN.tile([128, BL], F32, name="negmean", tag="negmean")
        nc.vector.tensor_scalar(negmean[:], sumh[:], -1.0 / H, None,
                                op0=ALU.mult)
        msq = pLN.tile([128, BL], F32, name="msq", tag="msq")
        nc.vector.tensor_tensor(msq[:], negmean[:], negmean[:], op=ALU.mult)
        var = pLN.tile([128, BL], F32, name="var", tag="var")
        nc.vector.scalar_tensor_tensor(var[:], sumsq[:], 1.0 / H, msq[:],
                                       op0=ALU.mult, op1=ALU.subtract)
        std = pLN.tile([128, BL], F32, name="std", tag="std")
        nc.scalar.activation(std[:], var[:], AF.Sqrt, bias=bc_sb[:, 26:27])
        rstd = pLN.tile([128, BL], F32, name="rstd", tag="rstd")
        nc.vector.reciprocal(rstd[:], std[:])

        tp2_ps = psC(F32R)
        for gg in range(BL):
            hnorm = pLN.tile([128, 128], F32R, name="hnorm", tag="hnorm",
                             bufs=2)
            nc.vector.tensor_scalar(hnorm[:], hn[:, gg * 128:(gg + 1) * 128],
                                    negmean[:, gg:gg + 1], rstd[:, gg:gg + 1],
                                    op0=ALU.add, op1=ALU.mult)
            nc.tensor.transpose(tp2_ps[:, gg * 128:(gg + 1) * 128],
                                hnorm[:], WR("idn"))
        hid_new = pNM.tile([128, BL * N], BF16, name=f"hid{l + 1}",
                           tag=f"hid{(l + 1) % 2}")
        nc.vector.tensor_scalar(hid_new[:], tp2_ps[:, 0:512].bitcast(F32),
                                bc_sb[:, 27 + l:28 + l],
                                bc_sb[:, 30 + l:31 + l],
                                op0=ALU.mult, op1=ALU.add)
        hid_prev = hid_new

    # ---- pooling + prediction MLP
    ge_sum = pLN.tile([128, BL], F32, name="ge_sum", tag="ge_sum")
    nc.vector.tensor_reduce(
        ge_sum[:], hid_prev[:].rearrange("p (g n) -> p g n", g=BL),
        axis=AX.X, op=ALU.add)
    ge = pLN.tile([128, BL], F32R, name="ge", tag="ge")
    nc.scalar.activation(ge[:], ge_sum[:], AF.Copy, scale=1.0 / N)
    o1 = psA()
    nc.tensor.matmul(o1[:, 0:BL], WR("Wh1"), ge[:], start=True, stop=True)
    t1 = pLN.tile([128, BL], F32R, name="t1", tag="t1")
    nc.scalar.activation(t1[:], o1[:, 0:BL], AF.Relu, bias=bc_sb[:, 21:22])
    o2 = psB()
    nc.tensor.matmul(o2[:, 0:BL], WR("Wh2"), t1[:], start=True, stop=True)
    out_sb = pLN.tile([OUT, BL], F32, name="out_sb", tag="out_sb")
    nc.scalar.activation(out_sb[:], o2[:, 0:BL], AF.Identity, bias=bh2_sb[:])
    nc.sync.dma_start(d["d_out"].ap(), out_sb[:])
    ctx.close()


# --------------------------------------------------------------------------
# Entry point.
# --------------------------------------------------------------------------

def build(inputs):
    struct, percore = _prep(inputs)
    A = _weight_arrays(inputs)
    wbmap = A.pop("_wbmap")
    wrmap = A.pop("_wrmap")
    key = (struct["S_graph"], struct["n_tiles"],
           tuple(struct["groups"]), struct["tile_w"], struct["has_empty"])
    if key not in _CACHE:
        _CACHE[key] = _build_program(struct, wbmap, wrmap,
                                     A["wb"].shape[1], A["wr"].shape[1])
    nc = _CACHE[key]

    in_maps = []
    for c in range(M):
        im = dict(
            gpair=percore["Gpair"][c], soh=percore["SOH"][c],
            xoh=percore["XOH"][c],
            maskrow=percore["maskrow"][c:c + 1],
            negrow=percore["negrow"][c:c + 1],
        )
        for k, v in A.items():
            im[k] = v
        in_maps.append(im)
    return nc, in_maps, struct


def kernel(**inputs):
    from concourse import bass_utils
    nc, in_maps, struct = build(inputs)
    res = bass_utils.run_bass_kernel_spmd(nc, in_maps, core_ids=list(range(M)))
    out = np.zeros((B, OUT), np.float32)
    for c in range(M):
        out[c * BL:(c + 1) * BL] = res.results[c]["out"].T
    return out


# revision 21
# speedup vs baseline: 1.1155x; 1.0847x over previous
"""Trainium2 Bass kernel for nn_BaselineModel_74509092651544 (CLRS-style MPNN).

Strategy
--------
Data-parallel over graphs: 32 graphs -> 8 cores x 4 graphs.  The dense
[B,N,N,H] message tensor is never materialized: only the ~61k unique
(graph,src,dst) edge slots survive the masked max, so the message MLP runs
on a padded CSR slot layout.

v2 (this file) vs the earlier baseline:
  * fp8 DoubleRow matmuls for the gather stage: m1/m2 (quantized fp8e4m3)
    ride as a 2-k-tile stationary pair against an interleaved Gsrc/Gdst
    one-hot moving tensor; the bond term uses a 24+24 row split of the
    one-hot counts.  1 cycle/slot instead of 3 on the PE.
  * bf16 weights/activations everywhere precision allows (validated
    ~7e-3 rel err vs the 2e-2 gate).
  * h-matmuls and LayerNorm batched across the 4 graphs per layer
    (512-wide) instead of per-graph 128-wide (f32r <256 runs at 1/4 rate).
  * DP-optimized receiver grouping (minimizes padded slots + per-group
    DVE reduce overhead).
  * Software-pipelined slot loop (skew 2) with PSUM rings 3/3/2.
  * relu1 on ACT, relu2 split ACT/DVE, segmented max on DVE.
"""

import sys
import numpy as np

sys.path.insert(0, "/opt/trn_rl_repo")

B, N, H, L, E, OUT = 32, 128, 128, 3, 65536, 128
M = 8                 # NeuronCores
BL = B // M           # graphs per core
NEG = -1e9
EPS = 1e-5
AV, BV = 128, 16
ACT_RELU2_SHARE = 0.65   # fraction of relu2 ops on ACT (rest on DVE)

_CACHE = {}


# --------------------------------------------------------------------------
# Host preprocessing: pure integer / relayout work.
# --------------------------------------------------------------------------

def _ffd_pack(groups):
    sizes = [R * K for (_, R, K) in groups]
    order = np.argsort(-np.asarray(sizes), kind="stable")
    bins, place = [], [None] * len(groups)
    for gi in order:
        sz = sizes[gi]
        for t in range(len(bins)):
            if bins[t] + sz <= 512:
                place[gi] = (t, bins[t])
                bins[t] += sz
                break
        else:
            place[gi] = (len(bins), 0)
            bins.append(sz)
    return bins, place


def _dp_groups(Kp, c_slot, c_group, max_r=128):
    n = len(Kp)
    INF = float("inf")
    dp = [INF] * (n + 1)
    dp[n] = 0.0
    choice = [1] * (n + 1)
    for p in range(n - 1, -1, -1):
        K = int(Kp[p])
        mr = min(n - p, 512 // K, max_r)
        best, best_r = INF, 1
        for R in range(1, mr + 1):
            c = R * K * c_slot + c_group + dp[p + R]
            if c < best:
                best, best_r = c, R
        dp[p], choice[p] = best, best_r
    groups = []
    p = 0
    while p < n:
        R = choice[p]
        groups.append((p, R, int(Kp[p])))
        p += R
    return groups


def _fixed_groups(Kp, r0):
    groups, p, n = [], 0, len(Kp)
    while p < n:
        K = int(Kp[p])
        R = min(r0, n - p)
        while R * K > 512:
            R //= 2
        groups.append((p, R, K))
        p += R
    return groups


def _choose_groups(Kp):
    """Pick the candidate minimizing a per-graph-layer time proxy (ns)."""
    cands = [_fixed_groups(Kp, 16),
             _dp_groups(Kp, 4.0, 100.0),
             _dp_groups(Kp, 4.0, 170.0, 16),
             _dp_groups(Kp, 2.5, 170.0)]
    best, best_c = None, float("inf")
    for gs in cands:
        bins, _ = _ffd_pack(gs)
        S = sum(R * K for (_, R, K) in gs)
        c = S * 4.04 + len(gs) * 170.0 + len(bins) * 395.0
        if c < best_c:
            best, best_c = gs, c
    return best


def _prep(inputs):
    x = np.asarray(inputs["x"]).astype(np.int64)            # [B*N, 9]
    ea = np.asarray(inputs["edge_attr"]).astype(np.int64)   # [E, 3]
    ei = np.asarray(inputs["edge_index"]).astype(np.int64)  # [2, E]

    g = ei[0] // N
    s = ei[0] % N
    d = ei[1] % N
    key = (g * N + s) * N + d
    uniq, inv = np.unique(key, return_inverse=True)
    US = uniq.size
    ug = uniq // (N * N)
    us = (uniq // N) % N
    ud = uniq % N

    # bond one-hot counts per unique slot  [US, 48]
    oh48 = np.zeros((US, 48), np.float32)
    for c in range(3):
        np.add.at(oh48, (inv, ea[:, c] + 16 * c), 1.0)

    # unique in-degree per (graph, receiver)
    deg = np.zeros((B, N), np.int64)
    np.add.at(deg, (ug, ud), 1)

    # receiver relabeling: position p holds the p-th highest-degree receiver
    rho = np.argsort(-deg, axis=1, kind="stable")        # [B, N] pos -> orig
    rho_inv = np.argsort(rho, axis=1)                    # orig -> pos
    degS = -np.sort(-deg, axis=1)                        # [B, N] desc
    Kp = np.maximum(degS.max(axis=0), 1)                 # [N] non-increasing

    groups = _choose_groups(Kp)                          # (p0, R, K)
    tiles_used, place = _ffd_pack(groups)
    n_tiles = len(tiles_used)
    tile_w = [int(w) for w in tiles_used]
    tile_base = np.concatenate([[0], np.cumsum(tile_w)[:-1]]).astype(np.int64)
    S_graph = int(((sum(tile_w) + 15) // 16) * 16)
    S_core = BL * S_graph

    # per-position lookup tables
    col_base_of_pos = np.zeros(N, np.int64)   # first column of the receiver
    K_of_pos = np.zeros(N, np.int64)
    for gi, (p0, R, K) in enumerate(groups):
        t, off = place[gi]
        for r in range(R):
            col_base_of_pos[p0 + r] = tile_base[t] + off + r * K
            K_of_pos[p0 + r] = K

    # slots ordered by (g, d, s): contiguous per receiver
    order = np.lexsort((us, ud, ug))
    og, od, osl = ug[order], ud[order], order
    osrc = us[order]
    recv_id = og * N + od
    first = np.concatenate([[0], np.flatnonzero(np.diff(recv_id)) + 1])
    k_rank = np.arange(len(og)) - first[np.searchsorted(recv_id[first], recv_id)]

    pos = rho_inv[og, od]
    core_r = og // BL
    col_r = (og % BL) * S_graph + col_base_of_pos[pos] + k_rank

    # padding: receivers with deg < K duplicate their first slot
    fg, fd = og[first], od[first]
    fpos = rho_inv[fg, fd]
    fdeg = deg[fg, fd]
    fK = K_of_pos[fpos]
    padc = (fK - fdeg).astype(np.int64)
    assert (padc >= 0).all()
    rep = np.repeat(np.arange(len(first)), padc)
    kpad = np.arange(len(rep)) - np.repeat(
        np.concatenate([[0], np.cumsum(padc)[:-1]]), padc
    ) + np.repeat(fdeg, padc)
    pg = fg[rep]
    core_p = pg // BL
    col_p = (pg % BL) * S_graph + col_base_of_pos[fpos[rep]] + kpad
    slot_p = osl[first][rep]
    src_p = osrc[first][rep]

    a_core = np.concatenate([core_r, core_p])
    a_col = np.concatenate([col_r, col_p])
    a_slot = np.concatenate([osl, slot_p])
    a_srcnew = np.concatenate([rho_inv[og, osrc], rho_inv[pg, src_p]])
    a_dstpos = np.concatenate([pos, fpos[rep]])

    import ml_dtypes
    FP8 = ml_dtypes.float8_e4m3fn
    flat = a_core * S_core + a_col
    # column-interleaved src/dst one-hot k-tiles: [M, 128, S_core*2]
    # (column s occupies bytes [2s, 2s+1]: k=0 -> Gsrc, k=1 -> Gdst)
    Gpair = np.zeros((M * S_core, 2, 128), np.float32)
    Gpair[flat, 0, a_srcnew] = 1.0
    Gpair[flat, 1, a_dstpos] = 1.0
    Gpair = np.ascontiguousarray(
        Gpair.reshape(M, S_core, 2, 128).transpose(0, 3, 1, 2)
        .reshape(M, 128, 2 * S_core)).astype(FP8)
    # bond count k-tiles (rows 0:24 / 24:48), column-interleaved:
    # [M, 24, S_core*2]
    SOH = np.zeros((M * S_core, 48), np.float32)
    SOH[flat] = oh48[a_slot]
    assert float(SOH.max()) <= 16.0
    SOH = np.ascontiguousarray(
        SOH.reshape(M, S_core, 2, 24).transpose(0, 3, 1, 2)
        .reshape(M, 24, 2 * S_core)).astype(FP8)

    # atom one-hot per core: [M, 9, AV, BL*N] in relabeled node order
    gg = np.repeat(np.arange(B), N)
    pp = np.tile(np.arange(N), B)
    orig = gg * N + rho[gg, pp]                    # [B*N] column -> orig node
    XOH = np.zeros((M, 9, AV, BL * N), np.float32)
    mcol = np.tile(np.arange(BL * N), M)
    mcore = np.repeat(np.arange(M), BL * N)
    for c in range(9):
        XOH[mcore, c, x[orig, c], mcol] = 1.0
    XOH = XOH.astype(FP8)

    # empty receivers (deg==0) -> need NEG mask path
    empt = (deg == 0)
    has_empty = bool(empt.any())
    maskrow = np.ones((M, BL * N), np.float32)
    negrow = np.zeros((M, BL * N), np.float32)
    if has_empty:
        eg, en = np.nonzero(empt)
        epos = rho_inv[eg, en]
        maskrow[eg // BL, (eg % BL) * N + epos] = 0.0
        negrow[eg // BL, (eg % BL) * N + epos] = NEG

    struct = dict(
        S_graph=S_graph, S_core=S_core, n_tiles=n_tiles,
        groups=[(p0, R, K, place[gi][0], place[gi][1])
                for gi, (p0, R, K) in enumerate(groups)],
        tile_w=tuple(tile_w), tile_base=tuple(int(b) for b in tile_base),
        has_empty=has_empty,
    )
    percore = dict(Gpair=Gpair, SOH=SOH, XOH=XOH,
                   maskrow=maskrow, negrow=negrow)
    return struct, percore


def _weight_arrays(inputs):
    import ml_dtypes
    BF16 = ml_dtypes.bfloat16
    f32 = np.float32

    def blob(cols):
        wmap = {}
        off = 0
        for name, arr in cols:
            wmap[name] = (off, arr.shape[1])
            off += arr.shape[1]
        data = np.concatenate([a for _, a in cols], 1)
        return np.ascontiguousarray(data), wmap

    Wm1 = np.asarray(inputs["Wm1"], f32)
    Wm2 = np.asarray(inputs["Wm2"], f32)
    bond_T = np.asarray(inputs["bond_emb"], f32).reshape(48, H).T  # [128, 48]
    # chunk 1: everything layer-0's slot phase needs (DMA'd first)
    cols_b = [("bondT", bond_T),
              ("We_0", np.asarray(inputs["We"], f32)[0]),
              ("m12_0_0",
               np.concatenate([Wm1[0, 0:128], Wm2[0, 0:128]], 1)),
              ("Wp1_0", np.asarray(inputs["Wp1"], f32)[0]),
              ("Wp2_0", np.asarray(inputs["Wp2"], f32)[0])]
    wb_split = sum(a.shape[1] for _, a in cols_b)
    # chunk 2: the rest (lands before layer-0's h phase)
    cols_b.append(("Wo1_0_0", np.asarray(inputs["Wo1"], f32)[0, 0:128]))
    for l in range(1, L):
        cols_b.append((f"We_{l}", np.asarray(inputs["We"], f32)[l]))
        cols_b.append((f"m12_{l}_0",
                       np.concatenate([Wm1[l, 0:128], Wm2[l, 0:128]], 1)))
        cols_b.append((f"m12_{l}_1",
                       np.concatenate([Wm1[l, 128:256], Wm2[l, 128:256]], 1)))
        cols_b.append((f"Wp1_{l}", np.asarray(inputs["Wp1"], f32)[l]))
        cols_b.append((f"Wp2_{l}", np.asarray(inputs["Wp2"], f32)[l]))
        cols_b.append((f"Wo1_{l}_0", np.asarray(inputs["Wo1"], f32)[l, 0:128]))
        cols_b.append((f"Wo1_{l}_1", np.asarray(inputs["Wo1"], f32)[l, 128:256]))
    for l in range(L):
        cols_b.append((f"Wo2_{l}", np.asarray(inputs["Wo2"], f32)[l]))
    wb, wbmap = blob(cols_b)

    cols_r = [("Wh1", np.asarray(inputs["Wh1"], f32)),
              ("Wh2", np.asarray(inputs["Wh2"], f32)),
              ("idn", np.eye(128, dtype=f32))]
    wr, wrmap = blob(cols_r)

    A = {}
    A["wb"] = wb.astype(BF16)
    A["wr"] = wr
    A["_wbmap"] = wbmap
    A["_wrmap"] = wrmap
    A["_wbsplit"] = wb_split

    at = np.asarray(inputs["atom_emb"], f32).transpose(1, 0, 2).reshape(AV, 9 * H)
    A["atomb"] = np.ascontiguousarray(at).astype(BF16)

    # bias columns [128, 33]: 4 pre-terms x L, 2 o-terms x L, bh1, bh2, eps,
    # ln_s x L, ln_b x L, bp1 x L, bp2 x L (same layout as baseline)
    bc = np.zeros((H, 33), f32)
    bc[:, 26] = EPS
    bc[:, 27:30] = np.asarray(inputs["ln_s"], f32).T
    bc[:, 30:33] = np.asarray(inputs["ln_b"], f32).T
    for l in range(L):
        bc[:, 4 * l + 0] = np.asarray(inputs["bm1"], f32)[l]
        bc[:, 4 * l + 1] = np.asarray(inputs["bm2"], f32)[l]
        bc[:, 4 * l + 2] = np.asarray(inputs["be"], f32)[l]
        bc[:, 4 * l + 3] = np.asarray(inputs["bg"], f32)[l]
        bc[:, 12 + 2 * l + 0] = np.asarray(inputs["bo1"], f32)[l]
        bc[:, 12 + 2 * l + 1] = np.asarray(inputs["bo2"], f32)[l]
        bc[:, 18 + l] = np.asarray(inputs["bp1"], f32)[l]
        bc[:, 23 + l] = np.asarray(inputs["bp2"], f32)[l]
    bc[:, 21] = np.asarray(inputs["bh1"], f32)
    bc[:, 22] = np.asarray(inputs["bh2"], f32)[:H]
    A["bias_cols"] = bc
    A["bh2_full"] = np.ascontiguousarray(
        np.asarray(inputs["bh2"], f32).reshape(OUT, 1))
    bp2f = np.zeros((H, 4), f32)
    bp2f[:, :L] = np.asarray(inputs["bp2"], f32).T
    A["bp2f"] = bp2f.astype(BF16)
    return A


# --------------------------------------------------------------------------
# Bass program.
# --------------------------------------------------------------------------

def _build_program(struct, wbmap, wrmap, wbc, wrc, wbsplit):
    import concourse.bacc as bacc
    import concourse.mybir as mybir
    import concourse.tile as tile

    F32 = mybir.dt.float32
    nc = bacc.Bacc("TRN2", target_bir_lowering=False, debug=False)

    BF16 = mybir.dt.bfloat16
    FP8 = mybir.dt.float8e4
    F32R = mybir.dt.float32r
    S_core = struct["S_core"]
    d = {}
    d["d_gpair"] = nc.dram_tensor("gpair", [128, 2 * S_core], FP8,
                                  kind="ExternalInput")
    d["d_soh"] = nc.dram_tensor("soh", [24, 2 * S_core], FP8,
                                kind="ExternalInput")
    d["d_xoh"] = nc.dram_tensor("xoh", [9, AV, BL * N], FP8,
                                kind="ExternalInput")
    d["d_atomb"] = nc.dram_tensor("atomb", [AV, 9 * H], BF16,
                                  kind="ExternalInput")
    d["d_wb"] = nc.dram_tensor("wb", [128, wbc], BF16, kind="ExternalInput")
    d["d_wr"] = nc.dram_tensor("wr", [128, wrc], F32R, kind="ExternalInput")
    d["d_bc"] = nc.dram_tensor("bias_cols", [H, 33], F32, kind="ExternalInput")
    d["d_bh2"] = nc.dram_tensor("bh2_full", [OUT, 1], F32, kind="ExternalInput")
    d["d_bp2f"] = nc.dram_tensor("bp2f", [H, 4], BF16, kind="ExternalInput")
    d["d_mask"] = nc.dram_tensor("maskrow", [1, BL * N], F32,
                                 kind="ExternalInput")
    d["d_neg"] = nc.dram_tensor("negrow", [1, BL * N], F32,
                                kind="ExternalInput")
    d["d_out"] = nc.dram_tensor("out", [OUT, BL], F32, kind="ExternalOutput")

    with tile.TileContext(nc) as tc:
        _emit(tc, nc, d, struct, wbmap, wrmap, mybir, wbsplit)
    nc.compile()
    return nc


def _emit(tc, nc, d, struct, wbmap, wrmap, mybir, wbsplit):
    import contextlib
    from collections import defaultdict
    ctx = contextlib.ExitStack()
    F32 = mybir.dt.float32
    F32R = mybir.dt.float32r
    BF16 = mybir.dt.bfloat16
    FP8 = mybir.dt.float8e4
    AF = mybir.ActivationFunctionType
    ALU = mybir.AluOpType
    AX = mybir.AxisListType
    DR = mybir.MatmulPerfMode.DoubleRow

    S_graph = struct["S_graph"]
    S_core = struct["S_core"]
    n_tiles = struct["n_tiles"]
    groups = struct["groups"]
    tile_w = struct["tile_w"]
    tile_base = struct["tile_base"]
    has_empty = struct["has_empty"]

    pG = ctx.enter_context(tc.tile_pool(name="pG", bufs=1))
    pW = ctx.enter_context(tc.tile_pool(name="pW", bufs=1))
    pAct = ctx.enter_context(tc.tile_pool(name="pAct", bufs=3))
    pNM = ctx.enter_context(tc.tile_pool(name="pNM", bufs=1))
    pMB = ctx.enter_context(tc.tile_pool(name="pMB", bufs=2))
    pLN = ctx.enter_context(tc.tile_pool(name="pLN", bufs=1))
    pIn = ctx.enter_context(tc.tile_pool(name="pIn", bufs=2))
    ps_a = ctx.enter_context(tc.tile_pool(name="ps_a", bufs=3, space="PSUM"))
    ps_b = ctx.enter_context(tc.tile_pool(name="ps_b", bufs=3, space="PSUM"))
    ps_c = ctx.enter_context(tc.tile_pool(name="ps_c", bufs=2, space="PSUM"))

    def psA(dt=F32):
        return ps_a.tile([128, 512], dt, name="psA", tag="psA")

    def psB(dt=F32):
        return ps_b.tile([128, 512], dt, name="psB", tag="psB")

    def psC(dt=F32):
        return ps_c.tile([128, 512], dt, name="psC", tag="psC")

    # ---- resident tiles
    gpair_sb = pG.tile([128, 2 * S_core], FP8, name="gpair_sb")
    soh_sb = pG.tile([24, 2 * S_core], FP8, name="soh_sb")

    def k2(tile_sb, c0, w):
        """[p, 2*(c0..c0+w)] interleaved slice -> [p, k=2, s=w] AP."""
        return tile_sb[:, 2 * c0:2 * (c0 + w)].rearrange(
            "p (s k) -> p k s", k=2)

    wbc = sum(w for _, w in wbmap.values())
    wb_sb = pW.tile([128, wbc], BF16, name="wb_sb")
    wr_sb = pW.tile([128, sum(w for _, w in wrmap.values())], F32R,
                    name="wr_sb")
    bc_sb = pW.tile([H, 33], F32, name="bc_sb")
    bh2_sb = pW.tile([OUT, 1], F32, name="bh2_sb")
    bp2f_sb = pW.tile([H, 4], BF16, name="bp2f_sb")
    atomb_sb = pW.tile([AV, 9 * H], BF16, name="atomb_sb")

    def WB(name):
        off, w = wbmap[name]
        return wb_sb[:, off:off + w]

    def WR(name):
        off, w = wrmap[name]
        return wr_sb[:, off:off + w]

    # ---- DMAs, warmup-critical first
    nc.sync.dma_start(wb_sb[:, 0:wbsplit], d["d_wb"].ap()[:, 0:wbsplit])
    nc.sync.dma_start(bc_sb[:], d["d_bc"].ap())
    nc.sync.dma_start(atomb_sb[:], d["d_atomb"].ap())
    xoh_tiles = []
    for c in range(9):
        xoh_sb = pIn.tile([AV, BL * N], FP8, name="xoh_sb", tag="xoh",
                          bufs=9)
        nc.sync.dma_start(xoh_sb[:], d["d_xoh"].ap()[c])
        xoh_tiles.append(xoh_sb)
    sl = slice(0, 2 * S_graph)
    nc.sync.dma_start(gpair_sb[:, sl], d["d_gpair"].ap()[:, sl])
    nc.sync.dma_start(soh_sb[:, sl], d["d_soh"].ap()[:, sl])
    for gg in range(1, BL):
        sl = slice(2 * gg * S_graph, 2 * (gg + 1) * S_graph)
        nc.sync.dma_start(gpair_sb[:, sl], d["d_gpair"].ap()[:, sl])
        nc.sync.dma_start(soh_sb[:, sl], d["d_soh"].ap()[:, sl])
    nc.sync.dma_start(wb_sb[:, wbsplit:], d["d_wb"].ap()[:, wbsplit:])
    nc.sync.dma_start(wr_sb[:], d["d_wr"].ap())
    nc.sync.dma_start(bp2f_sb[:], d["d_bp2f"].ap())
    nc.sync.dma_start(bh2_sb[:], d["d_bh2"].ap())
    if has_empty:
        mrow_sb = pW.tile([1, BL * N], F32, name="mrow_sb")
        nc.sync.dma_start(mrow_sb[:], d["d_mask"].ap())
        nrow_sb = pW.tile([1, BL * N], F32, name="nrow_sb")
        nc.sync.dma_start(nrow_sb[:], d["d_neg"].ap())
        mask_bc = pW.tile([128, BL * N], F32, name="mask_bc")
        nc.gpsimd.partition_broadcast(mask_bc[:], mrow_sb[:])
        neg_bc = pW.tile([128, BL * N], F32, name="neg_bc")
        nc.gpsimd.partition_broadcast(neg_bc[:], nrow_sb[:])

    # pin the activation table to the set that has Relu+Copy+Identity+Sqrt
    sq_dummy = pW.tile([128, 1], F32, name="sq_dummy")
    nc.scalar.activation(sq_dummy[:], bc_sb[:, 26:27], AF.Sqrt)

    # bias prework (bc only)
    bias_pre = pW.tile([128, L], F32, name="bias_pre")
    nc.vector.tensor_reduce(
        bias_pre[:], bc_sb[:, 0:4 * L].rearrange("p (l f) -> p l f", l=L),
        axis=AX.X, op=ALU.add)
    bo12 = pW.tile([128, L], F32, name="bo12")
    nc.vector.tensor_reduce(
        bo12[:], bc_sb[:, 12:12 + 2 * L].rearrange("p (l f) -> p l f", l=L),
        axis=AX.X, op=ALU.add)

    # ---- node features (feature-major), bf16
    nf_ps = psA()
    for c in range(9):
        nc.tensor.matmul(nf_ps[:], atomb_sb[:, c * H:(c + 1) * H],
                         xoh_tiles[c][:], start=(c == 0), stop=(c == 8))
    nf = pNM.tile([128, BL * N], BF16, name="nf")
    nc.scalar.activation(nf[:], nf_ps[:], AF.Copy)

    # layer-0 m12 (z = [nf, 0]) + fp8 copies, and layer-0 bond weights
    def emit_bw(l):
        bw_ps = psB()
        nc.tensor.matmul(bw_ps[0:24, 0:128], WB("bondT")[:, 0:24],
                         WB(f"We_{l}"), start=True, stop=True)
        nc.tensor.matmul(bw_ps[0:24, 128:256], WB("bondT")[:, 24:48],
                         WB(f"We_{l}"), start=True, stop=True,
                         skip_group_check=True)
        bw24 = pMB.tile([24, 256], FP8, name="bw24", tag=f"bw24_{l}", bufs=1)
        nc.scalar.activation(bw24[:], bw_ps[0:24, 0:256], AF.Copy)
        return bw24

    bw24_l = {0: emit_bw(0)}

    mt_l = {0: pMB.tile([128, BL * 256], FP8, name="mt0", tag="mt", bufs=2)}
    for half in range(2):
        ps = psA()
        for gg in (2 * half, 2 * half + 1):
            off = (gg % 2) * 256
            gsl = slice(gg * N, (gg + 1) * N)
            nc.tensor.matmul(ps[:, off:off + 256], nf[:, gsl],
                             WB("m12_0_0"), start=True, stop=True)
        nc.scalar.activation(mt_l[0][:, half * 512:(half + 1) * 512],
                             ps[:, 0:512], AF.Copy)

    # relu2 engine split (Bresenham across all layers)
    state = dict(relu2_acc=0.0)

    def emit_unit_pre(l, gg, t):
        w = tile_w[t]
        c0 = gg * S_graph + tile_base[t]
        pre = psA()
        mt_g = mt_l[l][:, gg * 256:(gg + 1) * 256].rearrange(
            "p (k f) -> p k f", k=2)
        nc.tensor.matmul(pre[:, 0:w], mt_g, k2(gpair_sb, c0, w),
                         start=True, stop=False, perf_mode=DR)
        bw_k = bw24_l[l][:].rearrange("p (k f) -> p k f", k=2)
        nc.tensor.matmul(pre[:, 0:w], bw_k, k2(soh_sb, c0, w),
                         start=False, stop=True, perf_mode=DR)
        msgs1 = pAct.tile([128, 512], BF16, name="msgs1", tag="msgs1",
                          bufs=3)
        nc.scalar.activation(msgs1[:, 0:w], pre[:, 0:w], AF.Relu,
                             bias=bias_pre[:, l:l + 1])
        return msgs1

    def emit_unit_p1(l, gg, t, msgs1):
        w = tile_w[t]
        p1 = psB()
        nc.tensor.matmul(p1[:, 0:w], WB(f"Wp1_{l}"), msgs1[:, 0:w],
                         start=True, stop=True)
        msgs2 = pAct.tile([128, 512], BF16, name="msgs2", tag="msgs2",
                          bufs=3)
        state["relu2_acc"] += ACT_RELU2_SHARE
        if state["relu2_acc"] >= 1.0:
            state["relu2_acc"] -= 1.0
            nc.scalar.activation(msgs2[:, 0:w], p1[:, 0:w], AF.Relu,
                                 bias=bc_sb[:, 18 + l:19 + l])
        else:
            nc.vector.tensor_scalar(msgs2[:, 0:w], p1[:, 0:w],
                                    bc_sb[:, 18 + l:19 + l], 0.0,
                                    op0=ALU.add, op1=ALU.max)
        return msgs2

    def emit_unit_p2(l, gg, t, msgs2, msgs_max):
        w = tile_w[t]
        p2 = psC()
        nc.tensor.matmul(p2[:, 0:w], WB(f"Wp2_{l}"), msgs2[:, 0:w],
                         start=True, stop=True)
        for (p0, R, K, gt, off) in groups:
            if gt != t:
                continue
            nc.vector.tensor_reduce(
                msgs_max[:, gg * N + p0: gg * N + p0 + R],
                p2[:, off:off + R * K].rearrange("p (r k) -> p r k", r=R),
                axis=AX.X, op=ALU.max)

    bias_h_l = {}

    hid = {0: None}
    for l in range(L):
        msgs_max = pLN.tile([128, BL * N], BF16, name="msgs_max",
                            tag="msgs_max", bufs=2)
        hn = pLN.tile([128, BL * N], F32, name="hn", tag="hn", bufs=1)
        hsq = pLN.tile([128, BL * N], BF16, name="hsq", tag="hsq", bufs=1)
        sumh = pLN.tile([128, BL], F32, name="sumh", tag="sumh")
        sumsq = pLN.tile([128, BL], F32, name="sumsq", tag="sumsq")
        negmean = pLN.tile([128, BL], F32, name="negmean", tag="negmean")
        msq = pLN.tile([128, BL], F32, name="msq", tag="msq")
        var = pLN.tile([128, BL], F32, name="var", tag="var")
        std = pLN.tile([128, BL], F32, name="std", tag="std")
        rstd = pLN.tile([128, BL], F32, name="rstd", tag="rstd")
        h_fm = pLN.tile([128, BL * N], F32R, name="h_fm", tag="h_fm", bufs=1)
        hid_new = pNM.tile([128, BL * N], BF16, name=f"hid{l + 1}",
                           tag=f"hid{(l + 1) % 2}")
        hid[l + 1] = hid_new
        lref = l

        def mk_block_h(pair, l=lref):
            def fn():
                if l not in bias_h_l:
                    if has_empty:
                        bias_h_l[l] = bo12[:, l:l + 1]
                    else:
                        bh_ps = psC()
                        nc.tensor.matmul(bh_ps[:, 0:2], WB(f"Wo2_{l}"),
                                         bp2f_sb[:, l:l + 2],
                                         start=True, stop=True)
                        bh = pMB.tile([128, 1], F32, name="bias_h",
                                      tag=f"bias_h{l}", bufs=1)
                        nc.vector.tensor_tensor(bh[:], bh_ps[:, 0:1],
                                                bo12[:, l:l + 1], op=ALU.add)
                        bias_h_l[l] = bh[:]
                msgs_src = msgs_max
                if has_empty:
                    mmf = pLN.tile([128, BL * N], F32, name="mmf", tag="mmf",
                                   bufs=1)
                    nc.vector.scalar_tensor_tensor(
                        mmf[:], msgs_max[:], bc_sb[:, 23 + l:24 + l],
                        mask_bc[:], op0=ALU.add, op1=ALU.mult)
                    nc.vector.tensor_tensor(mmf[:], mmf[:], neg_bc[:],
                                            op=ALU.add)
                    mmb = pLN.tile([128, BL * N], BF16, name="mmb", tag="mmb",
                                   bufs=1)
                    nc.scalar.activation(mmb[:], mmf[:], AF.Copy)
                    msgs_src = mmb
                g0 = pair[0]
                psl = slice(g0 * N, (g0 + 2) * N)
                h_ps = psA()
                nc.tensor.matmul(h_ps[:, 0:256], WB(f"Wo1_{l}_0"),
                                 nf[:, psl], start=True, stop=False)
                if l > 0:
                    nc.tensor.matmul(h_ps[:, 0:256], WB(f"Wo1_{l}_1"),
                                     hid[l][:, psl], start=False, stop=False)
                nc.tensor.matmul(h_ps[:, 0:256], WB(f"Wo2_{l}"),
                                 msgs_src[:, psl], start=False, stop=True)
                nc.scalar.activation(h_fm[:, psl], h_ps[:, 0:256], AF.Relu,
                                     bias=bias_h_l[l])
            return fn

        def mk_block_tp(pair, l=lref):
            def fn():
                g0 = pair[0]
                psl = slice(g0 * N, (g0 + 2) * N)
                ssl = slice(g0, g0 + 2)
                tp_ps = psB(F32R)
                for i, gg in enumerate(pair):
                    nc.tensor.transpose(
                        tp_ps[:, i * 128:(i + 1) * 128],
                        h_fm[:, gg * N:(gg + 1) * N], WR("idn"))
                nc.vector.tensor_scalar(hn[:, psl],
                                        tp_ps[:, 0:256].bitcast(F32),
                                        0.0, None, op0=ALU.add)
                nc.vector.tensor_reduce(
                    sumh[:, ssl],
                    hn[:, psl].rearrange("p (g n) -> p g n", g=2),
                    axis=AX.X, op=ALU.add)
                nc.vector.scalar_tensor_tensor(
                    hsq[:, psl], hn[:, psl], 0.0, hn[:, psl],
                    op0=ALU.add, op1=ALU.mult)
                nc.vector.tensor_reduce(
                    sumsq[:, ssl],
                    hsq[:, psl].rearrange("p (g n) -> p g n", g=2),
                    axis=AX.X, op=ALU.add)
                nc.vector.tensor_scalar(negmean[:, ssl], sumh[:, ssl],
                                        -1.0 / H, None, op0=ALU.mult)
                nc.vector.tensor_tensor(msq[:, ssl], negmean[:, ssl],
                                        negmean[:, ssl], op=ALU.mult)
                nc.vector.scalar_tensor_tensor(
                    var[:, ssl], sumsq[:, ssl], 1.0 / H, msq[:, ssl],
                    op0=ALU.mult, op1=ALU.subtract)
                nc.scalar.activation(std[:, ssl], var[:, ssl], AF.Sqrt,
                                     bias=bc_sb[:, 26:27])
                nc.vector.reciprocal(rstd[:, ssl], std[:, ssl])
            return fn

        def mk_block_norm(pair, l=lref):
            def fn():
                g0 = pair[0]
                psl = slice(g0 * N, (g0 + 2) * N)
                tp2_ps = psC(F32R)
                for i, gg in enumerate(pair):
                    hnorm = pLN.tile([128, 128], F32R, name="hnorm",
                                     tag="hnorm", bufs=2)
                    nc.gpsimd.tensor_scalar(hnorm[:],
                                            hn[:, gg * N:(gg + 1) * N],
                                            negmean[:, gg:gg + 1],
                                            rstd[:, gg:gg + 1],
                                            op0=ALU.add, op1=ALU.mult)
                    nc.tensor.transpose(tp2_ps[:, i * 128:(i + 1) * 128],
                                        hnorm[:], WR("idn"))
                nc.vector.tensor_scalar(hid_new[:, psl],
                                        tp2_ps[:, 0:256].bitcast(F32),
                                        bc_sb[:, 27 + l:28 + l],
                                        bc_sb[:, 30 + l:31 + l],
                                        op0=ALU.mult, op1=ALU.add)
            return fn

        def mk_block_next(pair, l=lref):
            def fn():
                nl = l + 1
                if nl >= L:
                    return
                if nl not in bw24_l:
                    bw24_l[nl] = emit_bw(nl)
                    mt_l[nl] = pMB.tile([128, BL * 256], FP8,
                                        name=f"mt{nl}", tag="mt", bufs=2)
                ps = psA()
                for i, gg in enumerate(pair):
                    off = i * 256
                    gsl = slice(gg * N, (gg + 1) * N)
                    nc.tensor.matmul(ps[:, off:off + 256], nf[:, gsl],
                                     WB(f"m12_{nl}_0"), start=True,
                                     stop=False)
                    nc.tensor.matmul(ps[:, off:off + 256],
                                     hid[nl][:, gsl],
                                     WB(f"m12_{nl}_1"), start=False,
                                     stop=True)
                g0 = pair[0]
                nc.scalar.activation(
                    mt_l[nl][:, g0 * 256:(g0 + 2) * 256], ps[:, 0:512],
                    AF.Copy)
            return fn

        units = [(gg, t) for gg in range(BL) for t in range(n_tiles)]
        nU = len(units)
        post = defaultdict(list)
        for pair in ((0, 1), (2, 3)):
            s0 = (pair[1] + 1) * n_tiles - 1 + 4
            post[s0 + 1].append(mk_block_h(pair))
            post[s0 + 2].append(mk_block_tp(pair))
            post[s0 + 3].append(mk_block_norm(pair))
            post[s0 + 4].append(mk_block_next(pair))
        st1, st2 = {}, {}
        for step in range(nU + 9):
            if step < nU:
                gg, t = units[step]
                st1[step] = emit_unit_pre(l, gg, t)
            if 0 <= step - 2 < nU:
                u = step - 2
                gg, t = units[u]
                st2[u] = emit_unit_p1(l, gg, t, st1.pop(u))
            if 0 <= step - 4 < nU:
                u = step - 4
                gg, t = units[u]
                emit_unit_p2(l, gg, t, st2.pop(u), msgs_max)
            for fn in post.pop(step, []):
                fn()

    # ---- pooling + prediction MLP
    ge_sum = pLN.tile([128, BL], F32, name="ge_sum", tag="ge_sum")
    nc.vector.tensor_reduce(
        ge_sum[:], hid[L][:].rearrange("p (g n) -> p g n", g=BL),
        axis=AX.X, op=ALU.add)
    ge = pLN.tile([128, BL], F32R, name="ge", tag="ge")
    nc.scalar.activation(ge[:], ge_sum[:], AF.Copy, scale=1.0 / N)
    o1 = psA()
    nc.tensor.matmul(o1[:, 0:BL], WR("Wh1"), ge[:], start=True, stop=True)
    t1 = pLN.tile([128, BL], F32R, name="t1", tag="t1")
    nc.scalar.activation(t1[:], o1[:, 0:BL], AF.Relu, bias=bc_sb[:, 21:22])
    o2 = psB()
    nc.tensor.matmul(o2[:, 0:BL], WR("Wh2"), t1[:], start=True, stop=True)
    out_sb = pLN.tile([OUT, BL], F32, name="out_sb", tag="out_sb")
    nc.scalar.activation(out_sb[:], o2[:, 0:BL], AF.Identity, bias=bh2_sb[:])
    nc.sync.dma_start(d["d_out"].ap(), out_sb[:])
    ctx.close()


# --------------------------------------------------------------------------
# Entry point.
# --------------------------------------------------------------------------

def build(inputs):
    struct, percore = _prep(inputs)
    A = _weight_arrays(inputs)
    wbmap = A.pop("_wbmap")
    wrmap = A.pop("_wrmap")
    wbsplit = A.pop("_wbsplit")
    key = (struct["S_graph"], struct["n_tiles"],
           tuple(struct["groups"]), struct["tile_w"], struct["has_empty"])
    if key not in _CACHE:
        _CACHE[key] = _build_program(struct, wbmap, wrmap,
                                     A["wb"].shape[1], A["wr"].shape[1],
                                     wbsplit)
    nc = _CACHE[key]

    in_maps = []
    for c in range(M):
        im = dict(
            gpair=percore["Gpair"][c], soh=percore["SOH"][c],
            xoh=percore["XOH"][c],
            maskrow=percore["maskrow"][c:c + 1],
            negrow=percore["negrow"][c:c + 1],
        )
        for k, v in A.items():
            im[k] = v
        in_maps.append(im)
    return nc, in_maps, struct


def kernel(**inputs):
    from concourse import bass_utils
    nc, in_maps, struct = build(inputs)
    res = bass_utils.run_bass_kernel_spmd(nc, in_maps, core_ids=list(range(M)))
    out = np.zeros((B, OUT), np.float32)
    for c in range(M):
        out[c * BL:(c + 1) * BL] = res.results[c]["out"].T
    return out


# revision 38
# speedup vs baseline: 1.1699x; 1.0488x over previous
"""Trainium2 Bass kernel for nn_BaselineModel_74509092651544 (CLRS-style MPNN).

Strategy
--------
Data-parallel over graphs: 32 graphs -> 8 cores x 4 graphs.  The dense
[B,N,N,H] message tensor is never materialized: only the ~61k unique
(graph,src,dst) edge slots survive the masked max, so the message MLP runs
on a padded CSR slot layout.

v2 (this file) vs the earlier baseline:
  * fp8 DoubleRow matmuls for the gather stage: m1/m2 (quantized fp8e4m3)
    ride as a 2-k-tile stationary pair against an interleaved Gsrc/Gdst
    one-hot moving tensor; the bond term uses a 24+24 row split of the
    one-hot counts.  1 cycle/slot instead of 3 on the PE.
  * bf16 weights/activations everywhere precision allows (validated
    ~7e-3 rel err vs the 2e-2 gate).
  * h-matmuls and LayerNorm batched across the 4 graphs per layer
    (512-wide) instead of per-graph 128-wide (f32r <256 runs at 1/4 rate).
  * DP-optimized receiver grouping (minimizes padded slots + per-group
    DVE reduce overhead).
  * Software-pipelined slot loop (skew 2) with PSUM rings 3/3/2.
  * relu1 on ACT, relu2 split ACT/DVE, segmented max on DVE.
"""

import sys
import numpy as np

sys.path.insert(0, "/opt/trn_rl_repo")

B, N, H, L, E, OUT = 32, 128, 128, 3, 65536, 128
M = 8                 # NeuronCores
BL = B // M           # graphs per core
NEG = -1e9
EPS = 1e-5
AV, BV = 128, 16
ACT_RELU2_SHARE = 0.62   # fraction of relu2 ops on ACT (rest on DVE)

_CACHE = {}


# --------------------------------------------------------------------------
# Host preprocessing: pure integer / relayout work.
# --------------------------------------------------------------------------

def _ffd_pack(groups):
    sizes = [R * K for (_, R, K) in groups]
    order = np.argsort(-np.asarray(sizes), kind="stable")
    bins, place = [], [None] * len(groups)
    for gi in order:
        sz = sizes[gi]
        for t in range(len(bins)):
            if bins[t] + sz <= 512:
                place[gi] = (t, bins[t])
                bins[t] += sz
                break
        else:
            place[gi] = (len(bins), 0)
            bins.append(sz)
    return bins, place


def _dp_groups(Kp, c_slot, c_group, max_r=128):
    n = len(Kp)
    INF = float("inf")
    dp = [INF] * (n + 1)
    dp[n] = 0.0
    choice = [1] * (n + 1)
    for p in range(n - 1, -1, -1):
        K = int(Kp[p])
        mr = min(n - p, 512 // K, max_r)
        best, best_r = INF, 1
        for R in range(1, mr + 1):
            c = R * K * c_slot + c_group + dp[p + R]
            if c < best:
                best, best_r = c, R
        dp[p], choice[p] = best, best_r
    groups = []
    p = 0
    while p < n:
        R = choice[p]
        groups.append((p, R, int(Kp[p])))
        p += R
    return groups


def _fixed_groups(Kp, r0):
    groups, p, n = [], 0, len(Kp)
    while p < n:
        K = int(Kp[p])
        R = min(r0, n - p)
        while R * K > 512:
            R //= 2
        groups.append((p, R, K))
        p += R
    return groups


def _choose_groups(Kp):
    """Pick the candidate minimizing a per-graph-layer time proxy (ns)."""
    cands = [_fixed_groups(Kp, 16),
             _dp_groups(Kp, 4.0, 100.0),
             _dp_groups(Kp, 4.0, 170.0, 16),
             _dp_groups(Kp, 2.5, 170.0)]
    best, best_c = None, float("inf")
    for gs in cands:
        bins, _ = _ffd_pack(gs)
        S = sum(R * K for (_, R, K) in gs)
        c = S * 4.04 + len(gs) * 170.0 + len(bins) * 395.0
        if c < best_c:
            best, best_c = gs, c
    return best


def _prep(inputs):
    x = np.asarray(inputs["x"]).astype(np.int64)            # [B*N, 9]
    ea = np.asarray(inputs["edge_attr"]).astype(np.int64)   # [E, 3]
    ei = np.asarray(inputs["edge_index"]).astype(np.int64)  # [2, E]

    g = ei[0] // N
    s = ei[0] % N
    d = ei[1] % N
    key = (g * N + s) * N + d
    uniq, inv = np.unique(key, return_inverse=True)
    US = uniq.size
    ug = uniq // (N * N)
    us = (uniq // N) % N
    ud = uniq % N

    # bond one-hot counts per unique slot  [US, 48]
    oh48 = np.zeros((US, 48), np.float32)
    for c in range(3):
        np.add.at(oh48, (inv, ea[:, c] + 16 * c), 1.0)

    # unique in-degree per (graph, receiver)
    deg = np.zeros((B, N), np.int64)
    np.add.at(deg, (ug, ud), 1)

    # receiver relabeling: position p holds the p-th highest-degree receiver
    rho = np.argsort(-deg, axis=1, kind="stable")        # [B, N] pos -> orig
    rho_inv = np.argsort(rho, axis=1)                    # orig -> pos
    degS = -np.sort(-deg, axis=1)                        # [B, N] desc
    Kp = np.maximum(degS.max(axis=0), 1)                 # [N] non-increasing

    groups = _choose_groups(Kp)                          # (p0, R, K)
    tiles_used, place = _ffd_pack(groups)
    n_tiles = len(tiles_used)
    tile_w = [int(w) for w in tiles_used]
    tile_base = np.concatenate([[0], np.cumsum(tile_w)[:-1]]).astype(np.int64)
    S_graph = int(((sum(tile_w) + 15) // 16) * 16)
    S_core = BL * S_graph

    # per-position lookup tables
    col_base_of_pos = np.zeros(N, np.int64)   # first column of the receiver
    K_of_pos = np.zeros(N, np.int64)
    for gi, (p0, R, K) in enumerate(groups):
        t, off = place[gi]
        for r in range(R):
            col_base_of_pos[p0 + r] = tile_base[t] + off + r * K
            K_of_pos[p0 + r] = K

    # slots ordered by (g, d, s): contiguous per receiver
    order = np.lexsort((us, ud, ug))
    og, od, osl = ug[order], ud[order], order
    osrc = us[order]
    recv_id = og * N + od
    first = np.concatenate([[0], np.flatnonzero(np.diff(recv_id)) + 1])
    k_rank = np.arange(len(og)) - first[np.searchsorted(recv_id[first], recv_id)]

    pos = rho_inv[og, od]
    core_r = og // BL
    col_r = (og % BL) * S_graph + col_base_of_pos[pos] + k_rank

    # padding: receivers with deg < K duplicate their first slot
    fg, fd = og[first], od[first]
    fpos = rho_inv[fg, fd]
    fdeg = deg[fg, fd]
    fK = K_of_pos[fpos]
    padc = (fK - fdeg).astype(np.int64)
    assert (padc >= 0).all()
    rep = np.repeat(np.arange(len(first)), padc)
    kpad = np.arange(len(rep)) - np.repeat(
        np.concatenate([[0], np.cumsum(padc)[:-1]]), padc
    ) + np.repeat(fdeg, padc)
    pg = fg[rep]
    core_p = pg // BL
    col_p = (pg % BL) * S_graph + col_base_of_pos[fpos[rep]] + kpad
    slot_p = osl[first][rep]
    src_p = osrc[first][rep]

    a_core = np.concatenate([core_r, core_p])
    a_col = np.concatenate([col_r, col_p])
    a_slot = np.concatenate([osl, slot_p])
    a_srcnew = np.concatenate([rho_inv[og, osrc], rho_inv[pg, src_p]])
    a_dstpos = np.concatenate([pos, fpos[rep]])

    import ml_dtypes
    FP8 = ml_dtypes.float8_e4m3fn
    flat = a_core * S_core + a_col
    # column-interleaved src/dst one-hot k-tiles: [M, 128, S_core*2]
    # (column s occupies bytes [2s, 2s+1]: k=0 -> Gsrc, k=1 -> Gdst)
    Gpair = np.zeros((M * S_core, 2, 128), np.float32)
    Gpair[flat, 0, a_srcnew] = 1.0
    Gpair[flat, 1, a_dstpos] = 1.0
    Gpair = np.ascontiguousarray(
        Gpair.reshape(M, S_core, 2, 128).transpose(0, 3, 1, 2)
        .reshape(M, 128, 2 * S_core)).astype(FP8)
    # bond count k-tiles (rows 0:24 / 24:48), column-interleaved:
    # [M, 24, S_core*2]
    SOH = np.zeros((M * S_core, 48), np.float32)
    SOH[flat] = oh48[a_slot]
    assert float(SOH.max()) <= 16.0
    SOH = np.ascontiguousarray(
        SOH.reshape(M, S_core, 2, 24).transpose(0, 3, 1, 2)
        .reshape(M, 24, 2 * S_core)).astype(FP8)

    # atom one-hot per core: [M, 9, AV, BL*N] in relabeled node order
    gg = np.repeat(np.arange(B), N)
    pp = np.tile(np.arange(N), B)
    orig = gg * N + rho[gg, pp]                    # [B*N] column -> orig node
    XOH = np.zeros((M, 9, AV, BL * N), np.float32)
    mcol = np.tile(np.arange(BL * N), M)
    mcore = np.repeat(np.arange(M), BL * N)
    for c in range(9):
        XOH[mcore, c, x[orig, c], mcol] = 1.0
    # one flat [AV, 9*BL*N] tensor per core -> single DMA
    XOH = np.ascontiguousarray(
        XOH.transpose(0, 2, 1, 3).reshape(M, AV, 9 * BL * N)).astype(FP8)

    # empty receivers (deg==0) -> need NEG mask path
    empt = (deg == 0)
    has_empty = bool(empt.any())
    maskrow = np.ones((M, BL * N), np.float32)
    negrow = np.zeros((M, BL * N), np.float32)
    if has_empty:
        eg, en = np.nonzero(empt)
        epos = rho_inv[eg, en]
        maskrow[eg // BL, (eg % BL) * N + epos] = 0.0
        negrow[eg // BL, (eg % BL) * N + epos] = NEG

    struct = dict(
        S_graph=S_graph, S_core=S_core, n_tiles=n_tiles,
        groups=[(p0, R, K, place[gi][0], place[gi][1])
                for gi, (p0, R, K) in enumerate(groups)],
        tile_w=tuple(tile_w), tile_base=tuple(int(b) for b in tile_base),
        has_empty=has_empty,
    )
    percore = dict(Gpair=Gpair, SOH=SOH, XOH=XOH,
                   maskrow=maskrow, negrow=negrow)
    return struct, percore


def _weight_arrays(inputs):
    import ml_dtypes
    BF16 = ml_dtypes.bfloat16
    f32 = np.float32

    def blob(cols):
        wmap = {}
        off = 0
        for name, arr in cols:
            wmap[name] = (off, arr.shape[1])
            off += arr.shape[1]
        data = np.concatenate([a for _, a in cols], 1)
        return np.ascontiguousarray(data), wmap

    Wm1 = np.asarray(inputs["Wm1"], f32)
    Wm2 = np.asarray(inputs["Wm2"], f32)
    bond_T = np.asarray(inputs["bond_emb"], f32).reshape(48, H).T  # [128, 48]
    # chunk 1: everything layer-0's slot phase needs (DMA'd first)
    cols_b = [("bondT", bond_T),
              ("We_0", np.asarray(inputs["We"], f32)[0]),
              ("m12_0_0",
               np.concatenate([Wm1[0, 0:128], Wm2[0, 0:128]], 1)),
              ("Wp1_0", np.asarray(inputs["Wp1"], f32)[0]),
              ("Wp2_0", np.asarray(inputs["Wp2"], f32)[0])]
    wb_split = sum(a.shape[1] for _, a in cols_b)
    # chunk 2: the rest (lands before layer-0's h phase)
    cols_b.append(("Wo1_0_0", np.asarray(inputs["Wo1"], f32)[0, 0:128]))
    for l in range(1, L):
        cols_b.append((f"We_{l}", np.asarray(inputs["We"], f32)[l]))
        cols_b.append((f"m12_{l}_0",
                       np.concatenate([Wm1[l, 0:128], Wm2[l, 0:128]], 1)))
        cols_b.append((f"m12_{l}_1",
                       np.concatenate([Wm1[l, 128:256], Wm2[l, 128:256]], 1)))
        cols_b.append((f"Wp1_{l}", np.asarray(inputs["Wp1"], f32)[l]))
        cols_b.append((f"Wp2_{l}", np.asarray(inputs["Wp2"], f32)[l]))
        cols_b.append((f"Wo1_{l}_0", np.asarray(inputs["Wo1"], f32)[l, 0:128]))
        cols_b.append((f"Wo1_{l}_1", np.asarray(inputs["Wo1"], f32)[l, 128:256]))
    for l in range(L):
        cols_b.append((f"Wo2_{l}", np.asarray(inputs["Wo2"], f32)[l]))
    wb, wbmap = blob(cols_b)

    cols_r = [("Wh1", np.asarray(inputs["Wh1"], f32)),
              ("Wh2", np.asarray(inputs["Wh2"], f32)),
              ("idn", np.eye(128, dtype=f32))]
    wr, wrmap = blob(cols_r)

    A = {}
    A["wb"] = wb.astype(BF16)
    A["wr"] = wr
    A["_wbmap"] = wbmap
    A["_wrmap"] = wrmap
    A["_wbsplit"] = wb_split

    at = np.asarray(inputs["atom_emb"], f32).transpose(1, 0, 2).reshape(AV, 9 * H)
    A["atomb"] = np.ascontiguousarray(at).astype(BF16)

    # bias columns [128, 33]: 4 pre-terms x L, 2 o-terms x L, bh1, bh2, eps,
    # ln_s x L, ln_b x L, bp1 x L, bp2 x L (same layout as baseline)
    bc = np.zeros((H, 33), f32)
    bc[:, 26] = EPS
    bc[:, 27:30] = np.asarray(inputs["ln_s"], f32).T
    bc[:, 30:33] = np.asarray(inputs["ln_b"], f32).T
    for l in range(L):
        bc[:, 4 * l + 0] = np.asarray(inputs["bm1"], f32)[l]
        bc[:, 4 * l + 1] = np.asarray(inputs["bm2"], f32)[l]
        bc[:, 4 * l + 2] = np.asarray(inputs["be"], f32)[l]
        bc[:, 4 * l + 3] = np.asarray(inputs["bg"], f32)[l]
        bc[:, 12 + 2 * l + 0] = np.asarray(inputs["bo1"], f32)[l]
        bc[:, 12 + 2 * l + 1] = np.asarray(inputs["bo2"], f32)[l]
        bc[:, 18 + l] = np.asarray(inputs["bp1"], f32)[l]
        bc[:, 23 + l] = np.asarray(inputs["bp2"], f32)[l]
    bc[:, 21] = np.asarray(inputs["bh1"], f32)
    bc[:, 22] = np.asarray(inputs["bh2"], f32)[:H]
    A["bias_cols"] = bc
    A["bh2_full"] = np.ascontiguousarray(
        np.asarray(inputs["bh2"], f32).reshape(OUT, 1))
    bp2f = np.zeros((H, 4), f32)
    bp2f[:, :L] = np.asarray(inputs["bp2"], f32).T
    A["bp2f"] = bp2f.astype(BF16)
    return A


# --------------------------------------------------------------------------
# Bass program.
# --------------------------------------------------------------------------

def _build_program(struct, wbmap, wrmap, wbc, wrc, wbsplit):
    import concourse.bacc as bacc
    import concourse.mybir as mybir
    import concourse.tile as tile

    F32 = mybir.dt.float32
    nc = bacc.Bacc("TRN2", target_bir_lowering=False, debug=False)

    BF16 = mybir.dt.bfloat16
    FP8 = mybir.dt.float8e4
    F32R = mybir.dt.float32r
    S_core = struct["S_core"]
    d = {}
    d["d_gpair"] = nc.dram_tensor("gpair", [128, 2 * S_core], FP8,
                                  kind="ExternalInput")
    d["d_soh"] = nc.dram_tensor("soh", [24, 2 * S_core], FP8,
                                kind="ExternalInput")
    d["d_xoh"] = nc.dram_tensor("xoh", [AV, 9 * BL * N], FP8,
                                kind="ExternalInput")
    d["d_atomb"] = nc.dram_tensor("atomb", [AV, 9 * H], BF16,
                                  kind="ExternalInput")
    d["d_wb"] = nc.dram_tensor("wb", [128, wbc], BF16, kind="ExternalInput")
    d["d_wr"] = nc.dram_tensor("wr", [128, wrc], F32R, kind="ExternalInput")
    d["d_bc"] = nc.dram_tensor("bias_cols", [H, 33], F32, kind="ExternalInput")
    d["d_bh2"] = nc.dram_tensor("bh2_full", [OUT, 1], F32, kind="ExternalInput")
    d["d_bp2f"] = nc.dram_tensor("bp2f", [H, 4], BF16, kind="ExternalInput")
    d["d_mask"] = nc.dram_tensor("maskrow", [1, BL * N], F32,
                                 kind="ExternalInput")
    d["d_neg"] = nc.dram_tensor("negrow", [1, BL * N], F32,
                                kind="ExternalInput")
    d["d_out"] = nc.dram_tensor("out", [OUT, BL], F32, kind="ExternalOutput")

    with tile.TileContext(nc) as tc:
        _emit(tc, nc, d, struct, wbmap, wrmap, mybir, wbsplit)
    nc.compile()
    return nc


def _emit(tc, nc, d, struct, wbmap, wrmap, mybir, wbsplit):
    import contextlib
    from collections import defaultdict
    ctx = contextlib.ExitStack()
    F32 = mybir.dt.float32
    F32R = mybir.dt.float32r
    BF16 = mybir.dt.bfloat16
    FP8 = mybir.dt.float8e4
    AF = mybir.ActivationFunctionType
    ALU = mybir.AluOpType
    AX = mybir.AxisListType
    DR = mybir.MatmulPerfMode.DoubleRow

    S_graph = struct["S_graph"]
    S_core = struct["S_core"]
    n_tiles = struct["n_tiles"]
    groups = struct["groups"]
    tile_w = struct["tile_w"]
    tile_base = struct["tile_base"]
    has_empty = struct["has_empty"]

    pG = ctx.enter_context(tc.tile_pool(name="pG", bufs=1))
    pW = ctx.enter_context(tc.tile_pool(name="pW", bufs=1))
    pAct = ctx.enter_context(tc.tile_pool(name="pAct", bufs=3))
    pNM = ctx.enter_context(tc.tile_pool(name="pNM", bufs=1))
    pMB = ctx.enter_context(tc.tile_pool(name="pMB", bufs=2))
    pLN = ctx.enter_context(tc.tile_pool(name="pLN", bufs=1))
    pIn = ctx.enter_context(tc.tile_pool(name="pIn", bufs=2))
    ps_a = ctx.enter_context(tc.tile_pool(name="ps_a", bufs=2, space="PSUM"))
    ps_b = ctx.enter_context(tc.tile_pool(name="ps_b", bufs=2, space="PSUM"))
    ps_c = ctx.enter_context(tc.tile_pool(name="ps_c", bufs=2, space="PSUM"))

    def psA(dt=F32):
        """Double-bank tile: pre pairs (also m12/h_ps/nf_ps, half-used)."""
        return ps_a.tile([128, 1024], dt, name="psA", tag="psA")

    def psB(dt=F32):
        return ps_b.tile([128, 512], dt, name="psB", tag="psB")

    def psC(dt=F32):
        return ps_c.tile([128, 512], dt, name="psC", tag="psC")

    # ---- resident tiles
    gpair_sb = pG.tile([128, 2 * S_core], FP8, name="gpair_sb")
    soh_sb = pG.tile([24, 2 * S_core], FP8, name="soh_sb")

    def k2(tile_sb, c0, w):
        """[p, 2*(c0..c0+w)] interleaved slice -> [p, k=2, s=w] AP."""
        return tile_sb[:, 2 * c0:2 * (c0 + w)].rearrange(
            "p (s k) -> p k s", k=2)

    wbc = sum(w for _, w in wbmap.values())
    wb_sb = pW.tile([128, wbc], BF16, name="wb_sb")
    wr_sb = pW.tile([128, sum(w for _, w in wrmap.values())], F32R,
                    name="wr_sb")
    bc_sb = pW.tile([H, 33], F32, name="bc_sb")
    bh2_sb = pW.tile([OUT, 1], F32, name="bh2_sb")
    bp2f_sb = pW.tile([H, 4], BF16, name="bp2f_sb")
    atomb_sb = pW.tile([AV, 9 * H], BF16, name="atomb_sb")

    def WB(name):
        off, w = wbmap[name]
        return wb_sb[:, off:off + w]

    def WR(name):
        off, w = wrmap[name]
        return wr_sb[:, off:off + w]

    # ---- DMAs, warmup-critical first
    nc.sync.dma_start(wb_sb[:, 0:wbsplit], d["d_wb"].ap()[:, 0:wbsplit])
    nc.sync.dma_start(bc_sb[:], d["d_bc"].ap())
    nc.sync.dma_start(atomb_sb[:], d["d_atomb"].ap())
    xoh_all = pIn.tile([AV, 9 * BL * N], FP8, name="xoh_all", tag="xoh",
                       bufs=1)
    nc.sync.dma_start(xoh_all[:], d["d_xoh"].ap())
    sl = slice(0, 2 * S_graph)
    nc.sync.dma_start(gpair_sb[:, sl], d["d_gpair"].ap()[:, sl])
    nc.sync.dma_start(soh_sb[:, sl], d["d_soh"].ap()[:, sl])
    nc.sync.dma_start(wb_sb[:, wbsplit:], d["d_wb"].ap()[:, wbsplit:])
    slr = slice(2 * S_graph, 2 * BL * S_graph)
    nc.sync.dma_start(gpair_sb[:, slr], d["d_gpair"].ap()[:, slr])
    nc.sync.dma_start(soh_sb[:, slr], d["d_soh"].ap()[:, slr])
    nc.sync.dma_start(wr_sb[:], d["d_wr"].ap())
    nc.sync.dma_start(bp2f_sb[:], d["d_bp2f"].ap())
    nc.sync.dma_start(bh2_sb[:], d["d_bh2"].ap())
    if has_empty:
        mrow_sb = pW.tile([1, BL * N], F32, name="mrow_sb")
        nc.sync.dma_start(mrow_sb[:], d["d_mask"].ap())
        nrow_sb = pW.tile([1, BL * N], F32, name="nrow_sb")
        nc.sync.dma_start(nrow_sb[:], d["d_neg"].ap())
        mask_bc = pW.tile([128, BL * N], F32, name="mask_bc")
        nc.gpsimd.partition_broadcast(mask_bc[:], mrow_sb[:])
        neg_bc = pW.tile([128, BL * N], F32, name="neg_bc")
        nc.gpsimd.partition_broadcast(neg_bc[:], nrow_sb[:])

    # pin the activation table to the set that has Relu+Copy+Identity+Sqrt
    sq_dummy = pW.tile([128, 1], F32, name="sq_dummy")
    nc.scalar.activation(sq_dummy[:], bc_sb[:, 26:27], AF.Sqrt)

    # bias prework (bc only)
    bias_pre = pW.tile([128, L], F32, name="bias_pre")
    nc.vector.tensor_reduce(
        bias_pre[:], bc_sb[:, 0:4 * L].rearrange("p (l f) -> p l f", l=L),
        axis=AX.X, op=ALU.add)
    bo12 = pW.tile([128, L], F32, name="bo12")
    nc.vector.tensor_reduce(
        bo12[:], bc_sb[:, 12:12 + 2 * L].rearrange("p (l f) -> p l f", l=L),
        axis=AX.X, op=ALU.add)

    # ---- node features (feature-major), bf16
    nf_ps = psA()
    for c in range(9):
        nc.tensor.matmul(nf_ps[:, 0:512], atomb_sb[:, c * H:(c + 1) * H],
                         xoh_all[:, c * BL * N:(c + 1) * BL * N],
                         start=(c == 0), stop=(c == 8))
    nf = pNM.tile([128, BL * N], BF16, name="nf")
    nc.scalar.activation(nf[:], nf_ps[:, 0:512], AF.Copy)

    # layer-0 m12 (z = [nf, 0]) + fp8 copies, and layer-0 bond weights
    def emit_bw(l):
        bw_ps = psB()
        nc.tensor.matmul(bw_ps[0:24, 0:128], WB("bondT")[:, 0:24],
                         WB(f"We_{l}"), start=True, stop=True)
        nc.tensor.matmul(bw_ps[0:24, 128:256], WB("bondT")[:, 24:48],
                         WB(f"We_{l}"), start=True, stop=True,
                         skip_group_check=True)
        bw24 = pMB.tile([24, 256], FP8, name="bw24", tag=f"bw24_{l}", bufs=1)
        nc.scalar.activation(bw24[:], bw_ps[0:24, 0:256], AF.Copy)
        return bw24

    bw24_l = {0: emit_bw(0)}

    mt_l = {0: pMB.tile([128, BL * 256], FP8, name="mt0", tag="mt", bufs=2)}
    for half in range(2):
        ps = psA()
        for gg in (2 * half, 2 * half + 1):
            off = (gg % 2) * 256
            gsl = slice(gg * N, (gg + 1) * N)
            nc.tensor.matmul(ps[:, off:off + 256], nf[:, gsl],
                             WB("m12_0_0"), start=True, stop=True)
        nc.scalar.activation(mt_l[0][:, half * 512:(half + 1) * 512],
                             ps[:, 0:512], AF.Copy)

    # relu2 engine split (Bresenham across all layers)
    state = dict(relu2_acc=0.0)

    def emit_chunk_pre(l, gg, tt):
        pre = psA()
        mt_g = mt_l[l][:, gg * 256:(gg + 1) * 256].rearrange(
            "p (k f) -> p k f", k=2)
        bw_k = bw24_l[l][:].rearrange("p (k f) -> p k f", k=2)
        for i, t in enumerate(tt):
            w = tile_w[t]
            if i + 1 < len(tt):
                # widen to fill the 512-col half: the paired relu below must
                # not read unwritten PSUM (extra gather cols are in-bounds;
                # their relu output lands in unread slots of msgs1)
                assert tile_base[t] + 512 <= S_graph
                w = 512
            c0 = gg * S_graph + tile_base[t]
            sl = slice(i * 512, i * 512 + w)
            nc.tensor.matmul(pre[:, sl], mt_g, k2(gpair_sb, c0, w),
                             start=True, stop=False, perf_mode=DR)
            nc.tensor.matmul(pre[:, sl], bw_k, k2(soh_sb, c0, w),
                             start=False, stop=True, perf_mode=DR)
        wtot = 512 * (len(tt) - 1) + tile_w[tt[-1]]
        msgs1 = pAct.tile([128, 1024], BF16, name="msgs1", tag="msgs1",
                          bufs=2)
        nc.scalar.activation(msgs1[:, 0:wtot], pre[:, 0:wtot], AF.Relu,
                             bias=bias_pre[:, l:l + 1])
        return msgs1

    def emit_chunk_p1(l, gg, tt, msgs1):
        outs = []
        for i, t in enumerate(tt):
            w = tile_w[t]
            p1 = psB()
            nc.tensor.matmul(p1[:, 0:w], WB(f"Wp1_{l}"),
                             msgs1[:, i * 512:i * 512 + w],
                             start=True, stop=True)
            msgs2 = pAct.tile([128, 512], BF16, name="msgs2", tag="msgs2",
                              bufs=3)
            state["relu2_acc"] += ACT_RELU2_SHARE
            if state["relu2_acc"] >= 1.0:
                state["relu2_acc"] -= 1.0
                nc.scalar.activation(msgs2[:, 0:w], p1[:, 0:w], AF.Relu,
                                     bias=bc_sb[:, 18 + l:19 + l])
            else:
                nc.vector.tensor_scalar(msgs2[:, 0:w], p1[:, 0:w],
                                        bc_sb[:, 18 + l:19 + l], 0.0,
                                        op0=ALU.add, op1=ALU.max)
            outs.append(msgs2)
        return outs

    def emit_chunk_p2(l, gg, tt, msgs2s, msgs_max):
        for i, t in enumerate(tt):
            w = tile_w[t]
            p2 = psC()
            nc.tensor.matmul(p2[:, 0:w], WB(f"Wp2_{l}"), msgs2s[i][:, 0:w],
                             start=True, stop=True)
            for (p0, R, K, gt, off) in groups:
                if gt != t:
                    continue
                nc.vector.tensor_reduce(
                    msgs_max[:, gg * N + p0: gg * N + p0 + R],
                    p2[:, off:off + R * K].rearrange("p (r k) -> p r k", r=R),
                    axis=AX.X, op=ALU.max)

    bias_h_l = {}

    hid = {0: None}
    TL = {}
    for l in range(L):
        TL[l] = dict(
            msgs_max=pLN.tile([128, BL * N], BF16, name="msgs_max",
                              tag="msgs_max", bufs=2),
            hn=pLN.tile([128, BL * N], F32, name="hn", tag="hn", bufs=1),
            hsq=pLN.tile([128, BL * N], BF16, name="hsq", tag="hsq", bufs=1),
            sumh=pLN.tile([128, BL], F32, name="sumh", tag="sumh"),
            sumsq=pLN.tile([128, BL], F32, name="sumsq", tag="sumsq"),
            negmean=pLN.tile([128, BL], F32, name="negmean", tag="negmean"),
            msq=pLN.tile([128, BL], F32, name="msq", tag="msq"),
            var=pLN.tile([128, BL], F32, name="var", tag="var"),
            std=pLN.tile([128, BL], F32, name="std", tag="std"),
            rstd=pLN.tile([128, BL], F32, name="rstd", tag="rstd"),
            h_fm=pLN.tile([128, BL * N], F32R, name="h_fm", tag="h_fm",
                          bufs=1),
            hid_new=pNM.tile([128, BL * N], BF16, name=f"hid{l + 1}",
                             tag=f"hid{(l + 1) % 2}"),
        )
        hid[l + 1] = TL[l]["hid_new"]

    def mk_block_h(l, pair):
        def fn():
            t = TL[l]
            msgs_max = t["msgs_max"]
            if l not in bias_h_l:
                if has_empty:
                    bias_h_l[l] = bo12[:, l:l + 1]
                else:
                    bh_ps = psC()
                    nc.tensor.matmul(bh_ps[:, 0:2], WB(f"Wo2_{l}"),
                                     bp2f_sb[:, l:l + 2],
                                     start=True, stop=True)
                    bh = pMB.tile([128, 1], F32, name="bias_h",
                                  tag=f"bias_h{l}", bufs=1)
                    nc.vector.tensor_tensor(bh[:], bh_ps[:, 0:1],
                                            bo12[:, l:l + 1], op=ALU.add)
                    bias_h_l[l] = bh[:]
            msgs_src = msgs_max
            if has_empty:
                mmf = pLN.tile([128, BL * N], F32, name="mmf", tag="mmf",
                               bufs=1)
                nc.vector.scalar_tensor_tensor(
                    mmf[:], msgs_max[:], bc_sb[:, 23 + l:24 + l],
                    mask_bc[:], op0=ALU.add, op1=ALU.mult)
                nc.vector.tensor_tensor(mmf[:], mmf[:], neg_bc[:],
                                        op=ALU.add)
                mmb = pLN.tile([128, BL * N], BF16, name="mmb", tag="mmb",
                               bufs=1)
                nc.scalar.activation(mmb[:], mmf[:], AF.Copy)
                msgs_src = mmb
            g0 = pair[0]
            psl = slice(g0 * N, (g0 + 2) * N)
            h_ps = psA()
            nc.tensor.matmul(h_ps[:, 0:256], WB(f"Wo1_{l}_0"),
                             nf[:, psl], start=True, stop=False)
            if l > 0:
                nc.tensor.matmul(h_ps[:, 0:256], WB(f"Wo1_{l}_1"),
                                 hid[l][:, psl], start=False, stop=False)
            nc.tensor.matmul(h_ps[:, 0:256], WB(f"Wo2_{l}"),
                             msgs_src[:, psl], start=False, stop=True)
            nc.scalar.activation(t["h_fm"][:, psl], h_ps[:, 0:256], AF.Relu,
                                 bias=bias_h_l[l])
        return fn

    def mk_block_tp(l, pair):
        def fn():
            t = TL[l]
            hn, hsq = t["hn"], t["hsq"]
            g0 = pair[0]
            psl = slice(g0 * N, (g0 + 2) * N)
            ssl = slice(g0, g0 + 2)
            tp_ps = psB(F32R)
            for i, gg in enumerate(pair):
                nc.tensor.transpose(
                    tp_ps[:, i * 128:(i + 1) * 128],
                    t["h_fm"][:, gg * N:(gg + 1) * N], WR("idn"))
            nc.vector.tensor_scalar(hn[:, psl],
                                    tp_ps[:, 0:256].bitcast(F32),
                                    0.0, None, op0=ALU.add)
            nc.vector.tensor_reduce(
                t["sumh"][:, ssl],
                hn[:, psl].rearrange("p (g n) -> p g n", g=2),
                axis=AX.X, op=ALU.add)
            nc.vector.scalar_tensor_tensor(
                hsq[:, psl], hn[:, psl], 0.0, hn[:, psl],
                op0=ALU.add, op1=ALU.mult)
            nc.vector.tensor_reduce(
                t["sumsq"][:, ssl],
                hsq[:, psl].rearrange("p (g n) -> p g n", g=2),
                axis=AX.X, op=ALU.add)
            nc.gpsimd.tensor_scalar(t["negmean"][:, ssl], t["sumh"][:, ssl],
                                    -1.0 / H, None, op0=ALU.mult)
            nc.vector.tensor_tensor(t["msq"][:, ssl], t["negmean"][:, ssl],
                                    t["negmean"][:, ssl], op=ALU.mult)
            nc.vector.scalar_tensor_tensor(
                t["var"][:, ssl], t["sumsq"][:, ssl], 1.0 / H,
                t["msq"][:, ssl], op0=ALU.mult, op1=ALU.subtract)
            nc.scalar.activation(t["std"][:, ssl], t["var"][:, ssl], AF.Sqrt,
                                 bias=bc_sb[:, 26:27])
            nc.vector.reciprocal(t["rstd"][:, ssl], t["std"][:, ssl])
        return fn

    def mk_block_norm(l, pair):
        def fn():
            t = TL[l]
            g0 = pair[0]
            psl = slice(g0 * N, (g0 + 2) * N)
            tp2_ps = psC(F32R)
            for i, gg in enumerate(pair):
                hnorm = pLN.tile([128, 128], F32R, name="hnorm",
                                 tag="hnorm", bufs=2)
                nc.gpsimd.tensor_scalar(hnorm[:],
                                        t["hn"][:, gg * N:(gg + 1) * N],
                                        t["negmean"][:, gg:gg + 1],
                                        t["rstd"][:, gg:gg + 1],
                                        op0=ALU.add, op1=ALU.mult)
                nc.tensor.transpose(tp2_ps[:, i * 128:(i + 1) * 128],
                                    hnorm[:], WR("idn"))
            nc.vector.tensor_scalar(t["hid_new"][:, psl],
                                    tp2_ps[:, 0:256].bitcast(F32),
                                    bc_sb[:, 27 + l:28 + l],
                                    bc_sb[:, 30 + l:31 + l],
                                    op0=ALU.mult, op1=ALU.add)
        return fn

    def mk_block_next(l, pair):
        def fn():
            nl = l + 1
            if nl >= L:
                return
            if nl not in bw24_l:
                bw24_l[nl] = emit_bw(nl)
                mt_l[nl] = pMB.tile([128, BL * 256], FP8,
                                    name=f"mt{nl}", tag="mt", bufs=2)
            ps = psA()
            for i, gg in enumerate(pair):
                off = i * 256
                gsl = slice(gg * N, (gg + 1) * N)
                nc.tensor.matmul(ps[:, off:off + 256], nf[:, gsl],
                                 WB(f"m12_{nl}_0"), start=True, stop=False)
                nc.tensor.matmul(ps[:, off:off + 256], hid[nl][:, gsl],
                                 WB(f"m12_{nl}_1"), start=False, stop=True)
            g0 = pair[0]
            nc.scalar.activation(
                mt_l[nl][:, g0 * 256:(g0 + 2) * 256], ps[:, 0:512],
                AF.Copy)
        return fn

    # chunk construction: pair widest (widened to its full bank) with
    # narrowest (its true width bounds the relu read)
    order_t = sorted(range(n_tiles), key=lambda t: -tile_w[t])
    tile_chunks = []
    avail = list(order_t)
    while len(avail) >= 2:
        f = avail[0]
        if tile_base[f] + 512 <= S_graph:
            tile_chunks.append((f, avail[-1]))
            avail = avail[1:-1]
        else:
            tile_chunks.append((f,))
            avail = avail[1:]
    if avail:
        tile_chunks.append((avail[0],))
    cpg = len(tile_chunks)
    nUL = BL * cpg

    units = [(l, gg, tt) for l in range(L)
             for gg in range(BL) for tt in tile_chunks]
    nU = len(units)
    post = defaultdict(list)
    for l in range(L):
        for pair in ((0, 1), (2, 3)):
            s0 = l * nUL + (pair[1] + 1) * cpg - 1 + 2
            post[s0 + 1].append(mk_block_h(l, pair))
            post[s0 + 2].append(mk_block_tp(l, pair))
            post[s0 + 3].append(mk_block_norm(l, pair))
            post[s0 + 4].append(mk_block_next(l, pair))
    st1, st2 = {}, {}
    for step in range(nU + 7):
        if step < nU:
            l, gg, tt = units[step]
            st1[step] = emit_chunk_pre(l, gg, tt)
        if 0 <= step - 1 < nU:
            u = step - 1
            l, gg, tt = units[u]
            st2[u] = emit_chunk_p1(l, gg, tt, st1.pop(u))
        if 0 <= step - 2 < nU:
            u = step - 2
            l, gg, tt = units[u]
            emit_chunk_p2(l, gg, tt, st2.pop(u), TL[l]["msgs_max"])
        for fn in post.pop(step, []):
            fn()

    # ---- pooling + prediction MLP
    ge_sum = pLN.tile([128, BL], F32, name="ge_sum", tag="ge_sum")
    nc.vector.tensor_reduce(
        ge_sum[:], hid[L][:].rearrange("p (g n) -> p g n", g=BL),
        axis=AX.X, op=ALU.add)
    ge = pLN.tile([128, BL], F32R, name="ge", tag="ge")
    nc.scalar.activation(ge[:], ge_sum[:], AF.Copy, scale=1.0 / N)
    o1 = psA()
    nc.tensor.matmul(o1[:, 0:BL], WR("Wh1"), ge[:], start=True, stop=True)
    t1 = pLN.tile([128, BL], F32R, name="t1", tag="t1")
    nc.scalar.activation(t1[:], o1[:, 0:BL], AF.Relu, bias=bc_sb[:, 21:22])
    o2 = psB()
    nc.tensor.matmul(o2[:, 0:BL], WR("Wh2"), t1[:], start=True, stop=True)
    out_sb = pLN.tile([OUT, BL], F32, name="out_sb", tag="out_sb")
    nc.scalar.activation(out_sb[:], o2[:, 0:BL], AF.Identity, bias=bh2_sb[:])
    nc.sync.dma_start(d["d_out"].ap(), out_sb[:])
    ctx.close()


# --------------------------------------------------------------------------
# Entry point.
# --------------------------------------------------------------------------

def build(inputs):
    struct, percore = _prep(inputs)
    A = _weight_arrays(inputs)
    wbmap = A.pop("_wbmap")
    wrmap = A.pop("_wrmap")
    wbsplit = A.pop("_wbsplit")
    key = (struct["S_graph"], struct["n_tiles"],
           tuple(struct["groups"]), struct["tile_w"], struct["has_empty"])
    if key not in _CACHE:
        _CACHE[key] = _build_program(struct, wbmap, wrmap,
                                     A["wb"].shape[1], A["wr"].shape[1],
                                     wbsplit)
    nc = _CACHE[key]

    in_maps = []
    for c in range(M):
        im = dict(
            gpair=percore["Gpair"][c], soh=percore["SOH"][c],
            xoh=percore["XOH"][c],
            maskrow=percore["maskrow"][c:c + 1],
            negrow=percore["negrow"][c:c + 1],
        )
        for k, v in A.items():
            im[k] = v
        in_maps.append(im)
    return nc, in_maps, struct


def kernel(**inputs):
    from concourse import bass_utils
    nc, in_maps, struct = build(inputs)
    res = bass_utils.run_bass_kernel_spmd(nc, in_maps, core_ids=list(range(M)))
    out = np.zeros((B, OUT), np.float32)
    for c in range(M):
        out[c * BL:(c + 1) * BL] = res.results[c]["out"].T
    return out


# revision 42
# speedup vs baseline: 1.1812x; 1.0097x over previous
"""Trainium2 Bass kernel for nn_BaselineModel_74509092651544 (CLRS-style MPNN).

Strategy
--------
Data-parallel over graphs: 32 graphs -> 8 cores x 4 graphs.  The dense
[B,N,N,H] message tensor is never materialized: only the ~61k unique
(graph,src,dst) edge slots survive the masked max, so the message MLP runs
on a padded CSR slot layout.

v2 (this file) vs the earlier baseline:
  * fp8 DoubleRow matmuls for the gather stage: m1/m2 (quantized fp8e4m3)
    ride as a 2-k-tile stationary pair against an interleaved Gsrc/Gdst
    one-hot moving tensor; the bond term uses a 24+24 row split of the
    one-hot counts.  1 cycle/slot instead of 3 on the PE.
  * bf16 weights/activations everywhere precision allows (validated
    ~7e-3 rel err vs the 2e-2 gate).
  * h-matmuls and LayerNorm batched across the 4 graphs per layer
    (512-wide) instead of per-graph 128-wide (f32r <256 runs at 1/4 rate).
  * DP-optimized receiver grouping (minimizes padded slots + per-group
    DVE reduce overhead).
  * Software-pipelined slot loop (skew 2) with PSUM rings 3/3/2.
  * relu1 on ACT, relu2 split ACT/DVE, segmented max on DVE.
"""

import sys
import numpy as np

sys.path.insert(0, "/opt/trn_rl_repo")

B, N, H, L, E, OUT = 32, 128, 128, 3, 65536, 128
M = 8                 # NeuronCores
BL = B // M           # graphs per core
NEG = -1e9
EPS = 1e-5
AV, BV = 128, 16
ACT_RELU2_SHARE = 0.62   # fraction of relu2 ops on ACT (rest on DVE)

_CACHE = {}


# --------------------------------------------------------------------------
# Host preprocessing: pure integer / relayout work.
# --------------------------------------------------------------------------

def _ffd_pack(groups):
    sizes = [R * K for (_, R, K) in groups]
    order = np.argsort(-np.asarray(sizes), kind="stable")
    bins, place = [], [None] * len(groups)
    for gi in order:
        sz = sizes[gi]
        for t in range(len(bins)):
            if bins[t] + sz <= 512:
                place[gi] = (t, bins[t])
                bins[t] += sz
                break
        else:
            place[gi] = (len(bins), 0)
            bins.append(sz)
    return bins, place


def _dp_groups(Kp, c_slot, c_group, max_r=128):
    n = len(Kp)
    INF = float("inf")
    dp = [INF] * (n + 1)
    dp[n] = 0.0
    choice = [1] * (n + 1)
    for p in range(n - 1, -1, -1):
        K = int(Kp[p])
        mr = min(n - p, 512 // K, max_r)
        best, best_r = INF, 1
        for R in range(1, mr + 1):
            c = R * K * c_slot + c_group + dp[p + R]
            if c < best:
                best, best_r = c, R
        dp[p], choice[p] = best, best_r
    groups = []
    p = 0
    while p < n:
        R = choice[p]
        groups.append((p, R, int(Kp[p])))
        p += R
    return groups


def _fixed_groups(Kp, r0):
    groups, p, n = [], 0, len(Kp)
    while p < n:
        K = int(Kp[p])
        R = min(r0, n - p)
        while R * K > 512:
            R //= 2
        groups.append((p, R, K))
        p += R
    return groups


def _choose_groups(Kp):
    """Pick the candidate minimizing a per-graph-layer time proxy (ns)."""
    cands = [_fixed_groups(Kp, 16),
             _dp_groups(Kp, 4.0, 100.0),
             _dp_groups(Kp, 4.0, 170.0, 16),
             _dp_groups(Kp, 2.5, 170.0)]
    best, best_c = None, float("inf")
    for gs in cands:
        bins, _ = _ffd_pack(gs)
        S = sum(R * K for (_, R, K) in gs)
        c = S * 4.04 + len(gs) * 170.0 + len(bins) * 395.0
        if c < best_c:
            best, best_c = gs, c
    return best


def _prep(inputs):
    x = np.asarray(inputs["x"]).astype(np.int64)            # [B*N, 9]
    ea = np.asarray(inputs["edge_attr"]).astype(np.int64)   # [E, 3]
    ei = np.asarray(inputs["edge_index"]).astype(np.int64)  # [2, E]

    g = ei[0] // N
    s = ei[0] % N
    d = ei[1] % N
    key = (g * N + s) * N + d
    uniq, inv = np.unique(key, return_inverse=True)
    US = uniq.size
    ug = uniq // (N * N)
    us = (uniq // N) % N
    ud = uniq % N

    # bond one-hot counts per unique slot  [US, 48]
    oh48 = np.zeros((US, 48), np.float32)
    for c in range(3):
        np.add.at(oh48, (inv, ea[:, c] + 16 * c), 1.0)

    # unique in-degree per (graph, receiver)
    deg = np.zeros((B, N), np.int64)
    np.add.at(deg, (ug, ud), 1)

    # receiver relabeling: position p holds the p-th highest-degree receiver
    rho = np.argsort(-deg, axis=1, kind="stable")        # [B, N] pos -> orig
    rho_inv = np.argsort(rho, axis=1)                    # orig -> pos
    degS = -np.sort(-deg, axis=1)                        # [B, N] desc
    Kp = np.maximum(degS.max(axis=0), 1)                 # [N] non-increasing

    groups = _choose_groups(Kp)                          # (p0, R, K)
    tiles_used, place = _ffd_pack(groups)
    n_tiles = len(tiles_used)
    tile_w = [int(w) for w in tiles_used]
    tile_base = np.concatenate([[0], np.cumsum(tile_w)[:-1]]).astype(np.int64)
    S_graph = int(((sum(tile_w) + 15) // 16) * 16)
    S_core = BL * S_graph

    # per-position lookup tables
    col_base_of_pos = np.zeros(N, np.int64)   # first column of the receiver
    K_of_pos = np.zeros(N, np.int64)
    for gi, (p0, R, K) in enumerate(groups):
        t, off = place[gi]
        for r in range(R):
            col_base_of_pos[p0 + r] = tile_base[t] + off + r * K
            K_of_pos[p0 + r] = K

    # slots ordered by (g, d, s): contiguous per receiver
    order = np.lexsort((us, ud, ug))
    og, od, osl = ug[order], ud[order], order
    osrc = us[order]
    recv_id = og * N + od
    first = np.concatenate([[0], np.flatnonzero(np.diff(recv_id)) + 1])
    k_rank = np.arange(len(og)) - first[np.searchsorted(recv_id[first], recv_id)]

    pos = rho_inv[og, od]
    core_r = og // BL
    col_r = (og % BL) * S_graph + col_base_of_pos[pos] + k_rank

    # padding: receivers with deg < K duplicate their first slot
    fg, fd = og[first], od[first]
    fpos = rho_inv[fg, fd]
    fdeg = deg[fg, fd]
    fK = K_of_pos[fpos]
    padc = (fK - fdeg).astype(np.int64)
    assert (padc >= 0).all()
    rep = np.repeat(np.arange(len(first)), padc)
    kpad = np.arange(len(rep)) - np.repeat(
        np.concatenate([[0], np.cumsum(padc)[:-1]]), padc
    ) + np.repeat(fdeg, padc)
    pg = fg[rep]
    core_p = pg // BL
    col_p = (pg % BL) * S_graph + col_base_of_pos[fpos[rep]] + kpad
    slot_p = osl[first][rep]
    src_p = osrc[first][rep]

    a_core = np.concatenate([core_r, core_p])
    a_col = np.concatenate([col_r, col_p])
    a_slot = np.concatenate([osl, slot_p])
    a_srcnew = np.concatenate([rho_inv[og, osrc], rho_inv[pg, src_p]])
    a_dstpos = np.concatenate([pos, fpos[rep]])

    import ml_dtypes
    FP8 = ml_dtypes.float8_e4m3fn
    flat = a_core * S_core + a_col
    # column-interleaved src/dst one-hot k-tiles: [M, 128, S_core*2]
    # (column s occupies bytes [2s, 2s+1]: k=0 -> Gsrc, k=1 -> Gdst)
    Gpair = np.zeros((M * S_core, 2, 128), np.float32)
    Gpair[flat, 0, a_srcnew] = 1.0
    Gpair[flat, 1, a_dstpos] = 1.0
    Gpair = np.ascontiguousarray(
        Gpair.reshape(M, S_core, 2, 128).transpose(0, 3, 1, 2)
        .reshape(M, 128, 2 * S_core)).astype(FP8)
    # bond count k-tiles (rows 0:24 / 24:48), column-interleaved:
    # [M, 24, S_core*2]
    SOH = np.zeros((M * S_core, 48), np.float32)
    SOH[flat] = oh48[a_slot]
    assert float(SOH.max()) <= 16.0
    SOH = np.ascontiguousarray(
        SOH.reshape(M, S_core, 2, 24).transpose(0, 3, 1, 2)
        .reshape(M, 24, 2 * S_core)).astype(FP8)

    # atom one-hot per core: [M, 9, AV, BL*N] in relabeled node order
    gg = np.repeat(np.arange(B), N)
    pp = np.tile(np.arange(N), B)
    orig = gg * N + rho[gg, pp]                    # [B*N] column -> orig node
    XOH = np.zeros((M, 9, AV, BL * N), np.float32)
    mcol = np.tile(np.arange(BL * N), M)
    mcore = np.repeat(np.arange(M), BL * N)
    for c in range(9):
        XOH[mcore, c, x[orig, c], mcol] = 1.0
    # one flat [AV, 9*BL*N] tensor per core -> single DMA
    XOH = np.ascontiguousarray(
        XOH.transpose(0, 2, 1, 3).reshape(M, AV, 9 * BL * N)).astype(FP8)

    # empty receivers (deg==0) -> need NEG mask path
    empt = (deg == 0)
    has_empty = bool(empt.any())
    maskrow = np.ones((M, BL * N), np.float32)
    negrow = np.zeros((M, BL * N), np.float32)
    if has_empty:
        eg, en = np.nonzero(empt)
        epos = rho_inv[eg, en]
        maskrow[eg // BL, (eg % BL) * N + epos] = 0.0
        negrow[eg // BL, (eg % BL) * N + epos] = NEG

    struct = dict(
        S_graph=S_graph, S_core=S_core, n_tiles=n_tiles,
        groups=[(p0, R, K, place[gi][0], place[gi][1])
                for gi, (p0, R, K) in enumerate(groups)],
        tile_w=tuple(tile_w), tile_base=tuple(int(b) for b in tile_base),
        has_empty=has_empty,
    )
    percore = dict(Gpair=Gpair, SOH=SOH, XOH=XOH,
                   maskrow=maskrow, negrow=negrow)
    return struct, percore


def _weight_arrays(inputs):
    import ml_dtypes
    BF16 = ml_dtypes.bfloat16
    f32 = np.float32

    def blob(cols):
        wmap = {}
        off = 0
        for name, arr in cols:
            wmap[name] = (off, arr.shape[1])
            off += arr.shape[1]
        data = np.concatenate([a for _, a in cols], 1)
        return np.ascontiguousarray(data), wmap

    Wm1 = np.asarray(inputs["Wm1"], f32)
    Wm2 = np.asarray(inputs["Wm2"], f32)
    bond_T = np.asarray(inputs["bond_emb"], f32).reshape(48, H).T  # [128, 48]
    # chunk 1: everything layer-0's slot phase needs (DMA'd first)
    cols_b = [("bondT", bond_T),
              ("We_0", np.asarray(inputs["We"], f32)[0]),
              ("m12_0_0",
               np.concatenate([Wm1[0, 0:128], Wm2[0, 0:128]], 1)),
              ("Wp1_0", np.asarray(inputs["Wp1"], f32)[0]),
              ("Wp2_0", np.asarray(inputs["Wp2"], f32)[0])]
    wb_split = sum(a.shape[1] for _, a in cols_b)
    # chunk 2: the rest (lands before layer-0's h phase)
    cols_b.append(("Wo1_0_0", np.asarray(inputs["Wo1"], f32)[0, 0:128]))
    for l in range(1, L):
        cols_b.append((f"We_{l}", np.asarray(inputs["We"], f32)[l]))
        cols_b.append((f"m12_{l}_0",
                       np.concatenate([Wm1[l, 0:128], Wm2[l, 0:128]], 1)))
        cols_b.append((f"m12_{l}_1",
                       np.concatenate([Wm1[l, 128:256], Wm2[l, 128:256]], 1)))
        cols_b.append((f"Wp1_{l}", np.asarray(inputs["Wp1"], f32)[l]))
        cols_b.append((f"Wp2_{l}", np.asarray(inputs["Wp2"], f32)[l]))
        cols_b.append((f"Wo1_{l}_0", np.asarray(inputs["Wo1"], f32)[l, 0:128]))
        cols_b.append((f"Wo1_{l}_1", np.asarray(inputs["Wo1"], f32)[l, 128:256]))
    for l in range(L):
        cols_b.append((f"Wo2_{l}", np.asarray(inputs["Wo2"], f32)[l]))
    wb, wbmap = blob(cols_b)

    cols_r = [("Wh1", np.asarray(inputs["Wh1"], f32)),
              ("Wh2", np.asarray(inputs["Wh2"], f32)),
              ("idn", np.eye(128, dtype=f32))]
    wr, wrmap = blob(cols_r)

    A = {}
    A["wb"] = wb.astype(BF16)
    A["wr"] = wr
    A["_wbmap"] = wbmap
    A["_wrmap"] = wrmap
    A["_wbsplit"] = wb_split

    at = np.asarray(inputs["atom_emb"], f32).transpose(1, 0, 2).reshape(AV, 9 * H)
    A["atomb"] = np.ascontiguousarray(at).astype(BF16)

    # bias columns [128, 33]: 4 pre-terms x L, 2 o-terms x L, bh1, bh2, eps,
    # ln_s x L, ln_b x L, bp1 x L, bp2 x L (same layout as baseline)
    bc = np.zeros((H, 34), f32)
    bc[:, 26] = EPS
    bc[:, 33] = np.asarray(inputs["ln_s"], f32)[L - 1] / N
    bc[:, 27:30] = np.asarray(inputs["ln_s"], f32).T
    bc[:, 30:33] = np.asarray(inputs["ln_b"], f32).T
    for l in range(L):
        bc[:, 4 * l + 0] = np.asarray(inputs["bm1"], f32)[l]
        bc[:, 4 * l + 1] = np.asarray(inputs["bm2"], f32)[l]
        bc[:, 4 * l + 2] = np.asarray(inputs["be"], f32)[l]
        bc[:, 4 * l + 3] = np.asarray(inputs["bg"], f32)[l]
        bc[:, 12 + 2 * l + 0] = np.asarray(inputs["bo1"], f32)[l]
        bc[:, 12 + 2 * l + 1] = np.asarray(inputs["bo2"], f32)[l]
        bc[:, 18 + l] = np.asarray(inputs["bp1"], f32)[l]
        bc[:, 23 + l] = np.asarray(inputs["bp2"], f32)[l]
    bc[:, 21] = np.asarray(inputs["bh1"], f32)
    bc[:, 22] = np.asarray(inputs["bh2"], f32)[:H]
    A["bias_cols"] = bc
    A["bh2_full"] = np.ascontiguousarray(
        np.asarray(inputs["bh2"], f32).reshape(OUT, 1))
    bp2f = np.zeros((H, 4), f32)
    bp2f[:, :L] = np.asarray(inputs["bp2"], f32).T
    A["bp2f"] = bp2f.astype(BF16)
    return A


# --------------------------------------------------------------------------
# Bass program.
# --------------------------------------------------------------------------

def _build_program(struct, wbmap, wrmap, wbc, wrc, wbsplit):
    import concourse.bacc as bacc
    import concourse.mybir as mybir
    import concourse.tile as tile

    F32 = mybir.dt.float32
    nc = bacc.Bacc("TRN2", target_bir_lowering=False, debug=False)

    BF16 = mybir.dt.bfloat16
    FP8 = mybir.dt.float8e4
    F32R = mybir.dt.float32r
    S_core = struct["S_core"]
    d = {}
    d["d_gpair"] = nc.dram_tensor("gpair", [128, 2 * S_core], FP8,
                                  kind="ExternalInput")
    d["d_soh"] = nc.dram_tensor("soh", [24, 2 * S_core], FP8,
                                kind="ExternalInput")
    d["d_xoh"] = nc.dram_tensor("xoh", [AV, 9 * BL * N], FP8,
                                kind="ExternalInput")
    d["d_atomb"] = nc.dram_tensor("atomb", [AV, 9 * H], BF16,
                                  kind="ExternalInput")
    d["d_wb"] = nc.dram_tensor("wb", [128, wbc], BF16, kind="ExternalInput")
    d["d_wr"] = nc.dram_tensor("wr", [128, wrc], F32R, kind="ExternalInput")
    d["d_bc"] = nc.dram_tensor("bias_cols", [H, 34], F32, kind="ExternalInput")
    d["d_bh2"] = nc.dram_tensor("bh2_full", [OUT, 1], F32, kind="ExternalInput")
    d["d_bp2f"] = nc.dram_tensor("bp2f", [H, 4], BF16, kind="ExternalInput")
    d["d_mask"] = nc.dram_tensor("maskrow", [1, BL * N], F32,
                                 kind="ExternalInput")
    d["d_neg"] = nc.dram_tensor("negrow", [1, BL * N], F32,
                                kind="ExternalInput")
    d["d_out"] = nc.dram_tensor("out", [OUT, BL], F32, kind="ExternalOutput")

    with tile.TileContext(nc) as tc:
        _emit(tc, nc, d, struct, wbmap, wrmap, mybir, wbsplit)
    nc.compile()
    return nc


def _emit(tc, nc, d, struct, wbmap, wrmap, mybir, wbsplit):
    import contextlib
    from collections import defaultdict
    ctx = contextlib.ExitStack()
    F32 = mybir.dt.float32
    F32R = mybir.dt.float32r
    BF16 = mybir.dt.bfloat16
    FP8 = mybir.dt.float8e4
    AF = mybir.ActivationFunctionType
    ALU = mybir.AluOpType
    AX = mybir.AxisListType
    DR = mybir.MatmulPerfMode.DoubleRow

    S_graph = struct["S_graph"]
    S_core = struct["S_core"]
    n_tiles = struct["n_tiles"]
    groups = struct["groups"]
    tile_w = struct["tile_w"]
    tile_base = struct["tile_base"]
    has_empty = struct["has_empty"]

    pG = ctx.enter_context(tc.tile_pool(name="pG", bufs=1))
    pW = ctx.enter_context(tc.tile_pool(name="pW", bufs=1))
    pAct = ctx.enter_context(tc.tile_pool(name="pAct", bufs=3))
    pNM = ctx.enter_context(tc.tile_pool(name="pNM", bufs=1))
    pMB = ctx.enter_context(tc.tile_pool(name="pMB", bufs=2))
    pLN = ctx.enter_context(tc.tile_pool(name="pLN", bufs=1))
    pIn = ctx.enter_context(tc.tile_pool(name="pIn", bufs=2))
    ps_a = ctx.enter_context(tc.tile_pool(name="ps_a", bufs=2, space="PSUM"))
    ps_b = ctx.enter_context(tc.tile_pool(name="ps_b", bufs=2, space="PSUM"))
    ps_c = ctx.enter_context(tc.tile_pool(name="ps_c", bufs=2, space="PSUM"))

    def psA(dt=F32):
        """Double-bank tile: pre pairs (also m12/h_ps/nf_ps, half-used)."""
        return ps_a.tile([128, 1024], dt, name="psA", tag="psA")

    def psB(dt=F32):
        return ps_b.tile([128, 512], dt, name="psB", tag="psB")

    def psC(dt=F32):
        return ps_c.tile([128, 512], dt, name="psC", tag="psC")

    # ---- resident tiles
    gpair_sb = pG.tile([128, 2 * S_core], FP8, name="gpair_sb")
    soh_sb = pG.tile([24, 2 * S_core], FP8, name="soh_sb")

    def k2(tile_sb, c0, w):
        """[p, 2*(c0..c0+w)] interleaved slice -> [p, k=2, s=w] AP."""
        return tile_sb[:, 2 * c0:2 * (c0 + w)].rearrange(
            "p (s k) -> p k s", k=2)

    wbc = sum(w for _, w in wbmap.values())
    wb_sb = pW.tile([128, wbc], BF16, name="wb_sb")
    wr_sb = pW.tile([128, sum(w for _, w in wrmap.values())], F32R,
                    name="wr_sb")
    bc_sb = pW.tile([H, 34], F32, name="bc_sb")
    bh2_sb = pW.tile([OUT, 1], F32, name="bh2_sb")
    bp2f_sb = pW.tile([H, 4], BF16, name="bp2f_sb")
    atomb_sb = pW.tile([AV, 9 * H], BF16, name="atomb_sb")

    def WB(name):
        off, w = wbmap[name]
        return wb_sb[:, off:off + w]

    def WR(name):
        off, w = wrmap[name]
        return wr_sb[:, off:off + w]

    # ---- DMAs, warmup-critical first
    nc.sync.dma_start(wb_sb[:, 0:wbsplit], d["d_wb"].ap()[:, 0:wbsplit])
    nc.sync.dma_start(bc_sb[:], d["d_bc"].ap())
    nc.sync.dma_start(atomb_sb[:], d["d_atomb"].ap())
    xoh_all = pIn.tile([AV, 9 * BL * N], FP8, name="xoh_all", tag="xoh",
                       bufs=1)
    nc.sync.dma_start(xoh_all[:], d["d_xoh"].ap())
    sl = slice(0, 2 * S_graph)
    nc.sync.dma_start(gpair_sb[:, sl], d["d_gpair"].ap()[:, sl])
    nc.sync.dma_start(soh_sb[:, sl], d["d_soh"].ap()[:, sl])
    nc.sync.dma_start(wb_sb[:, wbsplit:], d["d_wb"].ap()[:, wbsplit:])
    slr = slice(2 * S_graph, 2 * BL * S_graph)
    nc.sync.dma_start(gpair_sb[:, slr], d["d_gpair"].ap()[:, slr])
    nc.sync.dma_start(soh_sb[:, slr], d["d_soh"].ap()[:, slr])
    nc.sync.dma_start(wr_sb[:], d["d_wr"].ap())
    nc.sync.dma_start(bp2f_sb[:], d["d_bp2f"].ap())
    nc.sync.dma_start(bh2_sb[:], d["d_bh2"].ap())
    if has_empty:
        mrow_sb = pW.tile([1, BL * N], F32, name="mrow_sb")
        nc.sync.dma_start(mrow_sb[:], d["d_mask"].ap())
        nrow_sb = pW.tile([1, BL * N], F32, name="nrow_sb")
        nc.sync.dma_start(nrow_sb[:], d["d_neg"].ap())
        mask_bc = pW.tile([128, BL * N], F32, name="mask_bc")
        nc.gpsimd.partition_broadcast(mask_bc[:], mrow_sb[:])
        neg_bc = pW.tile([128, BL * N], F32, name="neg_bc")
        nc.gpsimd.partition_broadcast(neg_bc[:], nrow_sb[:])

    # pin the activation table to the set that has Relu+Copy+Identity+Sqrt
    sq_dummy = pW.tile([128, 1], F32, name="sq_dummy")
    nc.scalar.activation(sq_dummy[:], bc_sb[:, 26:27], AF.Sqrt)

    # bias prework (bc only)
    bias_pre = pW.tile([128, L], F32, name="bias_pre")
    nc.vector.tensor_reduce(
        bias_pre[:], bc_sb[:, 0:4 * L].rearrange("p (l f) -> p l f", l=L),
        axis=AX.X, op=ALU.add)
    bo12 = pW.tile([128, L], F32, name="bo12")
    nc.vector.tensor_reduce(
        bo12[:], bc_sb[:, 12:12 + 2 * L].rearrange("p (l f) -> p l f", l=L),
        axis=AX.X, op=ALU.add)

    # ---- node features (feature-major), bf16
    nf_ps = psA()
    for c in range(9):
        nc.tensor.matmul(nf_ps[:, 0:512], atomb_sb[:, c * H:(c + 1) * H],
                         xoh_all[:, c * BL * N:(c + 1) * BL * N],
                         start=(c == 0), stop=(c == 8))
    nf = pNM.tile([128, BL * N], BF16, name="nf")
    nc.scalar.activation(nf[:], nf_ps[:, 0:512], AF.Copy)

    # layer-0 m12 (z = [nf, 0]) + fp8 copies, and layer-0 bond weights
    def emit_bw(l):
        bw_ps = psB()
        nc.tensor.matmul(bw_ps[0:24, 0:128], WB("bondT")[:, 0:24],
                         WB(f"We_{l}"), start=True, stop=True)
        nc.tensor.matmul(bw_ps[0:24, 128:256], WB("bondT")[:, 24:48],
                         WB(f"We_{l}"), start=True, stop=True,
                         skip_group_check=True)
        bw24 = pMB.tile([24, 256], FP8, name="bw24", tag=f"bw24_{l}", bufs=1)
        nc.scalar.activation(bw24[:], bw_ps[0:24, 0:256], AF.Copy)
        return bw24

    bw24_l = {0: emit_bw(0)}

    mt_l = {0: pMB.tile([128, BL * 256], FP8, name="mt0", tag="mt", bufs=2)}
    for half in range(2):
        ps = psA()
        for gg in (2 * half, 2 * half + 1):
            off = (gg % 2) * 256
            gsl = slice(gg * N, (gg + 1) * N)
            nc.tensor.matmul(ps[:, off:off + 256], nf[:, gsl],
                             WB("m12_0_0"), start=True, stop=True)
        nc.scalar.activation(mt_l[0][:, half * 512:(half + 1) * 512],
                             ps[:, 0:512], AF.Copy)

    # relu2 engine split (Bresenham across all layers)
    state = dict(relu2_acc=0.0)

    def emit_chunk_pre(l, gg, tt):
        pre = psA()
        mt_g = mt_l[l][:, gg * 256:(gg + 1) * 256].rearrange(
            "p (k f) -> p k f", k=2)
        bw_k = bw24_l[l][:].rearrange("p (k f) -> p k f", k=2)
        for i, t in enumerate(tt):
            w = tile_w[t]
            if i + 1 < len(tt):
                # widen to fill the 512-col half: the paired relu below must
                # not read unwritten PSUM (extra gather cols are in-bounds;
                # their relu output lands in unread slots of msgs1)
                assert tile_base[t] + 512 <= S_graph
                w = 512
            c0 = gg * S_graph + tile_base[t]
            sl = slice(i * 512, i * 512 + w)
            nc.tensor.matmul(pre[:, sl], mt_g, k2(gpair_sb, c0, w),
                             start=True, stop=False, perf_mode=DR)
            nc.tensor.matmul(pre[:, sl], bw_k, k2(soh_sb, c0, w),
                             start=False, stop=True, perf_mode=DR)
        wtot = 512 * (len(tt) - 1) + tile_w[tt[-1]]
        msgs1 = pAct.tile([128, 1024], BF16, name="msgs1", tag="msgs1",
                          bufs=2)
        nc.scalar.activation(msgs1[:, 0:wtot], pre[:, 0:wtot], AF.Relu,
                             bias=bias_pre[:, l:l + 1])
        return msgs1

    def emit_chunk_p1(l, gg, tt, msgs1):
        outs = []
        for i, t in enumerate(tt):
            w = tile_w[t]
            p1 = psB()
            nc.tensor.matmul(p1[:, 0:w], WB(f"Wp1_{l}"),
                             msgs1[:, i * 512:i * 512 + w],
                             start=True, stop=True)
            msgs2 = pAct.tile([128, 512], BF16, name="msgs2", tag="msgs2",
                              bufs=3)
            state["relu2_acc"] += ACT_RELU2_SHARE
            if state["relu2_acc"] >= 1.0:
                state["relu2_acc"] -= 1.0
                nc.scalar.activation(msgs2[:, 0:w], p1[:, 0:w], AF.Relu,
                                     bias=bc_sb[:, 18 + l:19 + l])
            else:
                nc.vector.tensor_scalar(msgs2[:, 0:w], p1[:, 0:w],
                                        bc_sb[:, 18 + l:19 + l], 0.0,
                                        op0=ALU.add, op1=ALU.max)
            outs.append(msgs2)
        return outs

    def emit_chunk_p2(l, gg, tt, msgs2s, msgs_max):
        for i, t in enumerate(tt):
            w = tile_w[t]
            p2 = psC()
            nc.tensor.matmul(p2[:, 0:w], WB(f"Wp2_{l}"), msgs2s[i][:, 0:w],
                             start=True, stop=True)
            for (p0, R, K, gt, off) in groups:
                if gt != t:
                    continue
                nc.vector.tensor_reduce(
                    msgs_max[:, gg * N + p0: gg * N + p0 + R],
                    p2[:, off:off + R * K].rearrange("p (r k) -> p r k", r=R),
                    axis=AX.X, op=ALU.max)

    bias_h_l = {}
    ge_sum = pLN.tile([128, BL], F32, name="ge_sum", tag="ge_sum")

    hid = {0: None}
    TL = {}
    for l in range(L):
        TL[l] = dict(
            msgs_max=pLN.tile([128, BL * N], BF16, name="msgs_max",
                              tag="msgs_max", bufs=2),
            hn=pLN.tile([128, BL * N], F32, name="hn", tag="hn", bufs=1),
            hsq=pLN.tile([128, BL * N], BF16, name="hsq", tag="hsq", bufs=1),
            sumh=pLN.tile([128, BL], F32, name="sumh", tag="sumh"),
            sumsq=pLN.tile([128, BL], F32, name="sumsq", tag="sumsq"),
            negmean=pLN.tile([128, BL], F32, name="negmean", tag="negmean"),
            msq=pLN.tile([128, BL], F32, name="msq", tag="msq"),
            var=pLN.tile([128, BL], F32, name="var", tag="var"),
            std=pLN.tile([128, BL], F32, name="std", tag="std"),
            rstd=pLN.tile([128, BL], F32, name="rstd", tag="rstd"),
            h_fm=pLN.tile([128, BL * N], F32R, name="h_fm", tag="h_fm",
                          bufs=1),
            hid_new=(pNM.tile([128, BL * N], BF16, name=f"hid{l + 1}",
                              tag=f"hid{(l + 1) % 2}")
                     if l + 1 < L else None),
        )
        hid[l + 1] = TL[l]["hid_new"]

    def mk_block_h(l, pair):
        def fn():
            t = TL[l]
            msgs_max = t["msgs_max"]
            if l not in bias_h_l:
                if has_empty:
                    bias_h_l[l] = bo12[:, l:l + 1]
                else:
                    bh_ps = psC()
                    nc.tensor.matmul(bh_ps[:, 0:2], WB(f"Wo2_{l}"),
                                     bp2f_sb[:, l:l + 2],
                                     start=True, stop=True)
                    bh = pMB.tile([128, 1], F32, name="bias_h",
                                  tag=f"bias_h{l}", bufs=1)
                    nc.vector.tensor_tensor(bh[:], bh_ps[:, 0:1],
                                            bo12[:, l:l + 1], op=ALU.add)
                    bias_h_l[l] = bh[:]
            msgs_src = msgs_max
            if has_empty:
                mmf = pLN.tile([128, BL * N], F32, name="mmf", tag="mmf",
                               bufs=1)
                nc.vector.scalar_tensor_tensor(
                    mmf[:], msgs_max[:], bc_sb[:, 23 + l:24 + l],
                    mask_bc[:], op0=ALU.add, op1=ALU.mult)
                nc.vector.tensor_tensor(mmf[:], mmf[:], neg_bc[:],
                                        op=ALU.add)
                mmb = pLN.tile([128, BL * N], BF16, name="mmb", tag="mmb",
                               bufs=1)
                nc.scalar.activation(mmb[:], mmf[:], AF.Copy)
                msgs_src = mmb
            g0 = pair[0]
            psl = slice(g0 * N, (g0 + 2) * N)
            h_ps = psA()
            nc.tensor.matmul(h_ps[:, 0:256], WB(f"Wo1_{l}_0"),
                             nf[:, psl], start=True, stop=False)
            if l > 0:
                nc.tensor.matmul(h_ps[:, 0:256], WB(f"Wo1_{l}_1"),
                                 hid[l][:, psl], start=False, stop=False)
            nc.tensor.matmul(h_ps[:, 0:256], WB(f"Wo2_{l}"),
                             msgs_src[:, psl], start=False, stop=True)
            nc.scalar.activation(t["h_fm"][:, psl], h_ps[:, 0:256], AF.Relu,
                                 bias=bias_h_l[l])
        return fn

    def mk_block_tp(l, pair):
        def fn():
            t = TL[l]
            hn, hsq = t["hn"], t["hsq"]
            g0 = pair[0]
            psl = slice(g0 * N, (g0 + 2) * N)
            ssl = slice(g0, g0 + 2)
            tp_ps = psB(F32R)
            for i, gg in enumerate(pair):
                nc.tensor.transpose(
                    tp_ps[:, i * 128:(i + 1) * 128],
                    t["h_fm"][:, gg * N:(gg + 1) * N], WR("idn"))
            nc.vector.tensor_scalar(hn[:, psl],
                                    tp_ps[:, 0:256].bitcast(F32),
                                    0.0, None, op0=ALU.add)
            nc.vector.tensor_reduce(
                t["sumh"][:, ssl],
                hn[:, psl].rearrange("p (g n) -> p g n", g=2),
                axis=AX.X, op=ALU.add)
            nc.vector.scalar_tensor_tensor(
                hsq[:, psl], hn[:, psl], 0.0, hn[:, psl],
                op0=ALU.add, op1=ALU.mult)
            nc.vector.tensor_reduce(
                t["sumsq"][:, ssl],
                hsq[:, psl].rearrange("p (g n) -> p g n", g=2),
                axis=AX.X, op=ALU.add)
            nc.gpsimd.tensor_scalar(t["negmean"][:, ssl], t["sumh"][:, ssl],
                                    -1.0 / H, None, op0=ALU.mult)
            nc.vector.tensor_tensor(t["msq"][:, ssl], t["negmean"][:, ssl],
                                    t["negmean"][:, ssl], op=ALU.mult)
            nc.vector.scalar_tensor_tensor(
                t["var"][:, ssl], t["sumsq"][:, ssl], 1.0 / H,
                t["msq"][:, ssl], op0=ALU.mult, op1=ALU.subtract)
            nc.scalar.activation(t["std"][:, ssl], t["var"][:, ssl], AF.Sqrt,
                                 bias=bc_sb[:, 26:27])
            nc.vector.reciprocal(t["rstd"][:, ssl], t["std"][:, ssl])
        return fn

    def mk_block_norm(l, pair):
        def fn():
            t = TL[l]
            g0 = pair[0]
            psl = slice(g0 * N, (g0 + 2) * N)
            tp2_ps = psC(F32R)
            for i, gg in enumerate(pair):
                hnorm = pLN.tile([128, 128], F32R, name="hnorm",
                                 tag="hnorm", bufs=2)
                nc.gpsimd.tensor_scalar(hnorm[:],
                                        t["hn"][:, gg * N:(gg + 1) * N],
                                        t["negmean"][:, gg:gg + 1],
                                        t["rstd"][:, gg:gg + 1],
                                        op0=ALU.add, op1=ALU.mult)
                nc.tensor.transpose(tp2_ps[:, i * 128:(i + 1) * 128],
                                    hnorm[:], WR("idn"))
            if l + 1 < L:
                nc.vector.tensor_scalar(t["hid_new"][:, psl],
                                        tp2_ps[:, 0:256].bitcast(F32),
                                        bc_sb[:, 27 + l:28 + l],
                                        bc_sb[:, 30 + l:31 + l],
                                        op0=ALU.mult, op1=ALU.add)
            else:
                # pool directly: ge += mean_n(tp2) * ln_s + ln_b (per pair)
                nc.vector.tensor_reduce(
                    ge_sum[:, g0:g0 + 2],
                    tp2_ps[:, 0:256].bitcast(F32).rearrange(
                        "p (g n) -> p g n", g=2),
                    axis=AX.X, op=ALU.add)
        return fn

    def mk_block_next(l, pair):
        def fn():
            nl = l + 1
            if nl >= L:
                return
            if nl not in bw24_l:
                bw24_l[nl] = emit_bw(nl)
                mt_l[nl] = pMB.tile([128, BL * 256], FP8,
                                    name=f"mt{nl}", tag="mt", bufs=2)
            ps = psA()
            for i, gg in enumerate(pair):
                off = i * 256
                gsl = slice(gg * N, (gg + 1) * N)
                nc.tensor.matmul(ps[:, off:off + 256], nf[:, gsl],
                                 WB(f"m12_{nl}_0"), start=True, stop=False)
                nc.tensor.matmul(ps[:, off:off + 256], hid[nl][:, gsl],
                                 WB(f"m12_{nl}_1"), start=False, stop=True)
            g0 = pair[0]
            nc.scalar.activation(
                mt_l[nl][:, g0 * 256:(g0 + 2) * 256], ps[:, 0:512],
                AF.Copy)
        return fn

    # chunk construction: pair widest (widened to its full bank) with
    # narrowest (its true width bounds the relu read)
    order_t = sorted(range(n_tiles), key=lambda t: -tile_w[t])
    tile_chunks = []
    avail = list(order_t)
    while len(avail) >= 2:
        f = avail[0]
        if tile_base[f] + 512 <= S_graph:
            tile_chunks.append((f, avail[-1]))
            avail = avail[1:-1]
        else:
            tile_chunks.append((f,))
            avail = avail[1:]
    if avail:
        tile_chunks.append((avail[0],))
    cpg = len(tile_chunks)
    nUL = BL * cpg

    units = [(l, gg, tt) for l in range(L)
             for gg in range(BL) for tt in tile_chunks]
    nU = len(units)
    post = defaultdict(list)
    for l in range(L):
        for pair in ((0, 1), (2, 3)):
            s0 = l * nUL + (pair[1] + 1) * cpg - 1 + 2
            post[s0 + 1].append(mk_block_h(l, pair))
            post[s0 + 2].append(mk_block_tp(l, pair))
            post[s0 + 3].append(mk_block_norm(l, pair))
            post[s0 + 4].append(mk_block_next(l, pair))
    st1, st2 = {}, {}
    for step in range(nU + 7):
        if step < nU:
            l, gg, tt = units[step]
            st1[step] = emit_chunk_pre(l, gg, tt)
        if 0 <= step - 1 < nU:
            u = step - 1
            l, gg, tt = units[u]
            st2[u] = emit_chunk_p1(l, gg, tt, st1.pop(u))
        if 0 <= step - 2 < nU:
            u = step - 2
            l, gg, tt = units[u]
            emit_chunk_p2(l, gg, tt, st2.pop(u), TL[l]["msgs_max"])
        for fn in post.pop(step, []):
            fn()

    # ---- pooling + prediction MLP (ge_sum accumulated in block_norm)
    ge = pLN.tile([128, BL], F32R, name="ge", tag="ge")
    nc.vector.tensor_scalar(ge[:], ge_sum[:], bc_sb[:, 33:34],
                            bc_sb[:, 30 + L - 1:31 + L - 1],
                            op0=ALU.mult, op1=ALU.add)
    o1 = psA()
    nc.tensor.matmul(o1[:, 0:BL], WR("Wh1"), ge[:], start=True, stop=True)
    t1 = pLN.tile([128, BL], F32R, name="t1", tag="t1")
    nc.scalar.activation(t1[:], o1[:, 0:BL], AF.Relu, bias=bc_sb[:, 21:22])
    o2 = psB()
    nc.tensor.matmul(o2[:, 0:BL], WR("Wh2"), t1[:], start=True, stop=True)
    out_sb = pLN.tile([OUT, BL], F32, name="out_sb", tag="out_sb")
    nc.scalar.activation(out_sb[:], o2[:, 0:BL], AF.Identity, bias=bh2_sb[:])
    nc.sync.dma_start(d["d_out"].ap(), out_sb[:])
    ctx.close()


# --------------------------------------------------------------------------
# Entry point.
# --------------------------------------------------------------------------

def build(inputs):
    struct, percore = _prep(inputs)
    A = _weight_arrays(inputs)
    wbmap = A.pop("_wbmap")
    wrmap = A.pop("_wrmap")
    wbsplit = A.pop("_wbsplit")
    key = (struct["S_graph"], struct["n_tiles"],
           tuple(struct["groups"]), struct["tile_w"], struct["has_empty"])
    if key not in _CACHE:
        _CACHE[key] = _build_program(struct, wbmap, wrmap,
                                     A["wb"].shape[1], A["wr"].shape[1],
                                     wbsplit)
    nc = _CACHE[key]

    in_maps = []
    for c in range(M):
        im = dict(
            gpair=percore["Gpair"][c], soh=percore["SOH"][c],
            xoh=percore["XOH"][c],
            maskrow=percore["maskrow"][c:c + 1],
            negrow=percore["negrow"][c:c + 1],
        )
        for k, v in A.items():
            im[k] = v
        in_maps.append(im)
    return nc, in_maps, struct


def kernel(**inputs):
    from concourse import bass_utils
    nc, in_maps, struct = build(inputs)
    res = bass_utils.run_bass_kernel_spmd(nc, in_maps, core_ids=list(range(M)))
    out = np.zeros((B, OUT), np.float32)
    for c in range(M):
        out[c * BL:(c + 1) * BL] = res.results[c]["out"].T
    return out


# revision 46
# speedup vs baseline: 1.1923x; 1.0094x over previous
"""Trainium2 Bass kernel for nn_BaselineModel_74509092651544 (CLRS-style MPNN).

Strategy
--------
Data-parallel over graphs: 32 graphs -> 8 cores x 4 graphs.  The dense
[B,N,N,H] message tensor is never materialized: only the ~61k unique
(graph,src,dst) edge slots survive the masked max, so the message MLP runs
on a padded CSR slot layout.

v2 (this file) vs the earlier baseline:
  * fp8 DoubleRow matmuls for the gather stage: m1/m2 (quantized fp8e4m3)
    ride as a 2-k-tile stationary pair against an interleaved Gsrc/Gdst
    one-hot moving tensor; the bond term uses a 24+24 row split of the
    one-hot counts.  1 cycle/slot instead of 3 on the PE.
  * bf16 weights/activations everywhere precision allows (validated
    ~7e-3 rel err vs the 2e-2 gate).
  * h-matmuls and LayerNorm batched across the 4 graphs per layer
    (512-wide) instead of per-graph 128-wide (f32r <256 runs at 1/4 rate).
  * DP-optimized receiver grouping (minimizes padded slots + per-group
    DVE reduce overhead).
  * Software-pipelined slot loop (skew 2) with PSUM rings 3/3/2.
  * relu1 on ACT, relu2 split ACT/DVE, segmented max on DVE.
"""

import sys
import numpy as np

sys.path.insert(0, "/opt/trn_rl_repo")

B, N, H, L, E, OUT = 32, 128, 128, 3, 65536, 128
M = 8                 # NeuronCores
BL = B // M           # graphs per core
NEG = -1e9
EPS = 1e-5
AV, BV = 128, 16
ACT_RELU2_SHARE = 0.62   # fraction of relu2 ops on ACT (rest on DVE)

_CACHE = {}


# --------------------------------------------------------------------------
# Host preprocessing: pure integer / relayout work.
# --------------------------------------------------------------------------

def _ffd_pack(groups):
    sizes = [R * K for (_, R, K) in groups]
    order = np.argsort(-np.asarray(sizes), kind="stable")
    bins, place = [], [None] * len(groups)
    for gi in order:
        sz = sizes[gi]
        for t in range(len(bins)):
            if bins[t] + sz <= 512:
                place[gi] = (t, bins[t])
                bins[t] += sz
                break
        else:
            place[gi] = (len(bins), 0)
            bins.append(sz)
    return bins, place


def _dp_groups(Kp, c_slot, c_group, max_r=128):
    n = len(Kp)
    INF = float("inf")
    dp = [INF] * (n + 1)
    dp[n] = 0.0
    choice = [1] * (n + 1)
    for p in range(n - 1, -1, -1):
        K = int(Kp[p])
        mr = min(n - p, 512 // K, max_r)
        best, best_r = INF, 1
        for R in range(1, mr + 1):
            c = R * K * c_slot + c_group + dp[p + R]
            if c < best:
                best, best_r = c, R
        dp[p], choice[p] = best, best_r
    groups = []
    p = 0
    while p < n:
        R = choice[p]
        groups.append((p, R, int(Kp[p])))
        p += R
    return groups


def _fixed_groups(Kp, r0):
    groups, p, n = [], 0, len(Kp)
    while p < n:
        K = int(Kp[p])
        R = min(r0, n - p)
        while R * K > 512:
            R //= 2
        groups.append((p, R, K))
        p += R
    return groups


def _choose_groups(Kp):
    """Pick the candidate minimizing a per-graph-layer time proxy (ns)."""
    cands = [_fixed_groups(Kp, 16),
             _dp_groups(Kp, 4.0, 100.0),
             _dp_groups(Kp, 4.0, 170.0, 16),
             _dp_groups(Kp, 2.5, 170.0)]
    best, best_c = None, float("inf")
    for gs in cands:
        bins, _ = _ffd_pack(gs)
        S = sum(R * K for (_, R, K) in gs)
        c = S * 4.04 + len(gs) * 170.0 + len(bins) * 395.0
        if c < best_c:
            best, best_c = gs, c
    return best


def _prep(inputs):
    x = np.asarray(inputs["x"]).astype(np.int64)            # [B*N, 9]
    ea = np.asarray(inputs["edge_attr"]).astype(np.int64)   # [E, 3]
    ei = np.asarray(inputs["edge_index"]).astype(np.int64)  # [2, E]

    g = ei[0] // N
    s = ei[0] % N
    d = ei[1] % N
    key = (g * N + s) * N + d
    uniq, inv = np.unique(key, return_inverse=True)
    US = uniq.size
    ug = uniq // (N * N)
    us = (uniq // N) % N
    ud = uniq % N

    # bond one-hot counts per unique slot  [US, 48]
    oh48 = np.zeros((US, 48), np.float32)
    for c in range(3):
        np.add.at(oh48, (inv, ea[:, c] + 16 * c), 1.0)

    # unique in-degree per (graph, receiver)
    deg = np.zeros((B, N), np.int64)
    np.add.at(deg, (ug, ud), 1)

    # receiver relabeling: position p holds the p-th highest-degree receiver
    rho = np.argsort(-deg, axis=1, kind="stable")        # [B, N] pos -> orig
    rho_inv = np.argsort(rho, axis=1)                    # orig -> pos
    degS = -np.sort(-deg, axis=1)                        # [B, N] desc
    Kp = np.maximum(degS.max(axis=0), 1)                 # [N] non-increasing

    groups = _choose_groups(Kp)                          # (p0, R, K)
    tiles_used, place = _ffd_pack(groups)
    n_tiles = len(tiles_used)
    tile_w = [int(w) for w in tiles_used]
    tile_base = np.concatenate([[0], np.cumsum(tile_w)[:-1]]).astype(np.int64)
    S_graph = int(((sum(tile_w) + 15) // 16) * 16)
    S_core = BL * S_graph

    # per-position lookup tables
    col_base_of_pos = np.zeros(N, np.int64)   # first column of the receiver
    K_of_pos = np.zeros(N, np.int64)
    for gi, (p0, R, K) in enumerate(groups):
        t, off = place[gi]
        for r in range(R):
            col_base_of_pos[p0 + r] = tile_base[t] + off + r * K
            K_of_pos[p0 + r] = K

    # slots ordered by (g, d, s): contiguous per receiver
    order = np.lexsort((us, ud, ug))
    og, od, osl = ug[order], ud[order], order
    osrc = us[order]
    recv_id = og * N + od
    first = np.concatenate([[0], np.flatnonzero(np.diff(recv_id)) + 1])
    k_rank = np.arange(len(og)) - first[np.searchsorted(recv_id[first], recv_id)]

    pos = rho_inv[og, od]
    core_r = og // BL
    col_r = (og % BL) * S_graph + col_base_of_pos[pos] + k_rank

    # padding: receivers with deg < K duplicate their first slot
    fg, fd = og[first], od[first]
    fpos = rho_inv[fg, fd]
    fdeg = deg[fg, fd]
    fK = K_of_pos[fpos]
    padc = (fK - fdeg).astype(np.int64)
    assert (padc >= 0).all()
    rep = np.repeat(np.arange(len(first)), padc)
    kpad = np.arange(len(rep)) - np.repeat(
        np.concatenate([[0], np.cumsum(padc)[:-1]]), padc
    ) + np.repeat(fdeg, padc)
    pg = fg[rep]
    core_p = pg // BL
    col_p = (pg % BL) * S_graph + col_base_of_pos[fpos[rep]] + kpad
    slot_p = osl[first][rep]
    src_p = osrc[first][rep]

    a_core = np.concatenate([core_r, core_p])
    a_col = np.concatenate([col_r, col_p])
    a_slot = np.concatenate([osl, slot_p])
    a_srcnew = np.concatenate([rho_inv[og, osrc], rho_inv[pg, src_p]])
    a_dstpos = np.concatenate([pos, fpos[rep]])

    import ml_dtypes
    FP8 = ml_dtypes.float8_e4m3fn
    flat = a_core * S_core + a_col
    # column-interleaved src/dst one-hot k-tiles: [M, 128, S_core*2]
    # (column s occupies bytes [2s, 2s+1]: k=0 -> Gsrc, k=1 -> Gdst)
    Gpair = np.zeros((M * S_core, 2, 128), np.float32)
    Gpair[flat, 0, a_srcnew] = 1.0
    Gpair[flat, 1, a_dstpos] = 1.0
    Gpair = np.ascontiguousarray(
        Gpair.reshape(M, S_core, 2, 128).transpose(0, 3, 1, 2)
        .reshape(M, 128, 2 * S_core)).astype(FP8)
    # bond count k-tiles (rows 0:24 / 24:48), column-interleaved:
    # [M, 24, S_core*2]
    SOH = np.zeros((M * S_core, 48), np.float32)
    SOH[flat] = oh48[a_slot]
    assert float(SOH.max()) <= 16.0
    SOH = np.ascontiguousarray(
        SOH.reshape(M, S_core, 2, 24).transpose(0, 3, 1, 2)
        .reshape(M, 24, 2 * S_core)).astype(FP8)

    # atom one-hot per core: [M, 9, AV, BL*N] in relabeled node order
    gg = np.repeat(np.arange(B), N)
    pp = np.tile(np.arange(N), B)
    orig = gg * N + rho[gg, pp]                    # [B*N] column -> orig node
    XOH = np.zeros((M, 9, AV, BL * N), np.float32)
    mcol = np.tile(np.arange(BL * N), M)
    mcore = np.repeat(np.arange(M), BL * N)
    for c in range(9):
        XOH[mcore, c, x[orig, c], mcol] = 1.0
    # one flat [AV, 9*BL*N] tensor per core -> single DMA
    XOH = np.ascontiguousarray(
        XOH.transpose(0, 2, 1, 3).reshape(M, AV, 9 * BL * N)).astype(FP8)

    # empty receivers (deg==0) -> need NEG mask path
    empt = (deg == 0)
    has_empty = bool(empt.any())
    maskrow = np.ones((M, BL * N), np.float32)
    negrow = np.zeros((M, BL * N), np.float32)
    if has_empty:
        eg, en = np.nonzero(empt)
        epos = rho_inv[eg, en]
        maskrow[eg // BL, (eg % BL) * N + epos] = 0.0
        negrow[eg // BL, (eg % BL) * N + epos] = NEG

    struct = dict(
        S_graph=S_graph, S_core=S_core, n_tiles=n_tiles,
        groups=[(p0, R, K, place[gi][0], place[gi][1])
                for gi, (p0, R, K) in enumerate(groups)],
        tile_w=tuple(tile_w), tile_base=tuple(int(b) for b in tile_base),
        has_empty=has_empty,
    )
    percore = dict(Gpair=Gpair, SOH=SOH, XOH=XOH,
                   maskrow=maskrow, negrow=negrow)
    return struct, percore


def _weight_arrays(inputs):
    import ml_dtypes
    BF16 = ml_dtypes.bfloat16
    f32 = np.float32

    def blob(cols):
        wmap = {}
        off = 0
        for name, arr in cols:
            wmap[name] = (off, arr.shape[1])
            off += arr.shape[1]
        data = np.concatenate([a for _, a in cols], 1)
        return np.ascontiguousarray(data), wmap

    Wm1 = np.asarray(inputs["Wm1"], f32)
    Wm2 = np.asarray(inputs["Wm2"], f32)
    bond_T = np.asarray(inputs["bond_emb"], f32).reshape(48, H).T  # [128, 48]
    # chunk 1: everything layer-0's slot phase needs (DMA'd first)
    cols_b = [("bondT", bond_T),
              ("We_0", np.asarray(inputs["We"], f32)[0]),
              ("m12_0_0",
               np.concatenate([Wm1[0, 0:128], Wm2[0, 0:128]], 1)),
              ("Wp1_0", np.asarray(inputs["Wp1"], f32)[0]),
              ("Wp2_0", np.asarray(inputs["Wp2"], f32)[0])]
    wb_split = sum(a.shape[1] for _, a in cols_b)
    # chunk 2: the rest (lands before layer-0's h phase)
    cols_b.append(("Wo1_0_0", np.asarray(inputs["Wo1"], f32)[0, 0:128]))
    for l in range(1, L):
        cols_b.append((f"We_{l}", np.asarray(inputs["We"], f32)[l]))
        cols_b.append((f"m12_{l}_0",
                       np.concatenate([Wm1[l, 0:128], Wm2[l, 0:128]], 1)))
        cols_b.append((f"m12_{l}_1",
                       np.concatenate([Wm1[l, 128:256], Wm2[l, 128:256]], 1)))
        cols_b.append((f"Wp1_{l}", np.asarray(inputs["Wp1"], f32)[l]))
        cols_b.append((f"Wp2_{l}", np.asarray(inputs["Wp2"], f32)[l]))
        cols_b.append((f"Wo1_{l}_0", np.asarray(inputs["Wo1"], f32)[l, 0:128]))
        cols_b.append((f"Wo1_{l}_1", np.asarray(inputs["Wo1"], f32)[l, 128:256]))
    for l in range(L):
        cols_b.append((f"Wo2_{l}", np.asarray(inputs["Wo2"], f32)[l]))
    wb, wbmap = blob(cols_b)

    cols_r = [("Wh1", np.asarray(inputs["Wh1"], f32)),
              ("Wh2", np.asarray(inputs["Wh2"], f32)),
              ("idn", np.eye(128, dtype=f32))]
    wr, wrmap = blob(cols_r)

    A = {}
    A["wb"] = wb.astype(BF16)
    A["wr"] = wr
    A["_wbmap"] = wbmap
    A["_wrmap"] = wrmap
    A["_wbsplit"] = wb_split

    at = np.asarray(inputs["atom_emb"], f32).transpose(1, 0, 2).reshape(AV, 9 * H)
    A["atomb"] = np.ascontiguousarray(at).astype(BF16)

    # bias columns [128, 33]: 4 pre-terms x L, 2 o-terms x L, bh1, bh2, eps,
    # ln_s x L, ln_b x L, bp1 x L, bp2 x L (same layout as baseline)
    bc = np.zeros((H, 34), f32)
    bc[:, 26] = EPS
    bc[:, 33] = np.asarray(inputs["ln_s"], f32)[L - 1] / N
    bc[:, 27:30] = np.asarray(inputs["ln_s"], f32).T
    bc[:, 30:33] = np.asarray(inputs["ln_b"], f32).T
    for l in range(L):
        bc[:, 4 * l + 0] = np.asarray(inputs["bm1"], f32)[l]
        bc[:, 4 * l + 1] = np.asarray(inputs["bm2"], f32)[l]
        bc[:, 4 * l + 2] = np.asarray(inputs["be"], f32)[l]
        bc[:, 4 * l + 3] = np.asarray(inputs["bg"], f32)[l]
        bc[:, 12 + 2 * l + 0] = np.asarray(inputs["bo1"], f32)[l]
        bc[:, 12 + 2 * l + 1] = np.asarray(inputs["bo2"], f32)[l]
        bc[:, 18 + l] = np.asarray(inputs["bp1"], f32)[l]
        bc[:, 23 + l] = np.asarray(inputs["bp2"], f32)[l]
    bc[:, 21] = np.asarray(inputs["bh1"], f32)
    bc[:, 22] = np.asarray(inputs["bh2"], f32)[:H]
    A["bias_cols"] = bc
    A["bh2_full"] = np.ascontiguousarray(
        np.asarray(inputs["bh2"], f32).reshape(OUT, 1))
    bp2f = np.zeros((H, 4), f32)
    bp2f[:, :L] = np.asarray(inputs["bp2"], f32).T
    A["bp2f"] = bp2f.astype(BF16)
    return A


# --------------------------------------------------------------------------
# Bass program.
# --------------------------------------------------------------------------

def _build_program(struct, wbmap, wrmap, wbc, wrc, wbsplit):
    import concourse.bacc as bacc
    import concourse.mybir as mybir
    import concourse.tile as tile

    F32 = mybir.dt.float32
    nc = bacc.Bacc("TRN2", target_bir_lowering=False, debug=False)

    BF16 = mybir.dt.bfloat16
    FP8 = mybir.dt.float8e4
    F32R = mybir.dt.float32r
    S_core = struct["S_core"]
    d = {}
    d["d_gpair"] = nc.dram_tensor("gpair", [128, 2 * S_core], FP8,
                                  kind="ExternalInput")
    d["d_soh"] = nc.dram_tensor("soh", [24, 2 * S_core], FP8,
                                kind="ExternalInput")
    d["d_xoh"] = nc.dram_tensor("xoh", [AV, 9 * BL * N], FP8,
                                kind="ExternalInput")
    d["d_atomb"] = nc.dram_tensor("atomb", [AV, 9 * H], BF16,
                                  kind="ExternalInput")
    d["d_wb"] = nc.dram_tensor("wb", [128, wbc], BF16, kind="ExternalInput")
    d["d_wr"] = nc.dram_tensor("wr", [128, wrc], F32R, kind="ExternalInput")
    d["d_bc"] = nc.dram_tensor("bias_cols", [H, 34], F32, kind="ExternalInput")
    d["d_bh2"] = nc.dram_tensor("bh2_full", [OUT, 1], F32, kind="ExternalInput")
    d["d_bp2f"] = nc.dram_tensor("bp2f", [H, 4], BF16, kind="ExternalInput")
    d["d_mask"] = nc.dram_tensor("maskrow", [1, BL * N], F32,
                                 kind="ExternalInput")
    d["d_neg"] = nc.dram_tensor("negrow", [1, BL * N], F32,
                                kind="ExternalInput")
    d["d_out"] = nc.dram_tensor("out", [OUT, BL], F32, kind="ExternalOutput")

    with tile.TileContext(nc) as tc:
        _emit(tc, nc, d, struct, wbmap, wrmap, mybir, wbsplit)
    nc.compile()
    return nc


def _emit(tc, nc, d, struct, wbmap, wrmap, mybir, wbsplit):
    import contextlib
    from collections import defaultdict
    ctx = contextlib.ExitStack()
    F32 = mybir.dt.float32
    F32R = mybir.dt.float32r
    BF16 = mybir.dt.bfloat16
    FP8 = mybir.dt.float8e4
    AF = mybir.ActivationFunctionType
    ALU = mybir.AluOpType
    AX = mybir.AxisListType
    DR = mybir.MatmulPerfMode.DoubleRow

    S_graph = struct["S_graph"]
    S_core = struct["S_core"]
    n_tiles = struct["n_tiles"]
    groups = struct["groups"]
    tile_w = struct["tile_w"]
    tile_base = struct["tile_base"]
    has_empty = struct["has_empty"]

    pG = ctx.enter_context(tc.tile_pool(name="pG", bufs=1))
    pW = ctx.enter_context(tc.tile_pool(name="pW", bufs=1))
    pAct = ctx.enter_context(tc.tile_pool(name="pAct", bufs=3))
    pNM = ctx.enter_context(tc.tile_pool(name="pNM", bufs=1))
    pMB = ctx.enter_context(tc.tile_pool(name="pMB", bufs=2))
    pLN = ctx.enter_context(tc.tile_pool(name="pLN", bufs=1))
    pIn = ctx.enter_context(tc.tile_pool(name="pIn", bufs=2))
    ps_a = ctx.enter_context(tc.tile_pool(name="ps_a", bufs=2, space="PSUM"))
    ps_b = ctx.enter_context(tc.tile_pool(name="ps_b", bufs=2, space="PSUM"))
    ps_c = ctx.enter_context(tc.tile_pool(name="ps_c", bufs=2, space="PSUM"))

    def psA(dt=F32):
        """Double-bank tile: pre pairs (also m12/h_ps/nf_ps, half-used)."""
        return ps_a.tile([128, 1024], dt, name="psA", tag="psA")

    def psB(dt=F32):
        return ps_b.tile([128, 512], dt, name="psB", tag="psB")

    def psC(dt=F32):
        return ps_c.tile([128, 512], dt, name="psC", tag="psC")

    # ---- resident tiles
    gpair_sb = pG.tile([128, 2 * S_core], FP8, name="gpair_sb")
    soh_sb = pG.tile([24, 2 * S_core], FP8, name="soh_sb")

    def k2(tile_sb, c0, w):
        """[p, 2*(c0..c0+w)] interleaved slice -> [p, k=2, s=w] AP."""
        return tile_sb[:, 2 * c0:2 * (c0 + w)].rearrange(
            "p (s k) -> p k s", k=2)

    wbc = sum(w for _, w in wbmap.values())
    wb_sb = pW.tile([128, wbc], BF16, name="wb_sb")
    wr_sb = pW.tile([128, sum(w for _, w in wrmap.values())], F32R,
                    name="wr_sb")
    bc_sb = pW.tile([H, 34], F32, name="bc_sb")
    bh2_sb = pW.tile([OUT, 1], F32, name="bh2_sb")
    bp2f_sb = pW.tile([H, 4], BF16, name="bp2f_sb")
    atomb_sb = pW.tile([AV, 9 * H], BF16, name="atomb_sb")

    def WB(name):
        off, w = wbmap[name]
        return wb_sb[:, off:off + w]

    def WR(name):
        off, w = wrmap[name]
        return wr_sb[:, off:off + w]

    # ---- DMAs, warmup-critical first
    nc.sync.dma_start(wb_sb[:, 0:wbsplit], d["d_wb"].ap()[:, 0:wbsplit])
    nc.sync.dma_start(bc_sb[:], d["d_bc"].ap())
    nc.sync.dma_start(atomb_sb[:], d["d_atomb"].ap())
    xoh_all = pIn.tile([AV, 9 * BL * N], FP8, name="xoh_all", tag="xoh",
                       bufs=1)
    c3 = 3 * BL * N
    nc.sync.dma_start(xoh_all[:, 0:c3], d["d_xoh"].ap()[:, 0:c3])
    nc.sync.dma_start(xoh_all[:, c3:], d["d_xoh"].ap()[:, c3:])
    sl = slice(0, 2 * S_graph)
    nc.sync.dma_start(gpair_sb[:, sl], d["d_gpair"].ap()[:, sl])
    nc.sync.dma_start(soh_sb[:, sl], d["d_soh"].ap()[:, sl])
    slr = slice(2 * S_graph, 2 * BL * S_graph)
    nc.sync.dma_start(gpair_sb[:, slr], d["d_gpair"].ap()[:, slr])
    nc.sync.dma_start(soh_sb[:, slr], d["d_soh"].ap()[:, slr])
    nc.sync.dma_start(wb_sb[:, wbsplit:], d["d_wb"].ap()[:, wbsplit:])
    nc.sync.dma_start(wr_sb[:], d["d_wr"].ap())
    nc.sync.dma_start(bp2f_sb[:], d["d_bp2f"].ap())
    nc.sync.dma_start(bh2_sb[:], d["d_bh2"].ap())
    if has_empty:
        mrow_sb = pW.tile([1, BL * N], F32, name="mrow_sb")
        nc.sync.dma_start(mrow_sb[:], d["d_mask"].ap())
        nrow_sb = pW.tile([1, BL * N], F32, name="nrow_sb")
        nc.sync.dma_start(nrow_sb[:], d["d_neg"].ap())
        mask_bc = pW.tile([128, BL * N], F32, name="mask_bc")
        nc.gpsimd.partition_broadcast(mask_bc[:], mrow_sb[:])
        neg_bc = pW.tile([128, BL * N], F32, name="neg_bc")
        nc.gpsimd.partition_broadcast(neg_bc[:], nrow_sb[:])

    # pin the activation table to the set that has Relu+Copy+Identity+Sqrt
    sq_dummy = pW.tile([128, 1], F32, name="sq_dummy")
    nc.scalar.activation(sq_dummy[:], bc_sb[:, 26:27], AF.Sqrt)

    # bias prework (bc only)
    bias_pre = pW.tile([128, L], F32, name="bias_pre")
    nc.vector.tensor_reduce(
        bias_pre[:], bc_sb[:, 0:4 * L].rearrange("p (l f) -> p l f", l=L),
        axis=AX.X, op=ALU.add)
    bo12 = pW.tile([128, L], F32, name="bo12")
    nc.vector.tensor_reduce(
        bo12[:], bc_sb[:, 12:12 + 2 * L].rearrange("p (l f) -> p l f", l=L),
        axis=AX.X, op=ALU.add)

    # ---- node features (feature-major), bf16
    nf_ps = psA()
    for c in range(9):
        nc.tensor.matmul(nf_ps[:, 0:512], atomb_sb[:, c * H:(c + 1) * H],
                         xoh_all[:, c * BL * N:(c + 1) * BL * N],
                         start=(c == 0), stop=(c == 8))
    nf = pNM.tile([128, BL * N], BF16, name="nf")
    nc.scalar.activation(nf[:], nf_ps[:, 0:512], AF.Copy)

    # layer-0 m12 (z = [nf, 0]) + fp8 copies, and layer-0 bond weights
    def emit_bw(l):
        bw_ps = psB()
        nc.tensor.matmul(bw_ps[0:24, 0:128], WB("bondT")[:, 0:24],
                         WB(f"We_{l}"), start=True, stop=True)
        nc.tensor.matmul(bw_ps[0:24, 128:256], WB("bondT")[:, 24:48],
                         WB(f"We_{l}"), start=True, stop=True,
                         skip_group_check=True)
        bw24 = pMB.tile([24, 256], FP8, name="bw24", tag=f"bw24_{l}", bufs=1)
        nc.scalar.activation(bw24[:], bw_ps[0:24, 0:256], AF.Copy)
        return bw24

    bw24_l = {0: emit_bw(0)}

    mt_l = {0: pMB.tile([128, BL * 256], FP8, name="mt0", tag="mt", bufs=2)}
    for half in range(2):
        ps = psA()
        for gg in (2 * half, 2 * half + 1):
            off = (gg % 2) * 256
            gsl = slice(gg * N, (gg + 1) * N)
            nc.tensor.matmul(ps[:, off:off + 256], nf[:, gsl],
                             WB("m12_0_0"), start=True, stop=True)
        nc.scalar.activation(mt_l[0][:, half * 512:(half + 1) * 512],
                             ps[:, 0:512], AF.Copy)

    # relu2 engine split (Bresenham across all layers)
    state = dict(relu2_acc=0.0)

    def emit_chunk_pre(l, gg, tt):
        pre = psA()
        mt_g = mt_l[l][:, gg * 256:(gg + 1) * 256].rearrange(
            "p (k f) -> p k f", k=2)
        bw_k = bw24_l[l][:].rearrange("p (k f) -> p k f", k=2)
        for i, t in enumerate(tt):
            w = tile_w[t]
            if i + 1 < len(tt):
                # widen to fill the 512-col half: the paired relu below must
                # not read unwritten PSUM (extra gather cols are in-bounds;
                # their relu output lands in unread slots of msgs1)
                assert tile_base[t] + 512 <= S_graph
                w = 512
            c0 = gg * S_graph + tile_base[t]
            sl = slice(i * 512, i * 512 + w)
            nc.tensor.matmul(pre[:, sl], mt_g, k2(gpair_sb, c0, w),
                             start=True, stop=False, perf_mode=DR)
            nc.tensor.matmul(pre[:, sl], bw_k, k2(soh_sb, c0, w),
                             start=False, stop=True, perf_mode=DR)
        wtot = 512 * (len(tt) - 1) + tile_w[tt[-1]]
        msgs1 = pAct.tile([128, 1024], BF16, name="msgs1", tag="msgs1",
                          bufs=2)
        nc.scalar.activation(msgs1[:, 0:wtot], pre[:, 0:wtot], AF.Relu,
                             bias=bias_pre[:, l:l + 1])
        return msgs1

    def emit_chunk_p1(l, gg, tt, msgs1):
        outs = []
        for i, t in enumerate(tt):
            w = tile_w[t]
            p1 = psB()
            nc.tensor.matmul(p1[:, 0:w], WB(f"Wp1_{l}"),
                             msgs1[:, i * 512:i * 512 + w],
                             start=True, stop=True)
            msgs2 = pAct.tile([128, 512], BF16, name="msgs2", tag="msgs2",
                              bufs=3)
            state["relu2_acc"] += ACT_RELU2_SHARE
            if state["relu2_acc"] >= 1.0:
                state["relu2_acc"] -= 1.0
                nc.scalar.activation(msgs2[:, 0:w], p1[:, 0:w], AF.Relu,
                                     bias=bc_sb[:, 18 + l:19 + l])
            else:
                nc.vector.tensor_scalar(msgs2[:, 0:w], p1[:, 0:w],
                                        bc_sb[:, 18 + l:19 + l], 0.0,
                                        op0=ALU.add, op1=ALU.max)
            outs.append(msgs2)
        return outs

    def emit_chunk_p2(l, gg, tt, msgs2s, msgs_max):
        for i, t in enumerate(tt):
            w = tile_w[t]
            p2 = psC()
            nc.tensor.matmul(p2[:, 0:w], WB(f"Wp2_{l}"), msgs2s[i][:, 0:w],
                             start=True, stop=True)
            for (p0, R, K, gt, off) in groups:
                if gt != t:
                    continue
                nc.vector.tensor_reduce(
                    msgs_max[:, gg * N + p0: gg * N + p0 + R],
                    p2[:, off:off + R * K].rearrange("p (r k) -> p r k", r=R),
                    axis=AX.X, op=ALU.max)

    bias_h_l = {}
    ge_sum = pLN.tile([128, BL], F32, name="ge_sum", tag="ge_sum")

    hid = {0: None}
    TL = {}
    for l in range(L):
        TL[l] = dict(
            msgs_max=pLN.tile([128, BL * N], BF16, name="msgs_max",
                              tag="msgs_max", bufs=2),
            hn=pLN.tile([128, BL * N], F32, name="hn", tag="hn", bufs=1),
            hsq=pLN.tile([128, BL * N], BF16, name="hsq", tag="hsq", bufs=1),
            sumh=pLN.tile([128, BL], F32, name="sumh", tag="sumh"),
            sumsq=pLN.tile([128, BL], F32, name="sumsq", tag="sumsq"),
            negmean=pLN.tile([128, BL], F32, name="negmean", tag="negmean"),
            msq=pLN.tile([128, BL], F32, name="msq", tag="msq"),
            var=pLN.tile([128, BL], F32, name="var", tag="var"),
            std=pLN.tile([128, BL], F32, name="std", tag="std"),
            rstd=pLN.tile([128, BL], F32, name="rstd", tag="rstd"),
            h_fm=pLN.tile([128, BL * N], F32R, name="h_fm", tag="h_fm",
                          bufs=1),
            hid_new=(pNM.tile([128, BL * N], BF16, name=f"hid{l + 1}",
                              tag=f"hid{(l + 1) % 2}")
                     if l + 1 < L else None),
        )
        hid[l + 1] = TL[l]["hid_new"]

    def mk_block_h(l, pair):
        def fn():
            t = TL[l]
            msgs_max = t["msgs_max"]
            if l not in bias_h_l:
                if has_empty:
                    bias_h_l[l] = bo12[:, l:l + 1]
                else:
                    bh_ps = psC()
                    nc.tensor.matmul(bh_ps[:, 0:2], WB(f"Wo2_{l}"),
                                     bp2f_sb[:, l:l + 2],
                                     start=True, stop=True)
                    bh = pMB.tile([128, 1], F32, name="bias_h",
                                  tag=f"bias_h{l}", bufs=1)
                    nc.vector.tensor_tensor(bh[:], bh_ps[:, 0:1],
                                            bo12[:, l:l + 1], op=ALU.add)
                    bias_h_l[l] = bh[:]
            msgs_src = msgs_max
            if has_empty:
                mmf = pLN.tile([128, BL * N], F32, name="mmf", tag="mmf",
                               bufs=1)
                nc.vector.scalar_tensor_tensor(
                    mmf[:], msgs_max[:], bc_sb[:, 23 + l:24 + l],
                    mask_bc[:], op0=ALU.add, op1=ALU.mult)
                nc.vector.tensor_tensor(mmf[:], mmf[:], neg_bc[:],
                                        op=ALU.add)
                mmb = pLN.tile([128, BL * N], BF16, name="mmb", tag="mmb",
                               bufs=1)
                nc.scalar.activation(mmb[:], mmf[:], AF.Copy)
                msgs_src = mmb
            g0 = pair[0]
            psl = slice(g0 * N, (g0 + 2) * N)
            h_ps = psA()
            nc.tensor.matmul(h_ps[:, 0:256], WB(f"Wo1_{l}_0"),
                             nf[:, psl], start=True, stop=False)
            if l > 0:
                nc.tensor.matmul(h_ps[:, 0:256], WB(f"Wo1_{l}_1"),
                                 hid[l][:, psl], start=False, stop=False)
            nc.tensor.matmul(h_ps[:, 0:256], WB(f"Wo2_{l}"),
                             msgs_src[:, psl], start=False, stop=True)
            nc.scalar.activation(t["h_fm"][:, psl], h_ps[:, 0:256], AF.Relu,
                                 bias=bias_h_l[l])
        return fn

    def mk_block_tp(l, pair):
        def fn():
            t = TL[l]
            hn, hsq = t["hn"], t["hsq"]
            g0 = pair[0]
            psl = slice(g0 * N, (g0 + 2) * N)
            ssl = slice(g0, g0 + 2)
            tp_ps = psB(F32R)
            for i, gg in enumerate(pair):
                nc.tensor.transpose(
                    tp_ps[:, i * 128:(i + 1) * 128],
                    t["h_fm"][:, gg * N:(gg + 1) * N], WR("idn"))
            nc.vector.tensor_scalar(hn[:, psl],
                                    tp_ps[:, 0:256].bitcast(F32),
                                    0.0, None, op0=ALU.add)
            nc.vector.tensor_reduce(
                t["sumh"][:, ssl],
                hn[:, psl].rearrange("p (g n) -> p g n", g=2),
                axis=AX.X, op=ALU.add)
            nc.vector.scalar_tensor_tensor(
                hsq[:, psl], hn[:, psl], 0.0, hn[:, psl],
                op0=ALU.add, op1=ALU.mult)
            nc.vector.tensor_reduce(
                t["sumsq"][:, ssl],
                hsq[:, psl].rearrange("p (g n) -> p g n", g=2),
                axis=AX.X, op=ALU.add)
            nc.gpsimd.tensor_scalar(t["negmean"][:, ssl], t["sumh"][:, ssl],
                                    -1.0 / H, None, op0=ALU.mult)
            nc.vector.tensor_tensor(t["msq"][:, ssl], t["negmean"][:, ssl],
                                    t["negmean"][:, ssl], op=ALU.mult)
            nc.vector.scalar_tensor_tensor(
                t["var"][:, ssl], t["sumsq"][:, ssl], 1.0 / H,
                t["msq"][:, ssl], op0=ALU.mult, op1=ALU.subtract)
            nc.scalar.activation(t["std"][:, ssl], t["var"][:, ssl], AF.Sqrt,
                                 bias=bc_sb[:, 26:27])
            nc.vector.reciprocal(t["rstd"][:, ssl], t["std"][:, ssl])
        return fn

    def mk_block_norm(l, pair):
        def fn():
            t = TL[l]
            g0 = pair[0]
            psl = slice(g0 * N, (g0 + 2) * N)
            tp2_ps = psC(F32R)
            for i, gg in enumerate(pair):
                hnorm = pLN.tile([128, 128], F32R, name="hnorm",
                                 tag="hnorm", bufs=2)
                nc.gpsimd.tensor_scalar(hnorm[:],
                                        t["hn"][:, gg * N:(gg + 1) * N],
                                        t["negmean"][:, gg:gg + 1],
                                        t["rstd"][:, gg:gg + 1],
                                        op0=ALU.add, op1=ALU.mult)
                nc.tensor.transpose(tp2_ps[:, i * 128:(i + 1) * 128],
                                    hnorm[:], WR("idn"))
            if l + 1 < L:
                nc.vector.tensor_scalar(t["hid_new"][:, psl],
                                        tp2_ps[:, 0:256].bitcast(F32),
                                        bc_sb[:, 27 + l:28 + l],
                                        bc_sb[:, 30 + l:31 + l],
                                        op0=ALU.mult, op1=ALU.add)
            else:
                # pool directly: ge += mean_n(tp2) * ln_s + ln_b (per pair)
                nc.vector.tensor_reduce(
                    ge_sum[:, g0:g0 + 2],
                    tp2_ps[:, 0:256].bitcast(F32).rearrange(
                        "p (g n) -> p g n", g=2),
                    axis=AX.X, op=ALU.add)
        return fn

    def mk_block_next(l, pair):
        def fn():
            nl = l + 1
            if nl >= L:
                return
            if nl not in bw24_l:
                bw24_l[nl] = emit_bw(nl)
                mt_l[nl] = pMB.tile([128, BL * 256], FP8,
                                    name=f"mt{nl}", tag="mt", bufs=2)
            ps = psA()
            for i, gg in enumerate(pair):
                off = i * 256
                gsl = slice(gg * N, (gg + 1) * N)
                nc.tensor.matmul(ps[:, off:off + 256], nf[:, gsl],
                                 WB(f"m12_{nl}_0"), start=True, stop=False)
                nc.tensor.matmul(ps[:, off:off + 256], hid[nl][:, gsl],
                                 WB(f"m12_{nl}_1"), start=False, stop=True)
            g0 = pair[0]
            nc.scalar.activation(
                mt_l[nl][:, g0 * 256:(g0 + 2) * 256], ps[:, 0:512],
                AF.Copy)
        return fn

    # chunk construction: pair widest (widened to its full bank) with
    # narrowest (its true width bounds the relu read)
    order_t = sorted(range(n_tiles), key=lambda t: -tile_w[t])
    tile_chunks = []
    avail = list(order_t)
    while len(avail) >= 2:
        f = avail[0]
        if tile_base[f] + 512 <= S_graph:
            tile_chunks.append((f, avail[-1]))
            avail = avail[1:-1]
        else:
            tile_chunks.append((f,))
            avail = avail[1:]
    if avail:
        tile_chunks.append((avail[0],))
    cpg = len(tile_chunks)
    nUL = BL * cpg

    units = [(l, gg, tt) for l in range(L)
             for gg in range(BL) for tt in tile_chunks]
    nU = len(units)
    post = defaultdict(list)
    for l in range(L):
        for pair in ((0, 1), (2, 3)):
            s0 = l * nUL + (pair[1] + 1) * cpg - 1 + 2
            post[s0 + 1].append(mk_block_h(l, pair))
            post[s0 + 2].append(mk_block_tp(l, pair))
            post[s0 + 3].append(mk_block_norm(l, pair))
            post[s0 + 4].append(mk_block_next(l, pair))
    st1, st2 = {}, {}
    for step in range(nU + 7):
        if step < nU:
            l, gg, tt = units[step]
            st1[step] = emit_chunk_pre(l, gg, tt)
        if 0 <= step - 1 < nU:
            u = step - 1
            l, gg, tt = units[u]
            st2[u] = emit_chunk_p1(l, gg, tt, st1.pop(u))
        if 0 <= step - 2 < nU:
            u = step - 2
            l, gg, tt = units[u]
            emit_chunk_p2(l, gg, tt, st2.pop(u), TL[l]["msgs_max"])
        for fn in post.pop(step, []):
            fn()

    # ---- pooling + prediction MLP (ge_sum accumulated in block_norm)
    ge = pLN.tile([128, BL], F32R, name="ge", tag="ge")
    nc.vector.tensor_scalar(ge[:], ge_sum[:], bc_sb[:, 33:34],
                            bc_sb[:, 30 + L - 1:31 + L - 1],
                            op0=ALU.mult, op1=ALU.add)
    o1 = psA()
    nc.tensor.matmul(o1[:, 0:BL], WR("Wh1"), ge[:], start=True, stop=True)
    t1 = pLN.tile([128, BL], F32R, name="t1", tag="t1")
    nc.scalar.activation(t1[:], o1[:, 0:BL], AF.Relu, bias=bc_sb[:, 21:22])
    o2 = psB()
    nc.tensor.matmul(o2[:, 0:BL], WR("Wh2"), t1[:], start=True, stop=True)
    out_sb = pLN.tile([OUT, BL], F32, name="out_sb", tag="out_sb")
    nc.scalar.activation(out_sb[:], o2[:, 0:BL], AF.Identity, bias=bh2_sb[:])
    nc.sync.dma_start(d["d_out"].ap(), out_sb[:])
    ctx.close()


# --------------------------------------------------------------------------
# Entry point.
# --------------------------------------------------------------------------

def build(inputs):
    struct, percore = _prep(inputs)
    A = _weight_arrays(inputs)
    wbmap = A.pop("_wbmap")
    wrmap = A.pop("_wrmap")
    wbsplit = A.pop("_wbsplit")
    key = (struct["S_graph"], struct["n_tiles"],
           tuple(struct["groups"]), struct["tile_w"], struct["has_empty"])
    if key not in _CACHE:
        _CACHE[key] = _build_program(struct, wbmap, wrmap,
                                     A["wb"].shape[1], A["wr"].shape[1],
                                     wbsplit)
    nc = _CACHE[key]

    in_maps = []
    for c in range(M):
        im = dict(
            gpair=percore["Gpair"][c], soh=percore["SOH"][c],
            xoh=percore["XOH"][c],
            maskrow=percore["maskrow"][c:c + 1],
            negrow=percore["negrow"][c:c + 1],
        )
        for k, v in A.items():
            im[k] = v
        in_maps.append(im)
    return nc, in_maps, struct


def kernel(**inputs):
    from concourse import bass_utils
    nc, in_maps, struct = build(inputs)
    res = bass_utils.run_bass_kernel_spmd(nc, in_maps, core_ids=list(range(M)))
    out = np.zeros((B, OUT), np.float32)
    for c in range(M):
        out[c * BL:(c + 1) * BL] = res.results[c]["out"].T
    return out
